# revision 11
# baseline (speedup 1.0000x reference)
"""AlignmentModule on 8 Trainium2 cores - fp8 DoubleRow rewrite (90958 ns est).

Data-parallel over batch (2 per core). The heavy matmuls run as fp8e4 DoubleRow
(2 K-planes of 128 per pass, 0.5 cycles/row = 4x fp32r): conv stacks t1/t2/f2,
the Gram matmul H = G.ft2, the text-side u = W3^T.tx2m (the 1x1 f3 conv is
algebraically absorbed into the 4x-smaller text side; f2 row norms come from
fh = ft2*H read straight off H's PSUM), and the score cross ft2^T.u. The f1
conv (K=80) and the K=1 rank-2 augment (f2[f] + (t2[t]-c0) via f2q/ones x
ones/t2q) use plain fp8 matmuls: DoubleRow with overlapping tap-pair APs,
sub-128 contraction, or width-1 stationary wedges the exec unit on HW
(NRT_EXEC_UNIT_UNRECOVERABLE) even when walrus accepts it.

Row norms: t2row/f2row are ones-weight matmuls into partition-0 row slots of
the just-consumed t2/H PSUM tiles (no extra banks), evicted straight into the
partition-0 aug operand rows. Tail per f-tile pair: ACT sqrt (psum pair ->
fp16 dist), ACT exp(14-dist) with f32 row-accum (pair 0 instead paired-exp +
DVE reduce), one ACT ln per 8 tiles, outp = (prior_f16 - cc) - dist on DVE
(GPSIMD supports neither PSUM access nor TensorScalarPtr on HW), DMA out in
4-tile quads. The two batches' step lists are software-pipelined (batch 1
staggered 24 steps) so batch-0's ACT-heavy tail overlaps batch-1's
DVE-eviction-heavy convs.

Host pre-quantizes inputs/weights to fp8e4 (power-of-2 scales, ranges
asserted) shipped as uint8 bits, plus an fp16 prior. Measured on HW:
rel err 6.5e-4 vs the fp32 reference (fp8 conv chain + fp16 prior), vs
the 2e-2 gate. TimelineSim estimate 90958 ns vs 103596 ns for the fp32r
baseline.
"""

import numpy as np

import bass_rust as _bass_rust
import concourse.bass as bass
import concourse.mybir as mybir
from concourse.tile import TileContext

F32 = mybir.dt.float32
F16 = mybir.dt.float16
BF16 = mybir.dt.bfloat16
F8 = mybir.dt.float8e4
U8 = mybir.dt.uint8
AF = mybir.ActivationFunctionType
OP = mybir.AluOpType
AX = mybir.AxisListType
DR = mybir.MatmulPerfMode.DoubleRow

B, T_TEXT, T_FEATS, ADIM, ODIM = 16, 512, 2048, 256, 80
N_CORES = 8
B_LOC = B // N_CORES
TT, TF = T_TEXT, T_FEATS
NT = TF // 512   # 4 feats chunks of 512
NF = TF // 128   # 16 f-tiles per batch

# ---- fixed power-of-2 scales (validated in opt/sim_numerics2.py) ----
SX = 16.0
S_TW1, S_TW2 = 1024.0, 512.0
S_FW1, S_FW2, S_W3 = 512.0, 1024.0, 512.0
S_G = 64.0
A1 = 1.0 / 512.0       # tx1 evict; tile = 32*true
A2 = 1.0 / 2048.0      # tx2m evict; tile = 8*true
AF1 = 1.0 / 512.0      # ft1 tile = 16*true
AF2 = 1.0 / 8192.0     # ft2 tile = 2*true
BH = 1.0 / 32.0        # fh evict scalar
BU = 1.0 / 4096.0      # u evict; u tile = -1*true(W3^T tx2)
ONES2_F2 = 1.0 / 8.0   # f2row ones-weight = 1/(S_G*s_ft2^2*BH)
T2Q_MUL = 1.0 / 64.0   # t2row evict mult = 1/s_tx2m^2
C0 = 192.0             # t2q offset; sqrt bias adds it back
M_SHIFT = 14.0
SQ_BIAS = C0           # dist = sqrt(psum + C0)

PAIRED_SET = (0,)  # score pairs using paired-exp + DVE reduce
# engine homes for evictions: "dve" | "pool" | "act"(relu/identity/square ok)
# NOTE: GPSIMD (pool) cannot access PSUM on HW — psum-evictions are dve/act only.
HOMES = {
    "t1e": "act", "t2e": "act", "ue": "act",
    "f1e": "dve", "f2e": "dve", "fh": "dve",
    "rows": "act", "t2q": "dve", "txsq": "act",
    "outp": "dve",  # pool cannot do TensorScalarPtr on HW
}

WOFF = {}              # wblob free-dim offsets, filled by _pack_weights layout
WBLOB_W = 1536 + 512 + 1024 + 1536 + 512 + 512  # tw1,tw2,fw1,fw2,G,w3u


def _wblob_offsets():
    off, o = {}, 0
    for name, w in (("tw1", 1536), ("tw2", 512), ("fw1", 1024),
                    ("fw2", 1536), ("G", 512), ("w3u", 512)):
        off[name] = o
        o += w
    assert o == WBLOB_W
    return off


WOFF = _wblob_offsets()


def _split_excess_waits(nc, limit=1):
    """walrus CoreV3 CTRL codegen rejects >1 sync-wait per instruction.
    Hoist excess waits onto preceding NOPs on the same engine."""
    ctr = 0
    for f in nc.m.functions:
        for bb in f.blocks:
            insts = bb.instructions
            idx = 0
            while idx < len(insts):
                ins = insts[idx]
                si = ins.sync_info
                if si is not None and len(si.on_wait) > limit:
                    waits = list(si.on_wait)
                    extra, keep = waits[:-limit], waits[-limit:]
                    si.on_wait = keep
                    pos = idx
                    for j in range(0, len(extra), limit):
                        nop = mybir.InstNoOp(name=f"waitsplit_{ctr}", ins=[], outs=[])
                        ctr += 1
                        nop.engine = ins.engine
                        nop.sync_info = mybir.SyncInfo(
                            on_wait=extra[j : j + limit], on_update=[]
                        )
                        insts.insert(pos, nop)
                        pos += 1
                        idx += 1
                idx += 1
    return ctr


def _beta_binomial_prior():
    from scipy.special import gammaln

    T, N = T_FEATS, T_TEXT
    a = np.arange(1, T + 1, dtype=np.float64)[:, None]
    b = (T - np.arange(1, T + 1, dtype=np.float64) + 1.0)[:, None]
    k = np.arange(N, dtype=np.float64)[None, :]
    n = float(N)

    def betaln(x, y):
        return gammaln(x) + gammaln(y) - gammaln(x + y)

    logp = (
        gammaln(n + 1.0) - gammaln(k + 1.0) - gammaln(n - k + 1.0)
        + betaln(k + a, n - k + b) - betaln(a, b)
    )
    return logp.astype(np.float32)


def _build_nc():
    nc = bass.Bass(name="alignment")

    textT = nc.dram_tensor("textT", [B_LOC, ADIM, TT], U8, kind="ExternalInput")
    featsT = nc.dram_tensor("featsT", [B_LOC, ODIM, TF], U8, kind="ExternalInput")
    wblob = nc.dram_tensor("wblob", [128, WBLOB_W], U8, kind="ExternalInput")
    onesrow = nc.dram_tensor("onesrow", [1, TF], U8, kind="ExternalInput")
    priorD = nc.dram_tensor("prior", [TF, TT], F16, kind="ExternalInput")
    outD = nc.dram_tensor("out", [B_LOC, TF, TT], F32, kind="ExternalOutput")

    with TileContext(nc) as tc:
        with (
            tc.tile_pool(name="const", bufs=1) as const,
            tc.tile_pool(name="inp", bufs=2) as inp,
            tc.tile_pool(name="actp", bufs=2) as actp,
            tc.tile_pool(name="rowp", bufs=2) as rowp,
            tc.tile_pool(name="distp", bufs=17) as distp,
            tc.tile_pool(name="ep", bufs=3) as ep,
            tc.tile_pool(name="outq", bufs=3) as outqp,
            tc.tile_pool(name="ppA", bufs=2, space="PSUM") as ppA,
            tc.tile_pool(name="ppS", bufs=2, space="PSUM") as ppS,
        ):
            # ---- constants / weights ----
            wb = const.tile([128, WBLOB_W], F8)
            nc.sync.dma_start(out=wb[:].bitcast(U8), in_=wblob[:])
            ones1 = const.tile([128, 1], F8)
            nc.vector.memset(ones1[:], ONES2_F2)
            onesb = const.tile([128, 1], BF16)
            nc.vector.memset(onesb[:], 1.0)
            b_sq = const.tile([128, 1], F32)
            nc.vector.memset(b_sq[:], SQ_BIAS)
            b_m = const.tile([128, 1], F32)
            nc.vector.memset(b_m[:], M_SHIFT)
            prior_sb = const.tile([128, NF, TT], F16)

            def ev_scale_relu(home, out, ps, scale):
                if home == "act":
                    nc.scalar.activation(out, ps, AF.Relu, scale=scale)
                elif home == "split":
                    nc.vector.tensor_scalar(out[:, 0, :], ps[:, 0, :], scale,
                                            0.0, OP.mult, OP.max)
                    nc.gpsimd.tensor_scalar(out[:, 1, :], ps[:, 1, :], scale,
                                            0.0, OP.mult, OP.max)
                else:
                    eng = nc.vector if home == "dve" else nc.gpsimd
                    eng.tensor_scalar(out, ps, scale, 0.0, OP.mult, OP.max)

            def ev_scale(home, out, ps, scale):
                if home == "act":
                    nc.scalar.activation(out, ps, AF.Identity, scale=scale)
                elif home == "split":
                    nc.vector.tensor_scalar(out[:, 0, :], ps[:, 0, :], scale,
                                            None, OP.mult)
                    nc.gpsimd.tensor_scalar(out[:, 1, :], ps[:, 1, :], scale,
                                            None, OP.mult)
                else:
                    eng = nc.vector if home == "dve" else nc.gpsimd
                    eng.tensor_scalar(out, ps, scale, None, OP.mult)

            def wap(name, idx, planes=2, width=128):
                base = WOFF[name] + idx * planes * width
                return wb[:, base : base + planes * width].rearrange(
                    "p (c w) -> p c w", c=planes
                )

            def load_tx0(b):
                tx0 = inp.tile([128, 2, TT + 2], F8, tag="tx0")
                nc.vector.memset(tx0[:, :, 0:1], 0.0)
                nc.vector.memset(tx0[:, :, TT + 1 : TT + 2], 0.0)
                nc.sync.dma_start(
                    out=tx0[:, :, 1 : TT + 1].bitcast(U8),
                    in_=textT[b].rearrange("(c p) t -> p c t", p=128),
                )
                return tx0

            def load_ft0(b):
                ft0 = inp.tile([ODIM, TF + 3], F8, tag="ft0")
                nc.vector.memset(ft0[:, 0:1], 0.0)
                nc.vector.memset(ft0[:, TF + 1 : TF + 3], 0.0)
                nc.sync.dma_start(out=ft0[:, 1 : TF + 1].bitcast(U8), in_=featsT[b])
                return ft0

            def batch_ctx(b, tx0, ft0):
                """Allocate per-batch tiles and return the conv step list plus
                the tile handles the tail needs."""
                tx1 = actp.tile([128, 2, TT], F8, tag="tx1")
                tx2m = actp.tile([128, 2, TT], F8, tag="tx2m")
                txsq = actp.tile([128, 2, TT], BF16, tag="txsq")
                ft1 = actp.tile([128, 2, TF + 2], F8, tag="ft1")
                ft2 = actp.tile([128, 2, TF], F8, tag="ft2")
                fh = actp.tile([128, 2, TF], F8, tag="fh")
                u = actp.tile([128, 2, TT], F8, tag="u")
                augw = rowp.tile([1, 2, TF], F8, tag="augw")
                augx = rowp.tile([1, 2, TT], F8, tag="augx")

                box = {}
                steps = []

                def pads():
                    nc.vector.memset(ft1[:, :, 0:1], 0.0)
                    nc.vector.memset(ft1[:, :, TF + 1 : TF + 2], 0.0)
                    nc.sync.dma_start(out=augw[0:1, 1, :].bitcast(U8),
                                      in_=onesrow[0:1, :])
                    nc.sync.dma_start(out=augx[0:1, 0, :].bitcast(U8),
                                      in_=onesrow[0:1, 0:TT])

                def tap_pair(start):
                    a = ft0[0:ODIM, start : start + 512]
                    w = a.copy()
                    w.ap = _bass_rust.VecI64Pair([list(a.ap[0]), [1, 2], [1, 512]])
                    return w

                def f1_mm(n):
                    def f():
                        ps = ppA.tile([128, 2, 512], F32, tag="psA")
                        box[("f1", n)] = ps
                        for m in range(2):
                            for k in range(3):
                                d, pl = divmod(k, 2)
                                base = WOFF["fw1"] + (d * 2 + m) * 256 + pl * 128
                                lhs = wb[0:ODIM, base : base + 128]
                                nc.tensor.matmul(
                                    ps[:, m, :], lhs,
                                    ft0[0:ODIM, n * 512 + k : n * 512 + k + 512],
                                    start=(k == 0), stop=(k == 2),
                                )
                    return f

                def f1_ev(n):
                    def f():
                        ps = box.pop(("f1", n))
                        ev_scale_relu(HOMES["f1e"],
                                      ft1[:, :, 1 + n * 512 : 1 + (n + 1) * 512],
                                      ps[:, :, :], AF1)
                    return f

                def t1_mm():
                    ps = ppA.tile([128, 2, 512], F32, tag="psA")
                    box["t1"] = ps
                    for m in range(2):
                        for k in range(3):
                            nc.tensor.matmul(
                                ps[:, m, :], wap("tw1", k * 2 + m),
                                tx0[:, :, k : k + TT],
                                start=(k == 0), stop=(k == 2), perf_mode=DR,
                            )

                def t1_ev():
                    ps = box.pop("t1")
                    ev_scale_relu(HOMES["t1e"], tx1[:, :, :], ps[:, :, :], A1)

                def t2_mm():
                    ps = ppA.tile([128, 2, 512], F32, tag="psA")
                    box["t2"] = ps
                    for m in range(2):
                        nc.tensor.matmul(
                            ps[:, m, :], wap("tw2", m), tx1[:, :, :],
                            start=True, stop=True, perf_mode=DR,
                        )

                def t2_ev():
                    ps = box["t2"]
                    ev_scale(HOMES["t2e"], tx2m[:, :, :], ps[:, :, :], A2)

                def txsq_f():
                    ps = box["t2"]
                    if HOMES["txsq"] == "act":
                        nc.scalar.activation(txsq[:, :, :], ps[:, :, :], AF.Square,
                                             scale=A2)
                    else:
                        eng = nc.vector if HOMES["txsq"] == "dve" else nc.gpsimd
                        eng.tensor_tensor(txsq[:, :, :], tx2m[:, :, :],
                                          tx2m[:, :, :], OP.mult)

                def t2row_mm():
                    psT = box["t2"]  # reuse t2 psum tile (already evicted)
                    for cc_ in range(2):
                        nc.tensor.matmul(psT[0:1, 0, :], onesb[:], txsq[:, cc_, :],
                                         start=(cc_ == 0), stop=(cc_ == 1))

                def t2q_ev():
                    psT = box.pop("t2")
                    _e = {"dve": nc.vector, "pool": nc.gpsimd}[HOMES["t2q"]]
                    _e.tensor_scalar(
                        augx[0:1, 1, :], psT[0:1, 0, :],
                        T2Q_MUL, C0, OP.mult, OP.subtract,
                    )

                def u_mm():
                    ps = ppA.tile([128, 2, 512], F32, tag="psA")
                    box["u"] = ps
                    for m in range(2):
                        nc.tensor.matmul(
                            ps[:, m, :], wap("w3u", m), tx2m[:, :, :],
                            start=True, stop=True, perf_mode=DR,
                        )

                def u_ev():
                    ps = box.pop("u")
                    ev_scale(HOMES["ue"], u[:, :, :], ps[:, :, :], -BU)

                def f2_mm(n):
                    def f():
                        ps = ppA.tile([128, 2, 512], F32, tag="psA")
                        box[("f2", n)] = ps
                        for m in range(2):
                            for k in range(3):
                                nc.tensor.matmul(
                                    ps[:, m, :], wap("fw2", k * 2 + m),
                                    ft1[:, :, n * 512 + k : n * 512 + k + 512],
                                    start=(k == 0), stop=(k == 2), perf_mode=DR,
                                )
                    return f

                def f2_ev(n):
                    def f():
                        ps = box.pop(("f2", n))
                        ev_scale_relu(HOMES["f2e"],
                                      ft2[:, :, n * 512 : (n + 1) * 512],
                                      ps[:, :, :], AF2)
                    return f

                def h_mm(n):
                    def f():
                        ps = ppA.tile([128, 2, 512], F32, tag="psA")
                        box[("h", n)] = ps
                        for m in range(2):
                            nc.tensor.matmul(
                                ps[:, m, :], wap("G", m),
                                ft2[:, :, n * 512 : (n + 1) * 512],
                                start=True, stop=True, perf_mode=DR,
                            )
                    return f

                def fh_ev(n):
                    def f():
                        ps = box[("h", n)]
                        sl = slice(n * 512, (n + 1) * 512)
                        if HOMES["fh"] == "split":
                            nc.vector.scalar_tensor_tensor(
                                fh[:, 0, sl], ps[:, 0, :], BH, ft2[:, 0, sl],
                                OP.mult, OP.mult)
                            nc.gpsimd.scalar_tensor_tensor(
                                fh[:, 1, sl], ps[:, 1, :], BH, ft2[:, 1, sl],
                                OP.mult, OP.mult)
                        else:
                            eng = nc.vector if HOMES["fh"] == "dve" else nc.gpsimd
                            eng.scalar_tensor_tensor(
                                fh[:, :, sl], ps[:, :, :], BH, ft2[:, :, sl],
                                OP.mult, OP.mult)
                    return f

                def f2row_mm(n):
                    def f():
                        psH = box[("h", n)]  # reuse after fh_ev consumed it
                        for c_ in range(2):
                            nc.tensor.matmul(
                                psH[0:1, 0, :], ones1[:, :],
                                fh[:, c_, n * 512 : (n + 1) * 512],
                                start=(c_ == 0), stop=(c_ == 1),
                            )
                    return f

                def f2row_ev(n):
                    def f():
                        psH = box.pop(("h", n))
                        ev_scale(HOMES["rows"],
                                 augw[0:1, 0, n * 512 : (n + 1) * 512],
                                 psH[0:1, 0, :], 1.0)
                    return f

                parts = dict(
                    pads=pads, f1_mm=f1_mm, f1_ev=f1_ev, t1_mm=t1_mm, t1_ev=t1_ev,
                    t2_mm=t2_mm, t2_ev=t2_ev, txsq=txsq_f, t2row=t2row_mm,
                    t2q=t2q_ev, u_mm=u_mm, u_ev=u_ev, f2_mm=f2_mm, f2_ev=f2_ev,
                    h_mm=h_mm, fh_ev=fh_ev, f2row_mm=f2row_mm, f2row_ev=f2row_ev,
                )
                tiles = dict(tx2m=tx2m, ft2=ft2, u=u, augw=augw, augx=augx)
                return parts, tiles

            def tail_ctx(b, tiles):
                ft2, u = tiles["ft2"], tiles["u"]
                augw, augx = tiles["augw"], tiles["augx"]
                ssum = rowp.tile([128, NF], F32, tag="ssum")
                lns = rowp.tile([128, NF], F32, tag="lns")
                dist_tiles = {}

                def pair(j):
                    def f():
                        ps = ppS.tile([128, 2, 512], F32, tag="psS")
                        for h in range(2):
                            i = 2 * j + h
                            nc.tensor.matmul(
                                ps[:, h, :],
                                ft2[:, :, i * 128 : (i + 1) * 128],
                                u[:, :, :], start=True, stop=False, perf_mode=DR,
                            )
                            nc.tensor.matmul(
                                ps[:, h, :],
                                augw[0:1, 0, i * 128 : (i + 1) * 128],
                                augx[0:1, 0, :], start=False, stop=False,
                            )
                            nc.tensor.matmul(
                                ps[:, h, :],
                                augw[0:1, 1, i * 128 : (i + 1) * 128],
                                augx[0:1, 1, :], start=False, stop=True,
                            )
                        dist = distp.tile([128, 2, 512], F16, tag="dist")
                        nc.scalar.activation(dist[:], ps[:], AF.Sqrt, bias=b_sq[:])
                        dist_tiles[j] = dist
                    return f

                def exp_pair(j):
                    def f():
                        dist = dist_tiles[j]
                        if j in PAIRED_SET:
                            e = ep.tile([128, 2, 512], BF16, tag="e")
                            nc.scalar.activation(e[:], dist[:], AF.Exp,
                                                 scale=-1.0, bias=b_m[:])
                            nc.vector.tensor_reduce(
                                ssum[:, 2 * j : 2 * j + 2], e[:], AX.X, OP.add
                            )
                        else:
                            for h in range(2):
                                i = 2 * j + h
                                e = ep.tile([128, 2, 512], BF16, tag="e")
                                nc.scalar.activation(
                                    e[:, 0, :], dist[:, h, :], AF.Exp,
                                    scale=-1.0, bias=b_m[:],
                                    accum_out=ssum[:, i : i + 1],
                                )
                    return f

                def ln_half(h):
                    def f():
                        nc.scalar.activation(
                            lns[:, 8 * h : 8 * h + 8], ssum[:, 8 * h : 8 * h + 8],
                            AF.Ln, scale=float(np.exp(-M_SHIFT)),
                        )
                    return f

                def quad(qi, split=False):
                    def f():
                        oq = outqp.tile([128, 4, 512], F32, tag="outq")
                        for q in range(4):
                            i = 4 * qi + q
                            dist = dist_tiles[i // 2]
                            if HOMES["outp"] == "alt":
                                eng = nc.gpsimd if q % 2 == 0 else nc.vector
                            else:
                                eng = {"pool": nc.gpsimd, "dve": nc.vector}[HOMES["outp"]]
                            eng.scalar_tensor_tensor(
                                oq[:, q, :], prior_sb[:, i, :], lns[:, i : i + 1],
                                dist[:, i % 2, :], OP.subtract, OP.subtract,
                            )
                            if split and q % 2 == 1:
                                nc.sync.dma_start(
                                    out=outD[b, 512 * qi + 256 * (q // 2) :
                                             512 * qi + 256 * (q // 2) + 256,
                                             :].rearrange("(q p) t -> p q t", p=128),
                                    in_=oq[:, q - 1 : q + 1, :],
                                )
                        if not split:
                            nc.sync.dma_start(
                                out=outD[b, 512 * qi : 512 * (qi + 1), :].rearrange(
                                    "(q p) t -> p q t", p=128
                                ),
                                in_=oq[:, :, :],
                            )
                    return f

                return pair, exp_pair, ln_half, quad

            # ================= emission =================
            tx0_0 = load_tx0(0)
            ft0_0 = load_ft0(0)
            tx0_1 = load_tx0(1)
            ft0_1 = load_ft0(1)
            nc.sync.dma_start(
                out=prior_sb[:], in_=priorD.rearrange("(i p) t -> p i t", p=128)
            )

            def batch_steps(b, tx0, ft0):
                p, tiles = batch_ctx(b, tx0, ft0)
                t = tail_ctx(b, tiles)
                pair, expp, lnh, quad = t
                return [
                    p["pads"],
                    p["f1_mm"](0), p["t1_mm"], p["f1_ev"](0), p["t1_ev"],
                    p["f1_mm"](1), p["t2_mm"], p["f1_ev"](1), p["t2_ev"],
                    p["f2_mm"](0), p["txsq"], p["f2_ev"](0),
                    p["t2row"], p["h_mm"](0), p["t2q"], p["fh_ev"](0),
                    p["f1_mm"](2), p["u_mm"], p["f1_ev"](2), p["u_ev"],
                    p["f2row_mm"](0), p["f2row_ev"](0),
                    pair(0),
                    p["f2_mm"](1), p["f1_mm"](3),
                    pair(1), p["f2_ev"](1), expp(0), p["f1_ev"](3),
                    p["h_mm"](1), p["fh_ev"](1),
                    p["f2row_mm"](1), p["f2row_ev"](1),
                    pair(2), p["f2_mm"](2), expp(1), p["f2_ev"](2),
                    pair(3), p["h_mm"](2), expp(2), p["fh_ev"](2),
                    p["f2row_mm"](2), p["f2row_ev"](2),
                    pair(4), p["f2_mm"](3), expp(3), p["f2_ev"](3),
                    lnh(0), quad(0),
                    pair(5), p["h_mm"](3), expp(4), p["fh_ev"](3),
                    p["f2row_mm"](3), p["f2row_ev"](3),
                    quad(1),
                    pair(6), expp(5), pair(7), expp(6), expp(7),
                    lnh(1), quad(2), quad(3, split=True),
                ]

            steps0 = batch_steps(0, tx0_0, ft0_0)
            steps1 = batch_steps(1, tx0_1, ft0_1)
            import os
            STAG = int(os.environ.get("KV2_STAGGER", "24"))
            merged = []
            i0 = i1 = 0
            # emit STAG steps of batch0 first, then alternate
            while i0 < len(steps0) or i1 < len(steps1):
                if i0 < len(steps0):
                    merged.append(steps0[i0]); i0 += 1
                if i0 >= STAG and i1 < len(steps1):
                    merged.append(steps1[i1]); i1 += 1
            for s in merged:
                s()

    _split_excess_waits(nc)
    return nc


_NC = None


def _get_nc():
    global _NC
    if _NC is None:
        _NC = _build_nc()
    return _NC


def _q8(x, scale, limit=230.0):
    import ml_dtypes
    y = np.asarray(x, np.float32) * scale
    m = np.abs(y).max()
    assert m < limit, f"fp8 range exceeded: {m} * (scale {scale})"
    return y.astype(ml_dtypes.float8_e4m3)


def _prep_inputs(text, feats, t_w1, t_b1, t_w2, t_b2,
                 f_w1, f_b1, f_w2, f_b2, f_w3, f_b3):
    for bias in (t_b1, t_b2, f_b1, f_b2, f_b3):
        assert not np.asarray(bias).any(), "kernel assumes zero biases (per spec)"
    c = np.ascontiguousarray
    f4 = np.float32

    textT = _q8(c(np.asarray(text, f4).transpose(0, 2, 1)), SX)    # [B,256,512]
    featsT = _q8(c(np.asarray(feats, f4).transpose(0, 2, 1)), SX)  # [B,80,2048]

    # wblob [128, WBLOB_W] fp8: per lhsT (k/m) block of [p, 2, 128]
    blob = np.zeros((128, WBLOB_W), np.float32)

    def put(name, idx, arr):  # arr [128, 2, 128] f32 (pre-scale applied)
        base = WOFF[name] + idx * 256
        blob[:, base : base + 256] = arr.reshape(128, 256)

    tw1 = np.asarray(t_w1, f4).transpose(2, 1, 0)  # [3, cin, cout]
    for k in range(3):
        for m in range(2):
            a = tw1[k].reshape(2, 128, 256)[:, :, m * 128 : (m + 1) * 128]
            put("tw1", k * 2 + m, a.transpose(1, 0, 2) * S_TW1)
    tw2 = np.asarray(t_w2, f4)[:, :, 0].T  # [cin, cout]
    for m in range(2):
        a = tw2.reshape(2, 128, 256)[:, :, m * 128 : (m + 1) * 128]
        put("tw2", m, a.transpose(1, 0, 2) * S_TW2)
    fw1 = np.asarray(f_w1, f4).transpose(2, 1, 0)  # [3, 80, 256]
    for d in range(2):
        for m in range(2):
            a = np.zeros((128, 2, 128), np.float32)
            a[:80, 0] = fw1[2 * d][:, m * 128 : (m + 1) * 128]
            if 2 * d + 1 < 3:
                a[:80, 1] = fw1[2 * d + 1][:, m * 128 : (m + 1) * 128]
            put("fw1", d * 2 + m, a * S_FW1)
    fw2 = np.asarray(f_w2, f4).transpose(2, 1, 0)
    for k in range(3):
        for m in range(2):
            a = fw2[k].reshape(2, 128, 256)[:, :, m * 128 : (m + 1) * 128]
            put("fw2", k * 2 + m, a.transpose(1, 0, 2) * S_FW2)
    W3 = np.asarray(f_w3, f4)[:, :, 0]  # [cout, cin]
    G = (W3.T @ W3).astype(np.float32)
    for m in range(2):
        a = G.reshape(2, 128, 256)[:, :, m * 128 : (m + 1) * 128]
        put("G", m, a.transpose(1, 0, 2) * S_G)
    for m in range(2):  # w3u lhsT[c, d]: W3 itself
        a = W3.reshape(2, 128, 256)[:, :, m * 128 : (m + 1) * 128]
        put("w3u", m, a.transpose(1, 0, 2) * S_W3)

    m = np.abs(blob).max()
    assert m < 230.0, f"wblob fp8 range exceeded: {m}"
    import ml_dtypes
    blob8 = blob.astype(ml_dtypes.float8_e4m3)

    import ml_dtypes as _mld
    ones8 = np.ones((1, TF), _mld.float8_e4m3)
    shared = {
        "wblob": blob8.view(np.uint8),
        "onesrow": ones8.view(np.uint8),
        "prior": _beta_binomial_prior().astype(np.float16),
    }
    in_maps = []
    for core in range(N_CORES):
        mcore = dict(shared)
        mcore["textT"] = c(textT[core * B_LOC : (core + 1) * B_LOC]).view(np.uint8)
        mcore["featsT"] = c(featsT[core * B_LOC : (core + 1) * B_LOC]).view(np.uint8)
        in_maps.append(mcore)
    return in_maps


_CALLABLE = None


def _build_callable():
    """Compile once; return fn(in_maps) -> per-core output dicts (axon path)."""
    import jax
    import jax.numpy as jnp
    from jax.sharding import Mesh, NamedSharding, PartitionSpec
    from jax.experimental.shard_map import shard_map
    from concourse.bass2jax import (
        _bass_exec_p,
        install_neuronx_cc_hook,
        partition_id_tensor,
    )

    nc = _get_nc()
    install_neuronx_cc_hook()
    partition_name = nc.partition_id_tensor.name if nc.partition_id_tensor else None
    in_names, out_names, out_avals, zero_shapes = [], [], [], []
    for alloc in nc.m.functions[0].allocations:
        if not isinstance(alloc, mybir.MemoryLocationSet):
            continue
        name = alloc.memorylocations[0].name
        if alloc.kind == "ExternalInput":
            if name != partition_name:
                in_names.append(name)
        elif alloc.kind == "ExternalOutput":
            shape = tuple(alloc.tensor_shape)
            dtype = mybir.dt.np(alloc.dtype)
            out_names.append(name)
            out_avals.append(jax.core.ShapedArray(shape, dtype))
            zero_shapes.append(((N_CORES * shape[0],) + shape[1:], dtype))
    n_params = len(in_names)
    n_outs = len(out_avals)
    all_in_names = list(in_names) + out_names
    if partition_name is not None:
        all_in_names.append(partition_name)
    donate = tuple(range(n_params, n_params + n_outs))

    def _body(*args):
        operands = list(args)
        if partition_name is not None:
            operands.append(partition_id_tensor())
        outs = _bass_exec_p.bind(
            *operands,
            out_avals=tuple(out_avals),
            in_names=tuple(all_in_names),
            out_names=tuple(out_names),
            lowering_input_output_aliases=(),
            sim_require_finite=True,
            sim_require_nnan=True,
            nc=nc,
        )
        return tuple(outs)

    devices = jax.devices()[:N_CORES]
    mesh = Mesh(np.asarray(devices), ("core",))
    fn = jax.jit(
        shard_map(
            _body,
            mesh=mesh,
            in_specs=(PartitionSpec("core"),) * (n_params + n_outs),
            out_specs=(PartitionSpec("core"),) * n_outs,
            check_rep=False,
        ),
        donate_argnums=donate,
        keep_unused=True,
    )
    sharding = NamedSharding(mesh, PartitionSpec("core"))
    zfn = jax.jit(
        lambda: tuple(jnp.zeros(s, d) for s, d in zero_shapes),
        out_shardings=tuple(sharding for _ in zero_shapes),
    )

    def call(in_maps):
        concat_in = [
            np.concatenate([np.asarray(in_maps[c][n]) for c in range(N_CORES)], axis=0)
            for n in in_names
        ]
        out_arrs = fn(*concat_in, *zfn())
        return [
            {
                name: np.asarray(out_arrs[i]).reshape(
                    N_CORES, *out_avals[i].shape
                )[c]
                for i, name in enumerate(out_names)
            }
            for c in range(N_CORES)
        ]

    return call


def _run(inputs, **kw):
    global _CALLABLE
    import time as _time

    in_maps = _prep_inputs(
        inputs["text"], inputs["feats"],
        inputs["t_w1"], inputs["t_b1"], inputs["t_w2"], inputs["t_b2"],
        inputs["f_w1"], inputs["f_b1"], inputs["f_w2"], inputs["f_b2"],
        inputs["f_w3"], inputs["f_b3"],
    )
    results = None
    last_err = None
    if _CALLABLE is not False:
        for attempt in range(3):
            try:
                if _CALLABLE is None:
                    from concourse._compat import axon_active

                    if not axon_active():
                        raise RuntimeError("axon not active; use native path")
                    _CALLABLE = _build_callable()
                results = _CALLABLE(in_maps)
                break
            except Exception as e:
                last_err = e
                results = None
                if attempt < 2:
                    _time.sleep(20 * (attempt + 1))
        if results is None:
            _CALLABLE = False
    if results is None:
        from concourse.bass_utils import run_bass_kernel_spmd

        for attempt in range(3):
            try:
                results = run_bass_kernel_spmd(
                    _get_nc(), in_maps, core_ids=list(range(N_CORES))
                ).results
                break
            except Exception as e:
                last_err = e
                results = None
                if attempt < 2:
                    _time.sleep(20 * (attempt + 1))
    if results is None:
        raise last_err
    out = np.concatenate([r["out"] for r in results], axis=0)
    return out, results


def kernel(**inputs) -> np.ndarray:
    out, _ = _run(inputs)
    return out


# revision 12
# speedup vs baseline: 1.0144x; 1.0144x over previous
"""AlignmentModule on 8 Trainium2 cores — fp8 DoubleRow rewrite.

Data-parallel over batch (2 per core). All matmuls run as fp8e4 DoubleRow
(2 K-planes per pass, 0.5 cycles/row = 4x fp32r): conv stacks t1,t2 / f1,f2,
the Gram matmul H=G.ft2, the text-side-absorbed f3 (u = W3^T tx2m — the 1x1
f3 conv is algebraically moved to the 4x-smaller text side; f2 norms come from
fh = ft2*H read straight off H's PSUM), the score cross ft2^T.u, and a K=1
augmented DR that adds f2[f] + (t2[t]-c0) rank-2 terms into the same PSUM.

Norm rows: f2row/t2row are ones-weight DR matmuls into PSUM row slots
(partitions 0/32/64 + bank2), evicted same-partition to fp8 and DMA-gathered
to the partition-0 aug operand rows (engines cannot cross partitions; DMA can).

Tail per f-tile: ACT sqrt (psum pair -> fp16 dist), ACT exp(14-dist) with
f32 row-accum (a tunable number of pairs instead run paired-exp + DVE reduce),
one ACT ln per batch, then outp = (prior_f16 - cc) - dist on Pool, DMA out in
4-tile quads. Engine assignment of evictions is tuned: Pool takes t1/t2/u +
outp, DVE takes f1/f2/fh/txsq/rows, ACT takes sqrt/exp/ln.

Host pre-quantizes inputs/weights to fp8 (power-of-2 scales, ranges asserted)
and ships them as uint8 bits; fp16 prior. Total rel err vs the f32 reference
is ~6.5e-4 (validated offline), dominated by fp8 conv activations and the
fp16 prior.
"""

import numpy as np

import bass_rust as _bass_rust
import concourse.bass as bass
import concourse.mybir as mybir
from concourse.tile import TileContext

F32 = mybir.dt.float32
F16 = mybir.dt.float16
BF16 = mybir.dt.bfloat16
F8 = mybir.dt.float8e4
U8 = mybir.dt.uint8
AF = mybir.ActivationFunctionType
OP = mybir.AluOpType
AX = mybir.AxisListType
DR = mybir.MatmulPerfMode.DoubleRow

B, T_TEXT, T_FEATS, ADIM, ODIM = 16, 512, 2048, 256, 80
N_CORES = 8
B_LOC = B // N_CORES
TT, TF = T_TEXT, T_FEATS
NT = TF // 512   # 4 feats chunks of 512
NF = TF // 128   # 16 f-tiles per batch

# ---- fixed power-of-2 scales (validated in opt/sim_numerics2.py) ----
SX = 16.0
S_TW1, S_TW2 = 1024.0, 512.0
S_FW1, S_FW2, S_W3 = 512.0, 1024.0, 512.0
S_G = 64.0
A1 = 1.0 / 512.0       # tx1 evict; tile = 32*true
A2 = 1.0 / 2048.0      # tx2m evict; tile = 8*true
AF1 = 1.0 / 512.0      # ft1 tile = 16*true
AF2 = 1.0 / 8192.0     # ft2 tile = 2*true
BH = 1.0 / 32.0        # fh evict scalar
BU = 1.0 / 4096.0      # u evict; u tile = -1*true(W3^T tx2)
ONES2_F2 = 1.0 / 8.0   # f2row ones-weight = 1/(S_G*s_ft2^2*BH)
T2Q_MUL = 1.0 / 64.0   # t2row evict mult = 1/s_tx2m^2
C0 = 192.0             # t2q offset; sqrt bias adds it back
M_SHIFT = 14.0
SQ_BIAS = C0           # dist = sqrt(psum + C0)

PAIRED_SET = (0,)  # score pairs using paired-exp + DVE reduce
# engine homes for evictions: "dve" | "pool" | "act"(relu/identity/square ok)
# NOTE: GPSIMD (pool) cannot access PSUM on HW — psum-evictions are dve/act only.
HOMES = {
    "t1e": "act", "t2e": "act", "ue": "act",
    "f1e": "dve", "f2e": "dve", "fh": "dve",
    "rows": "act", "t2q": "dve", "txsq": "act",
    "outp": "dve",  # pool cannot do TensorScalarPtr on HW
}

WOFF = {}              # wblob free-dim offsets, filled by _pack_weights layout
WBLOB_W = 1536 + 512 + 1024 + 1536 + 512 + 512  # tw1,tw2,fw1,fw2,G,w3u


def _wblob_offsets():
    off, o = {}, 0
    for name, w in (("tw1", 1536), ("tw2", 512), ("fw1", 1024),
                    ("fw2", 1536), ("G", 512), ("w3u", 512)):
        off[name] = o
        o += w
    assert o == WBLOB_W
    return off


WOFF = _wblob_offsets()


def _split_excess_waits(nc, limit=1):
    """walrus CoreV3 CTRL codegen rejects >1 sync-wait per instruction.
    Hoist excess waits onto preceding NOPs on the same engine."""
    ctr = 0
    for f in nc.m.functions:
        for bb in f.blocks:
            insts = bb.instructions
            idx = 0
            while idx < len(insts):
                ins = insts[idx]
                si = ins.sync_info
                if si is not None and len(si.on_wait) > limit:
                    waits = list(si.on_wait)
                    extra, keep = waits[:-limit], waits[-limit:]
                    si.on_wait = keep
                    pos = idx
                    for j in range(0, len(extra), limit):
                        nop = mybir.InstNoOp(name=f"waitsplit_{ctr}", ins=[], outs=[])
                        ctr += 1
                        nop.engine = ins.engine
                        nop.sync_info = mybir.SyncInfo(
                            on_wait=extra[j : j + limit], on_update=[]
                        )
                        insts.insert(pos, nop)
                        pos += 1
                        idx += 1
                idx += 1
    return ctr


def _beta_binomial_prior():
    from scipy.special import gammaln

    T, N = T_FEATS, T_TEXT
    a = np.arange(1, T + 1, dtype=np.float64)[:, None]
    b = (T - np.arange(1, T + 1, dtype=np.float64) + 1.0)[:, None]
    k = np.arange(N, dtype=np.float64)[None, :]
    n = float(N)

    def betaln(x, y):
        return gammaln(x) + gammaln(y) - gammaln(x + y)

    logp = (
        gammaln(n + 1.0) - gammaln(k + 1.0) - gammaln(n - k + 1.0)
        + betaln(k + a, n - k + b) - betaln(a, b)
    )
    return logp.astype(np.float32)


def _build_nc():
    nc = bass.Bass(name="alignment")

    textT = nc.dram_tensor("textT", [B_LOC, ADIM, TT], U8, kind="ExternalInput")
    featsT = nc.dram_tensor("featsT", [B_LOC, ODIM, TF], U8, kind="ExternalInput")
    wblob = nc.dram_tensor("wblob", [128, WBLOB_W], U8, kind="ExternalInput")
    onesrow = nc.dram_tensor("onesrow", [1, TF], U8, kind="ExternalInput")
    priorD = nc.dram_tensor("prior", [TF, TT], F16, kind="ExternalInput")
    outD = nc.dram_tensor("out", [B_LOC, TF, TT], F32, kind="ExternalOutput")

    with TileContext(nc) as tc:
        with (
            tc.tile_pool(name="const", bufs=1) as const,
            tc.tile_pool(name="inp", bufs=2) as inp,
            tc.tile_pool(name="actp", bufs=2) as actp,
            tc.tile_pool(name="rowp", bufs=2) as rowp,
            tc.tile_pool(name="distp", bufs=17) as distp,
            tc.tile_pool(name="ep", bufs=3) as ep,
            tc.tile_pool(name="outq", bufs=3) as outqp,
            tc.tile_pool(name="ppA", bufs=2, space="PSUM") as ppA,
            tc.tile_pool(name="ppS", bufs=2, space="PSUM") as ppS,
        ):
            # ---- constants / weights ----
            wb = const.tile([128, WBLOB_W], F8)
            nc.sync.dma_start(out=wb[:].bitcast(U8), in_=wblob[:])
            ones1 = const.tile([128, 1], F8)
            nc.vector.memset(ones1[:], ONES2_F2)
            onesb = const.tile([128, 1], BF16)
            nc.vector.memset(onesb[:], 1.0)
            b_sq = const.tile([128, 1], F32)
            nc.vector.memset(b_sq[:], SQ_BIAS)
            b_m = const.tile([128, 1], F32)
            nc.vector.memset(b_m[:], M_SHIFT)
            prior_sb = const.tile([128, NF, TT], F16)

            def ev_scale_relu(home, out, ps, scale):
                if home == "act":
                    nc.scalar.activation(out, ps, AF.Relu, scale=scale)
                elif home == "split":
                    nc.vector.tensor_scalar(out[:, 0, :], ps[:, 0, :], scale,
                                            0.0, OP.mult, OP.max)
                    nc.gpsimd.tensor_scalar(out[:, 1, :], ps[:, 1, :], scale,
                                            0.0, OP.mult, OP.max)
                else:
                    eng = nc.vector if home == "dve" else nc.gpsimd
                    eng.tensor_scalar(out, ps, scale, 0.0, OP.mult, OP.max)

            def ev_scale(home, out, ps, scale):
                if home == "act":
                    nc.scalar.activation(out, ps, AF.Identity, scale=scale)
                elif home == "split":
                    nc.vector.tensor_scalar(out[:, 0, :], ps[:, 0, :], scale,
                                            None, OP.mult)
                    nc.gpsimd.tensor_scalar(out[:, 1, :], ps[:, 1, :], scale,
                                            None, OP.mult)
                else:
                    eng = nc.vector if home == "dve" else nc.gpsimd
                    eng.tensor_scalar(out, ps, scale, None, OP.mult)

            def wap(name, idx, planes=2, width=128):
                base = WOFF[name] + idx * planes * width
                return wb[:, base : base + planes * width].rearrange(
                    "p (c w) -> p c w", c=planes
                )

            def load_tx0(b):
                tx0 = inp.tile([128, 2, TT + 2], F8, tag="tx0")
                nc.vector.memset(tx0[:, :, 0:1], 0.0)
                nc.vector.memset(tx0[:, :, TT + 1 : TT + 2], 0.0)
                nc.sync.dma_start(
                    out=tx0[:, :, 1 : TT + 1].bitcast(U8),
                    in_=textT[b].rearrange("(c p) t -> p c t", p=128),
                )
                return tx0

            def load_ft0(b):
                ft0 = inp.tile([ODIM, TF + 3], F8, tag="ft0")
                nc.vector.memset(ft0[:, 0:1], 0.0)
                nc.vector.memset(ft0[:, TF + 1 : TF + 3], 0.0)
                nc.sync.dma_start(out=ft0[:, 1 : TF + 1].bitcast(U8), in_=featsT[b])
                return ft0

            def batch_ctx(b, tx0, ft0):
                """Allocate per-batch tiles and return the conv step list plus
                the tile handles the tail needs."""
                tx1 = actp.tile([128, 2, TT], F8, tag="tx1")
                tx2m = actp.tile([128, 2, TT], F8, tag="tx2m")
                txsq = actp.tile([128, 2, TT], BF16, tag="txsq")
                ft1 = actp.tile([128, 2, TF + 2], F8, tag="ft1")
                ft2 = actp.tile([128, 2, TF], F8, tag="ft2")
                fh = actp.tile([128, 2, TF], F8, tag="fh")
                u = actp.tile([128, 2, TT], F8, tag="u")
                augw = rowp.tile([1, 2, TF], F8, tag="augw")
                augx = rowp.tile([1, 2, TT], F8, tag="augx")

                box = {}
                steps = []

                def pads():
                    nc.vector.memset(ft1[:, :, 0:1], 0.0)
                    nc.vector.memset(ft1[:, :, TF + 1 : TF + 2], 0.0)
                    nc.sync.dma_start(out=augw[0:1, 1, :].bitcast(U8),
                                      in_=onesrow[0:1, :])
                    nc.sync.dma_start(out=augx[0:1, 0, :].bitcast(U8),
                                      in_=onesrow[0:1, 0:TT])

                def tap_pair(start):
                    a = ft0[0:ODIM, start : start + 512]
                    w = a.copy()
                    w.ap = _bass_rust.VecI64Pair([list(a.ap[0]), [1, 2], [1, 512]])
                    return w

                def f1_mm(n):
                    def f():
                        ps = ppA.tile([128, 2, 512], F32, tag="psA")
                        box[("f1", n)] = ps
                        for m in range(2):
                            for k in range(3):
                                d, pl = divmod(k, 2)
                                base = WOFF["fw1"] + (d * 2 + m) * 256 + pl * 128
                                lhs = wb[0:ODIM, base : base + 128]
                                nc.tensor.matmul(
                                    ps[:, m, :], lhs,
                                    ft0[0:ODIM, n * 512 + k : n * 512 + k + 512],
                                    start=(k == 0), stop=(k == 2),
                                )
                    return f

                def f1_ev(n):
                    def f():
                        ps = box.pop(("f1", n))
                        ev_scale_relu(HOMES["f1e"],
                                      ft1[:, :, 1 + n * 512 : 1 + (n + 1) * 512],
                                      ps[:, :, :], AF1)
                    return f

                def t1_mm():
                    ps = ppA.tile([128, 2, 512], F32, tag="psA")
                    box["t1"] = ps
                    for m in range(2):
                        for k in range(3):
                            nc.tensor.matmul(
                                ps[:, m, :], wap("tw1", k * 2 + m),
                                tx0[:, :, k : k + TT],
                                start=(k == 0), stop=(k == 2), perf_mode=DR,
                            )

                def t1_ev():
                    ps = box.pop("t1")
                    ev_scale_relu(HOMES["t1e"], tx1[:, :, :], ps[:, :, :], A1)

                def t2_mm():
                    ps = ppA.tile([128, 2, 512], F32, tag="psA")
                    box["t2"] = ps
                    for m in range(2):
                        nc.tensor.matmul(
                            ps[:, m, :], wap("tw2", m), tx1[:, :, :],
                            start=True, stop=True, perf_mode=DR,
                        )

                def t2_ev():
                    ps = box["t2"]
                    ev_scale(HOMES["t2e"], tx2m[:, :, :], ps[:, :, :], A2)

                def txsq_f():
                    ps = box["t2"]
                    if HOMES["txsq"] == "act":
                        nc.scalar.activation(txsq[:, :, :], ps[:, :, :], AF.Square,
                                             scale=A2)
                    else:
                        eng = nc.vector if HOMES["txsq"] == "dve" else nc.gpsimd
                        eng.tensor_tensor(txsq[:, :, :], tx2m[:, :, :],
                                          tx2m[:, :, :], OP.mult)

                def t2row_mm():
                    psT = box["t2"]  # reuse t2 psum tile (already evicted)
                    for cc_ in range(2):
                        nc.tensor.matmul(psT[0:1, 0, :], onesb[:], txsq[:, cc_, :],
                                         start=(cc_ == 0), stop=(cc_ == 1))

                def t2q_ev():
                    psT = box.pop("t2")
                    _e = {"dve": nc.vector, "pool": nc.gpsimd}[HOMES["t2q"]]
                    _e.tensor_scalar(
                        augx[0:1, 1, :], psT[0:1, 0, :],
                        T2Q_MUL, C0, OP.mult, OP.subtract,
                    )

                def u_mm():
                    ps = ppA.tile([128, 2, 512], F32, tag="psA")
                    box["u"] = ps
                    for m in range(2):
                        nc.tensor.matmul(
                            ps[:, m, :], wap("w3u", m), tx2m[:, :, :],
                            start=True, stop=True, perf_mode=DR,
                        )

                def u_ev():
                    ps = box.pop("u")
                    ev_scale(HOMES["ue"], u[:, :, :], ps[:, :, :], -BU)

                def f2_mm(n):
                    def f():
                        ps = ppA.tile([128, 2, 512], F32, tag="psA")
                        box[("f2", n)] = ps
                        for m in range(2):
                            for k in range(3):
                                nc.tensor.matmul(
                                    ps[:, m, :], wap("fw2", k * 2 + m),
                                    ft1[:, :, n * 512 + k : n * 512 + k + 512],
                                    start=(k == 0), stop=(k == 2), perf_mode=DR,
                                )
                    return f

                def f2_ev(n):
                    def f():
                        ps = box.pop(("f2", n))
                        ev_scale_relu(HOMES["f2e"],
                                      ft2[:, :, n * 512 : (n + 1) * 512],
                                      ps[:, :, :], AF2)
                    return f

                def h_mm(n):
                    def f():
                        ps = ppA.tile([128, 2, 512], F32, tag="psA")
                        box[("h", n)] = ps
                        for m in range(2):
                            nc.tensor.matmul(
                                ps[:, m, :], wap("G", m),
                                ft2[:, :, n * 512 : (n + 1) * 512],
                                start=True, stop=True, perf_mode=DR,
                            )
                    return f

                def fh_ev(n):
                    def f():
                        ps = box[("h", n)]
                        sl = slice(n * 512, (n + 1) * 512)
                        if HOMES["fh"] == "split":
                            nc.vector.scalar_tensor_tensor(
                                fh[:, 0, sl], ps[:, 0, :], BH, ft2[:, 0, sl],
                                OP.mult, OP.mult)
                            nc.gpsimd.scalar_tensor_tensor(
                                fh[:, 1, sl], ps[:, 1, :], BH, ft2[:, 1, sl],
                                OP.mult, OP.mult)
                        else:
                            eng = nc.vector if HOMES["fh"] == "dve" else nc.gpsimd
                            eng.scalar_tensor_tensor(
                                fh[:, :, sl], ps[:, :, :], BH, ft2[:, :, sl],
                                OP.mult, OP.mult)
                    return f

                def f2row_mm(n):
                    def f():
                        psH = box[("h", n)]  # reuse after fh_ev consumed it
                        for c_ in range(2):
                            nc.tensor.matmul(
                                psH[0:1, 0, :], ones1[:, :],
                                fh[:, c_, n * 512 : (n + 1) * 512],
                                start=(c_ == 0), stop=(c_ == 1),
                            )
                    return f

                def f2row_ev(n):
                    def f():
                        psH = box.pop(("h", n))
                        ev_scale(HOMES["rows"],
                                 augw[0:1, 0, n * 512 : (n + 1) * 512],
                                 psH[0:1, 0, :], 1.0)
                    return f

                parts = dict(
                    pads=pads, f1_mm=f1_mm, f1_ev=f1_ev, t1_mm=t1_mm, t1_ev=t1_ev,
                    t2_mm=t2_mm, t2_ev=t2_ev, txsq=txsq_f, t2row=t2row_mm,
                    t2q=t2q_ev, u_mm=u_mm, u_ev=u_ev, f2_mm=f2_mm, f2_ev=f2_ev,
                    h_mm=h_mm, fh_ev=fh_ev, f2row_mm=f2row_mm, f2row_ev=f2row_ev,
                )
                tiles = dict(tx2m=tx2m, ft2=ft2, u=u, augw=augw, augx=augx)
                return parts, tiles

            def tail_ctx(b, tiles):
                ft2, u = tiles["ft2"], tiles["u"]
                augw, augx = tiles["augw"], tiles["augx"]
                ssum = rowp.tile([128, NF], F32, tag="ssum")
                lns = rowp.tile([128, NF], F32, tag="lns")
                dist_tiles = {}

                def pair(j):
                    def f():
                        ps = ppS.tile([128, 2, 512], F32, tag="psS")
                        for h in range(2):
                            i = 2 * j + h
                            nc.tensor.matmul(
                                ps[:, h, :],
                                ft2[:, :, i * 128 : (i + 1) * 128],
                                u[:, :, :], start=True, stop=False, perf_mode=DR,
                            )
                            nc.tensor.matmul(
                                ps[:, h, :],
                                augw[0:1, 0, i * 128 : (i + 1) * 128],
                                augx[0:1, 0, :], start=False, stop=False,
                            )
                            nc.tensor.matmul(
                                ps[:, h, :],
                                augw[0:1, 1, i * 128 : (i + 1) * 128],
                                augx[0:1, 1, :], start=False, stop=True,
                            )
                        dist = distp.tile([128, 2, 512], F16, tag="dist")
                        nc.scalar.activation(dist[:], ps[:], AF.Sqrt, bias=b_sq[:])
                        dist_tiles[j] = dist
                    return f

                def exp_pair(j):
                    def f():
                        dist = dist_tiles[j]
                        if j in PAIRED_SET:
                            e = ep.tile([128, 2, 512], BF16, tag="e")
                            nc.scalar.activation(e[:], dist[:], AF.Exp,
                                                 scale=-1.0, bias=b_m[:])
                            nc.vector.tensor_reduce(
                                ssum[:, 2 * j : 2 * j + 2], e[:], AX.X, OP.add
                            )
                        else:
                            for h in range(2):
                                i = 2 * j + h
                                e = ep.tile([128, 2, 512], BF16, tag="e")
                                nc.scalar.activation(
                                    e[:, 0, :], dist[:, h, :], AF.Exp,
                                    scale=-1.0, bias=b_m[:],
                                    accum_out=ssum[:, i : i + 1],
                                )
                    return f

                def ln_half(h):
                    def f():
                        nc.scalar.activation(
                            lns[:, 8 * h : 8 * h + 8], ssum[:, 8 * h : 8 * h + 8],
                            AF.Ln, scale=float(np.exp(-M_SHIFT)),
                        )
                    return f

                def quad(qi, split=False):
                    def f():
                        oq = outqp.tile([128, 4, 512], F32, tag="outq")
                        for q in range(4):
                            i = 4 * qi + q
                            dist = dist_tiles[i // 2]
                            if HOMES["outp"] == "alt":
                                eng = nc.gpsimd if q % 2 == 0 else nc.vector
                            else:
                                eng = {"pool": nc.gpsimd, "dve": nc.vector}[HOMES["outp"]]
                            eng.scalar_tensor_tensor(
                                oq[:, q, :], prior_sb[:, i, :], lns[:, i : i + 1],
                                dist[:, i % 2, :], OP.subtract, OP.subtract,
                            )
                            if split and q % 2 == 1:
                                nc.sync.dma_start(
                                    out=outD[b, 512 * qi + 256 * (q // 2) :
                                             512 * qi + 256 * (q // 2) + 256,
                                             :].rearrange("(q p) t -> p q t", p=128),
                                    in_=oq[:, q - 1 : q + 1, :],
                                )
                        if not split:
                            nc.sync.dma_start(
                                out=outD[b, 512 * qi : 512 * (qi + 1), :].rearrange(
                                    "(q p) t -> p q t", p=128
                                ),
                                in_=oq[:, :, :],
                            )
                    return f

                return pair, exp_pair, ln_half, quad

            # ================= emission =================
            tx0_0 = load_tx0(0)
            ft0_0 = load_ft0(0)
            tx0_1 = load_tx0(1)
            ft0_1 = load_ft0(1)
            nc.sync.dma_start(
                out=prior_sb[:], in_=priorD.rearrange("(i p) t -> p i t", p=128)
            )

            def batch_steps(b, tx0, ft0):
                p, tiles = batch_ctx(b, tx0, ft0)
                t = tail_ctx(b, tiles)
                pair, expp, lnh, quad = t
                return [
                    p["pads"],
                    p["f1_mm"](0), p["t1_mm"], p["f1_ev"](0), p["t1_ev"],
                    p["f1_mm"](1), p["t2_mm"], p["f1_ev"](1), p["t2_ev"],
                    p["f2_mm"](0), p["txsq"], p["f2_ev"](0),
                    p["t2row"], p["h_mm"](0), p["t2q"], p["fh_ev"](0),
                    p["f1_mm"](2), p["u_mm"], p["f1_ev"](2), p["u_ev"],
                    p["f2row_mm"](0), p["f2row_ev"](0),
                    pair(0),
                    p["f2_mm"](1), p["f1_mm"](3),
                    pair(1), p["f2_ev"](1), expp(0), p["f1_ev"](3),
                    p["h_mm"](1), p["fh_ev"](1),
                    p["f2row_mm"](1), p["f2row_ev"](1),
                    pair(2), p["f2_mm"](2), expp(1), p["f2_ev"](2),
                    pair(3), p["h_mm"](2), expp(2), p["fh_ev"](2),
                    p["f2row_mm"](2), p["f2row_ev"](2),
                    pair(4), p["f2_mm"](3), expp(3), p["f2_ev"](3),
                    lnh(0), quad(0),
                    pair(5), p["h_mm"](3), expp(4), p["fh_ev"](3),
                    p["f2row_mm"](3), p["f2row_ev"](3),
                    quad(1),
                    pair(6), expp(5), pair(7), expp(6), expp(7),
                    lnh(1), quad(2, split=True), quad(3, split=True),
                ]

            steps0 = batch_steps(0, tx0_0, ft0_0)
            steps1 = batch_steps(1, tx0_1, ft0_1)
            import os
            STAG = int(os.environ.get("KV2_STAGGER", "24"))
            merged = []
            i0 = i1 = 0
            # emit STAG steps of batch0 first, then alternate
            while i0 < len(steps0) or i1 < len(steps1):
                if i0 < len(steps0):
                    merged.append(steps0[i0]); i0 += 1
                if i0 >= STAG and i1 < len(steps1):
                    merged.append(steps1[i1]); i1 += 1
            for s in merged:
                s()

    _split_excess_waits(nc)
    return nc


_NC = None


def _get_nc():
    global _NC
    if _NC is None:
        _NC = _build_nc()
    return _NC


def _q8(x, scale, limit=230.0):
    import ml_dtypes
    y = np.asarray(x, np.float32) * scale
    m = np.abs(y).max()
    assert m < limit, f"fp8 range exceeded: {m} * (scale {scale})"
    return y.astype(ml_dtypes.float8_e4m3)


def _prep_inputs(text, feats, t_w1, t_b1, t_w2, t_b2,
                 f_w1, f_b1, f_w2, f_b2, f_w3, f_b3):
    for bias in (t_b1, t_b2, f_b1, f_b2, f_b3):
        assert not np.asarray(bias).any(), "kernel assumes zero biases (per spec)"
    c = np.ascontiguousarray
    f4 = np.float32

    textT = _q8(c(np.asarray(text, f4).transpose(0, 2, 1)), SX)    # [B,256,512]
    featsT = _q8(c(np.asarray(feats, f4).transpose(0, 2, 1)), SX)  # [B,80,2048]

    # wblob [128, WBLOB_W] fp8: per lhsT (k/m) block of [p, 2, 128]
    blob = np.zeros((128, WBLOB_W), np.float32)

    def put(name, idx, arr):  # arr [128, 2, 128] f32 (pre-scale applied)
        base = WOFF[name] + idx * 256
        blob[:, base : base + 256] = arr.reshape(128, 256)

    tw1 = np.asarray(t_w1, f4).transpose(2, 1, 0)  # [3, cin, cout]
    for k in range(3):
        for m in range(2):
            a = tw1[k].reshape(2, 128, 256)[:, :, m * 128 : (m + 1) * 128]
            put("tw1", k * 2 + m, a.transpose(1, 0, 2) * S_TW1)
    tw2 = np.asarray(t_w2, f4)[:, :, 0].T  # [cin, cout]
    for m in range(2):
        a = tw2.reshape(2, 128, 256)[:, :, m * 128 : (m + 1) * 128]
        put("tw2", m, a.transpose(1, 0, 2) * S_TW2)
    fw1 = np.asarray(f_w1, f4).transpose(2, 1, 0)  # [3, 80, 256]
    for d in range(2):
        for m in range(2):
            a = np.zeros((128, 2, 128), np.float32)
            a[:80, 0] = fw1[2 * d][:, m * 128 : (m + 1) * 128]
            if 2 * d + 1 < 3:
                a[:80, 1] = fw1[2 * d + 1][:, m * 128 : (m + 1) * 128]
            put("fw1", d * 2 + m, a * S_FW1)
    fw2 = np.asarray(f_w2, f4).transpose(2, 1, 0)
    for k in range(3):
        for m in range(2):
            a = fw2[k].reshape(2, 128, 256)[:, :, m * 128 : (m + 1) * 128]
            put("fw2", k * 2 + m, a.transpose(1, 0, 2) * S_FW2)
    W3 = np.asarray(f_w3, f4)[:, :, 0]  # [cout, cin]
    G = (W3.T @ W3).astype(np.float32)
    for m in range(2):
        a = G.reshape(2, 128, 256)[:, :, m * 128 : (m + 1) * 128]
        put("G", m, a.transpose(1, 0, 2) * S_G)
    for m in range(2):  # w3u lhsT[c, d]: W3 itself
        a = W3.reshape(2, 128, 256)[:, :, m * 128 : (m + 1) * 128]
        put("w3u", m, a.transpose(1, 0, 2) * S_W3)

    m = np.abs(blob).max()
    assert m < 230.0, f"wblob fp8 range exceeded: {m}"
    import ml_dtypes
    blob8 = blob.astype(ml_dtypes.float8_e4m3)

    import ml_dtypes as _mld
    ones8 = np.ones((1, TF), _mld.float8_e4m3)
    shared = {
        "wblob": blob8.view(np.uint8),
        "onesrow": ones8.view(np.uint8),
        "prior": _beta_binomial_prior().astype(np.float16),
    }
    in_maps = []
    for core in range(N_CORES):
        mcore = dict(shared)
        mcore["textT"] = c(textT[core * B_LOC : (core + 1) * B_LOC]).view(np.uint8)
        mcore["featsT"] = c(featsT[core * B_LOC : (core + 1) * B_LOC]).view(np.uint8)
        in_maps.append(mcore)
    return in_maps


_CALLABLE = None


def _build_callable():
    """Compile once; return fn(in_maps) -> per-core output dicts (axon path)."""
    import jax
    import jax.numpy as jnp
    from jax.sharding import Mesh, NamedSharding, PartitionSpec
    from jax.experimental.shard_map import shard_map
    from concourse.bass2jax import (
        _bass_exec_p,
        install_neuronx_cc_hook,
        partition_id_tensor,
    )

    nc = _get_nc()
    install_neuronx_cc_hook()
    partition_name = nc.partition_id_tensor.name if nc.partition_id_tensor else None
    in_names, out_names, out_avals, zero_shapes = [], [], [], []
    for alloc in nc.m.functions[0].allocations:
        if not isinstance(alloc, mybir.MemoryLocationSet):
            continue
        name = alloc.memorylocations[0].name
        if alloc.kind == "ExternalInput":
            if name != partition_name:
                in_names.append(name)
        elif alloc.kind == "ExternalOutput":
            shape = tuple(alloc.tensor_shape)
            dtype = mybir.dt.np(alloc.dtype)
            out_names.append(name)
            out_avals.append(jax.core.ShapedArray(shape, dtype))
            zero_shapes.append(((N_CORES * shape[0],) + shape[1:], dtype))
    n_params = len(in_names)
    n_outs = len(out_avals)
    all_in_names = list(in_names) + out_names
    if partition_name is not None:
        all_in_names.append(partition_name)
    donate = tuple(range(n_params, n_params + n_outs))

    def _body(*args):
        operands = list(args)
        if partition_name is not None:
            operands.append(partition_id_tensor())
        outs = _bass_exec_p.bind(
            *operands,
            out_avals=tuple(out_avals),
            in_names=tuple(all_in_names),
            out_names=tuple(out_names),
            lowering_input_output_aliases=(),
            sim_require_finite=True,
            sim_require_nnan=True,
            nc=nc,
        )
        return tuple(outs)

    devices = jax.devices()[:N_CORES]
    mesh = Mesh(np.asarray(devices), ("core",))
    fn = jax.jit(
        shard_map(
            _body,
            mesh=mesh,
            in_specs=(PartitionSpec("core"),) * (n_params + n_outs),
            out_specs=(PartitionSpec("core"),) * n_outs,
            check_rep=False,
        ),
        donate_argnums=donate,
        keep_unused=True,
    )
    sharding = NamedSharding(mesh, PartitionSpec("core"))
    zfn = jax.jit(
        lambda: tuple(jnp.zeros(s, d) for s, d in zero_shapes),
        out_shardings=tuple(sharding for _ in zero_shapes),
    )

    def call(in_maps):
        concat_in = [
            np.concatenate([np.asarray(in_maps[c][n]) for c in range(N_CORES)], axis=0)
            for n in in_names
        ]
        out_arrs = fn(*concat_in, *zfn())
        return [
            {
                name: np.asarray(out_arrs[i]).reshape(
                    N_CORES, *out_avals[i].shape
                )[c]
                for i, name in enumerate(out_names)
            }
            for c in range(N_CORES)
        ]

    return call


def _run(inputs, **kw):
    global _CALLABLE
    import time as _time

    in_maps = _prep_inputs(
        inputs["text"], inputs["feats"],
        inputs["t_w1"], inputs["t_b1"], inputs["t_w2"], inputs["t_b2"],
        inputs["f_w1"], inputs["f_b1"], inputs["f_w2"], inputs["f_b2"],
        inputs["f_w3"], inputs["f_b3"],
    )
    results = None
    last_err = None
    if _CALLABLE is not False:
        for attempt in range(3):
            try:
                if _CALLABLE is None:
                    from concourse._compat import axon_active

                    if not axon_active():
                        raise RuntimeError("axon not active; use native path")
                    _CALLABLE = _build_callable()
                results = _CALLABLE(in_maps)
                break
            except Exception as e:
                last_err = e
                results = None
                if attempt < 2:
                    _time.sleep(20 * (attempt + 1))
        if results is None:
            _CALLABLE = False
    if results is None:
        from concourse.bass_utils import run_bass_kernel_spmd

        for attempt in range(3):
            try:
                results = run_bass_kernel_spmd(
                    _get_nc(), in_maps, core_ids=list(range(N_CORES))
                ).results
                break
            except Exception as e:
                last_err = e
                results = None
                if attempt < 2:
                    _time.sleep(20 * (attempt + 1))
    if results is None:
        raise last_err
    out = np.concatenate([r["out"] for r in results], axis=0)
    return out, results


def kernel(**inputs) -> np.ndarray:
    out, _ = _run(inputs)
    return out


# revision 13
# speedup vs baseline: 1.0291x; 1.0145x over previous
"""AlignmentModule on 8 Trainium2 cores — fp8 DoubleRow rewrite.

Data-parallel over batch (2 per core). All matmuls run as fp8e4 DoubleRow
(2 K-planes per pass, 0.5 cycles/row = 4x fp32r): conv stacks t1,t2 / f1,f2,
the Gram matmul H=G.ft2, the text-side-absorbed f3 (u = W3^T tx2m — the 1x1
f3 conv is algebraically moved to the 4x-smaller text side; f2 norms come from
fh = ft2*H read straight off H's PSUM), the score cross ft2^T.u, and a K=1
augmented DR that adds f2[f] + (t2[t]-c0) rank-2 terms into the same PSUM.

Norm rows: f2row/t2row are ones-weight DR matmuls into PSUM row slots
(partitions 0/32/64 + bank2), evicted same-partition to fp8 and DMA-gathered
to the partition-0 aug operand rows (engines cannot cross partitions; DMA can).

Tail per f-tile: ACT sqrt (psum pair -> fp16 dist), ACT exp(14-dist) with
f32 row-accum (a tunable number of pairs instead run paired-exp + DVE reduce),
one ACT ln per batch, then outp = (prior_f16 - cc) - dist on Pool, DMA out in
4-tile quads. Engine assignment of evictions is tuned: Pool takes t1/t2/u +
outp, DVE takes f1/f2/fh/txsq/rows, ACT takes sqrt/exp/ln.

Host pre-quantizes inputs/weights to fp8 (power-of-2 scales, ranges asserted)
and ships them as uint8 bits; fp16 prior. Total rel err vs the f32 reference
is ~6.5e-4 (validated offline), dominated by fp8 conv activations and the
fp16 prior.
"""

import numpy as np

import bass_rust as _bass_rust
import concourse.bass as bass
import concourse.mybir as mybir
from concourse.tile import TileContext

F32 = mybir.dt.float32
F16 = mybir.dt.float16
BF16 = mybir.dt.bfloat16
F8 = mybir.dt.float8e4
U8 = mybir.dt.uint8
AF = mybir.ActivationFunctionType
OP = mybir.AluOpType
AX = mybir.AxisListType
DR = mybir.MatmulPerfMode.DoubleRow

B, T_TEXT, T_FEATS, ADIM, ODIM = 16, 512, 2048, 256, 80
N_CORES = 8
B_LOC = B // N_CORES
TT, TF = T_TEXT, T_FEATS
NT = TF // 512   # 4 feats chunks of 512
NF = TF // 128   # 16 f-tiles per batch

# ---- fixed power-of-2 scales (validated in opt/sim_numerics2.py) ----
SX = 16.0
S_TW1, S_TW2 = 1024.0, 512.0
S_FW1, S_FW2, S_W3 = 512.0, 1024.0, 512.0
S_G = 64.0
A1 = 1.0 / 512.0       # tx1 evict; tile = 32*true
A2 = 1.0 / 2048.0      # tx2m evict; tile = 8*true
AF1 = 1.0 / 512.0      # ft1 tile = 16*true
AF2 = 1.0 / 8192.0     # ft2 tile = 2*true
BH = 1.0 / 32.0        # fh evict scalar
BU = 1.0 / 4096.0      # u evict; u tile = -1*true(W3^T tx2)
ONES2_F2 = 1.0 / 8.0   # f2row ones-weight = 1/(S_G*s_ft2^2*BH)
T2Q_MUL = 1.0 / 64.0   # t2row evict mult = 1/s_tx2m^2
C0 = 192.0             # t2q offset; sqrt bias adds it back
M_SHIFT = 14.0
SQ_BIAS = C0           # dist = sqrt(psum + C0)

PAIRED_SET = (0,)  # score pairs using paired-exp + DVE reduce
# engine homes for evictions: "dve" | "pool" | "act"(relu/identity/square ok)
# NOTE: GPSIMD (pool) cannot access PSUM on HW — psum-evictions are dve/act only.
HOMES = {
    "t1e": "act", "t2e": "act", "ue": "act",
    "f1e": "dve", "f2e": "dve", "fh": "dve",
    "rows": "act", "t2q": "dve", "txsq": "act",
    "outp": "dve",  # pool cannot do TensorScalarPtr on HW
}

WOFF = {}              # wblob free-dim offsets, filled by _pack_weights layout
WBLOB_W = 1536 + 512 + 1024 + 1536 + 512 + 512  # tw1,tw2,fw1,fw2,G,w3u


def _wblob_offsets():
    off, o = {}, 0
    for name, w in (("tw1", 1536), ("tw2", 512), ("fw1", 1024),
                    ("fw2", 1536), ("G", 512), ("w3u", 512)):
        off[name] = o
        o += w
    assert o == WBLOB_W
    return off


WOFF = _wblob_offsets()


def _split_excess_waits(nc, limit=1):
    """walrus CoreV3 CTRL codegen rejects >1 sync-wait per instruction.
    Hoist excess waits onto preceding NOPs on the same engine."""
    ctr = 0
    for f in nc.m.functions:
        for bb in f.blocks:
            insts = bb.instructions
            idx = 0
            while idx < len(insts):
                ins = insts[idx]
                si = ins.sync_info
                if si is not None and len(si.on_wait) > limit:
                    waits = list(si.on_wait)
                    extra, keep = waits[:-limit], waits[-limit:]
                    si.on_wait = keep
                    pos = idx
                    for j in range(0, len(extra), limit):
                        nop = mybir.InstNoOp(name=f"waitsplit_{ctr}", ins=[], outs=[])
                        ctr += 1
                        nop.engine = ins.engine
                        nop.sync_info = mybir.SyncInfo(
                            on_wait=extra[j : j + limit], on_update=[]
                        )
                        insts.insert(pos, nop)
                        pos += 1
                        idx += 1
                idx += 1
    return ctr


def _beta_binomial_prior():
    from scipy.special import gammaln

    T, N = T_FEATS, T_TEXT
    a = np.arange(1, T + 1, dtype=np.float64)[:, None]
    b = (T - np.arange(1, T + 1, dtype=np.float64) + 1.0)[:, None]
    k = np.arange(N, dtype=np.float64)[None, :]
    n = float(N)

    def betaln(x, y):
        return gammaln(x) + gammaln(y) - gammaln(x + y)

    logp = (
        gammaln(n + 1.0) - gammaln(k + 1.0) - gammaln(n - k + 1.0)
        + betaln(k + a, n - k + b) - betaln(a, b)
    )
    return logp.astype(np.float32)


def _build_nc():
    nc = bass.Bass(name="alignment")

    textT = nc.dram_tensor("textT", [B_LOC, ADIM, TT], U8, kind="ExternalInput")
    featsT = nc.dram_tensor("featsT", [B_LOC, ODIM, TF], U8, kind="ExternalInput")
    wblob = nc.dram_tensor("wblob", [128, WBLOB_W], U8, kind="ExternalInput")
    onesrow = nc.dram_tensor("onesrow", [1, TF], U8, kind="ExternalInput")
    priorD = nc.dram_tensor("prior", [TF, TT], F16, kind="ExternalInput")
    outD = nc.dram_tensor("out", [B_LOC, TF, TT], F32, kind="ExternalOutput")

    with TileContext(nc) as tc:
        with (
            tc.tile_pool(name="const", bufs=1) as const,
            tc.tile_pool(name="inp", bufs=2) as inp,
            tc.tile_pool(name="actp", bufs=2) as actp,
            tc.tile_pool(name="rowp", bufs=2) as rowp,
            tc.tile_pool(name="distp", bufs=17) as distp,
            tc.tile_pool(name="ep", bufs=3) as ep,
            tc.tile_pool(name="outq", bufs=3) as outqp,
            tc.tile_pool(name="ppA", bufs=2, space="PSUM") as ppA,
            tc.tile_pool(name="ppS", bufs=2, space="PSUM") as ppS,
        ):
            # ---- constants / weights ----
            wb = const.tile([128, WBLOB_W], F8)
            nc.sync.dma_start(out=wb[:].bitcast(U8), in_=wblob[:])
            ones1 = const.tile([128, 1], F8)
            nc.vector.memset(ones1[:], ONES2_F2)
            onesb = const.tile([128, 1], BF16)
            nc.vector.memset(onesb[:], 1.0)
            b_sq = const.tile([128, 1], F32)
            nc.vector.memset(b_sq[:], SQ_BIAS)
            b_m = const.tile([128, 1], F32)
            nc.vector.memset(b_m[:], M_SHIFT)
            prior_sb = const.tile([128, NF, TT], F16)

            def ev_scale_relu(home, out, ps, scale):
                if home == "act":
                    nc.scalar.activation(out, ps, AF.Relu, scale=scale)
                elif home == "split":
                    nc.vector.tensor_scalar(out[:, 0, :], ps[:, 0, :], scale,
                                            0.0, OP.mult, OP.max)
                    nc.gpsimd.tensor_scalar(out[:, 1, :], ps[:, 1, :], scale,
                                            0.0, OP.mult, OP.max)
                else:
                    eng = nc.vector if home == "dve" else nc.gpsimd
                    eng.tensor_scalar(out, ps, scale, 0.0, OP.mult, OP.max)

            def ev_scale(home, out, ps, scale):
                if home == "act":
                    nc.scalar.activation(out, ps, AF.Identity, scale=scale)
                elif home == "split":
                    nc.vector.tensor_scalar(out[:, 0, :], ps[:, 0, :], scale,
                                            None, OP.mult)
                    nc.gpsimd.tensor_scalar(out[:, 1, :], ps[:, 1, :], scale,
                                            None, OP.mult)
                else:
                    eng = nc.vector if home == "dve" else nc.gpsimd
                    eng.tensor_scalar(out, ps, scale, None, OP.mult)

            def wap(name, idx, planes=2, width=128):
                base = WOFF[name] + idx * planes * width
                return wb[:, base : base + planes * width].rearrange(
                    "p (c w) -> p c w", c=planes
                )

            def load_tx0(b):
                tx0 = inp.tile([128, 2, TT + 2], F8, tag="tx0")
                nc.vector.memset(tx0[:, :, 0:1], 0.0)
                nc.vector.memset(tx0[:, :, TT + 1 : TT + 2], 0.0)
                nc.sync.dma_start(
                    out=tx0[:, :, 1 : TT + 1].bitcast(U8),
                    in_=textT[b].rearrange("(c p) t -> p c t", p=128),
                )
                return tx0

            def load_ft0(b):
                ft0 = inp.tile([ODIM, TF + 3], F8, tag="ft0")
                nc.vector.memset(ft0[:, 0:1], 0.0)
                nc.vector.memset(ft0[:, TF + 1 : TF + 3], 0.0)
                nc.sync.dma_start(out=ft0[:, 1 : TF + 1].bitcast(U8), in_=featsT[b])
                return ft0

            def batch_ctx(b, tx0, ft0):
                """Allocate per-batch tiles and return the conv step list plus
                the tile handles the tail needs."""
                tx1 = actp.tile([128, 2, TT], F8, tag="tx1")
                tx2m = actp.tile([128, 2, TT], F8, tag="tx2m")
                txsq = actp.tile([128, 2, TT], BF16, tag="txsq")
                ft1 = actp.tile([128, 2, TF + 2], F8, tag="ft1")
                ft2 = actp.tile([128, 2, TF], F8, tag="ft2")
                fh = actp.tile([128, 2, TF], F8, tag="fh")
                u = actp.tile([128, 2, TT], F8, tag="u")
                augw = rowp.tile([1, 2, TF], F8, tag="augw")
                augx = rowp.tile([1, 2, TT], F8, tag="augx")

                box = {}
                steps = []

                def pads():
                    nc.vector.memset(ft1[:, :, 0:1], 0.0)
                    nc.vector.memset(ft1[:, :, TF + 1 : TF + 2], 0.0)
                    nc.sync.dma_start(out=augw[0:1, 1, :].bitcast(U8),
                                      in_=onesrow[0:1, :])
                    nc.sync.dma_start(out=augx[0:1, 0, :].bitcast(U8),
                                      in_=onesrow[0:1, 0:TT])

                def tap_pair(start):
                    a = ft0[0:ODIM, start : start + 512]
                    w = a.copy()
                    w.ap = _bass_rust.VecI64Pair([list(a.ap[0]), [1, 2], [1, 512]])
                    return w

                def f1_mm(n):
                    def f():
                        ps = ppA.tile([128, 2, 512], F32, tag="psA")
                        box[("f1", n)] = ps
                        for m in range(2):
                            for k in range(3):
                                d, pl = divmod(k, 2)
                                base = WOFF["fw1"] + (d * 2 + m) * 256 + pl * 128
                                lhs = wb[0:ODIM, base : base + 128]
                                nc.tensor.matmul(
                                    ps[:, m, :], lhs,
                                    ft0[0:ODIM, n * 512 + k : n * 512 + k + 512],
                                    start=(k == 0), stop=(k == 2),
                                )
                    return f

                def f1_ev(n):
                    def f():
                        ps = box.pop(("f1", n))
                        ev_scale_relu(HOMES["f1e"],
                                      ft1[:, :, 1 + n * 512 : 1 + (n + 1) * 512],
                                      ps[:, :, :], AF1)
                    return f

                def t1_mm():
                    ps = ppA.tile([128, 2, 512], F32, tag="psA")
                    box["t1"] = ps
                    for m in range(2):
                        for k in range(3):
                            nc.tensor.matmul(
                                ps[:, m, :], wap("tw1", k * 2 + m),
                                tx0[:, :, k : k + TT],
                                start=(k == 0), stop=(k == 2), perf_mode=DR,
                            )

                def t1_ev():
                    ps = box.pop("t1")
                    ev_scale_relu(HOMES["t1e"], tx1[:, :, :], ps[:, :, :], A1)

                def t2_mm():
                    ps = ppA.tile([128, 2, 512], F32, tag="psA")
                    box["t2"] = ps
                    for m in range(2):
                        nc.tensor.matmul(
                            ps[:, m, :], wap("tw2", m), tx1[:, :, :],
                            start=True, stop=True, perf_mode=DR,
                        )

                def t2_ev():
                    ps = box["t2"]
                    ev_scale(HOMES["t2e"], tx2m[:, :, :], ps[:, :, :], A2)

                def txsq_f():
                    ps = box["t2"]
                    if HOMES["txsq"] == "act":
                        nc.scalar.activation(txsq[:, :, :], ps[:, :, :], AF.Square,
                                             scale=A2)
                    else:
                        eng = nc.vector if HOMES["txsq"] == "dve" else nc.gpsimd
                        eng.tensor_tensor(txsq[:, :, :], tx2m[:, :, :],
                                          tx2m[:, :, :], OP.mult)

                def t2row_mm():
                    psT = box["t2"]  # reuse t2 psum tile (already evicted)
                    for cc_ in range(2):
                        nc.tensor.matmul(psT[0:1, 0, :], onesb[:], txsq[:, cc_, :],
                                         start=(cc_ == 0), stop=(cc_ == 1))

                def t2q_ev():
                    psT = box.pop("t2")
                    _e = {"dve": nc.vector, "pool": nc.gpsimd}[HOMES["t2q"]]
                    _e.tensor_scalar(
                        augx[0:1, 1, :], psT[0:1, 0, :],
                        T2Q_MUL, C0, OP.mult, OP.subtract,
                    )

                def u_mm():
                    ps = ppA.tile([128, 2, 512], F32, tag="psA")
                    box["u"] = ps
                    for m in range(2):
                        nc.tensor.matmul(
                            ps[:, m, :], wap("w3u", m), tx2m[:, :, :],
                            start=True, stop=True, perf_mode=DR,
                        )

                def u_ev():
                    ps = box.pop("u")
                    ev_scale(HOMES["ue"], u[:, :, :], ps[:, :, :], -BU)

                def f2_mm(n):
                    def f():
                        ps = ppA.tile([128, 2, 512], F32, tag="psA")
                        box[("f2", n)] = ps
                        for m in range(2):
                            for k in range(3):
                                nc.tensor.matmul(
                                    ps[:, m, :], wap("fw2", k * 2 + m),
                                    ft1[:, :, n * 512 + k : n * 512 + k + 512],
                                    start=(k == 0), stop=(k == 2), perf_mode=DR,
                                )
                    return f

                def f2_ev(n):
                    def f():
                        ps = box.pop(("f2", n))
                        ev_scale_relu(HOMES["f2e"],
                                      ft2[:, :, n * 512 : (n + 1) * 512],
                                      ps[:, :, :], AF2)
                    return f

                def h_mm(n):
                    def f():
                        ps = ppA.tile([128, 2, 512], F32, tag="psA")
                        box[("h", n)] = ps
                        for m in range(2):
                            nc.tensor.matmul(
                                ps[:, m, :], wap("G", m),
                                ft2[:, :, n * 512 : (n + 1) * 512],
                                start=True, stop=True, perf_mode=DR,
                            )
                    return f

                def fh_ev(n):
                    def f():
                        ps = box[("h", n)]
                        sl = slice(n * 512, (n + 1) * 512)
                        if HOMES["fh"] == "split":
                            nc.vector.scalar_tensor_tensor(
                                fh[:, 0, sl], ps[:, 0, :], BH, ft2[:, 0, sl],
                                OP.mult, OP.mult)
                            nc.gpsimd.scalar_tensor_tensor(
                                fh[:, 1, sl], ps[:, 1, :], BH, ft2[:, 1, sl],
                                OP.mult, OP.mult)
                        else:
                            eng = nc.vector if HOMES["fh"] == "dve" else nc.gpsimd
                            eng.scalar_tensor_tensor(
                                fh[:, :, sl], ps[:, :, :], BH, ft2[:, :, sl],
                                OP.mult, OP.mult)
                    return f

                def f2row_mm(n):
                    def f():
                        psH = box[("h", n)]  # reuse after fh_ev consumed it
                        for c_ in range(2):
                            nc.tensor.matmul(
                                psH[0:1, 0, :], ones1[:, :],
                                fh[:, c_, n * 512 : (n + 1) * 512],
                                start=(c_ == 0), stop=(c_ == 1),
                            )
                    return f

                def f2row_ev(n):
                    def f():
                        psH = box.pop(("h", n))
                        ev_scale(HOMES["rows"],
                                 augw[0:1, 0, n * 512 : (n + 1) * 512],
                                 psH[0:1, 0, :], 1.0)
                    return f

                parts = dict(
                    pads=pads, f1_mm=f1_mm, f1_ev=f1_ev, t1_mm=t1_mm, t1_ev=t1_ev,
                    t2_mm=t2_mm, t2_ev=t2_ev, txsq=txsq_f, t2row=t2row_mm,
                    t2q=t2q_ev, u_mm=u_mm, u_ev=u_ev, f2_mm=f2_mm, f2_ev=f2_ev,
                    h_mm=h_mm, fh_ev=fh_ev, f2row_mm=f2row_mm, f2row_ev=f2row_ev,
                )
                tiles = dict(tx2m=tx2m, ft2=ft2, u=u, augw=augw, augx=augx)
                return parts, tiles

            def tail_ctx(b, tiles):
                ft2, u = tiles["ft2"], tiles["u"]
                augw, augx = tiles["augw"], tiles["augx"]
                ssum = rowp.tile([128, NF], F32, tag="ssum")
                lns = rowp.tile([128, NF], F32, tag="lns")
                dist_tiles = {}

                def pair(j):
                    def f():
                        ps = ppS.tile([128, 2, 512], F32, tag="psS")
                        for h in range(2):
                            i = 2 * j + h
                            nc.tensor.matmul(
                                ps[:, h, :],
                                ft2[:, :, i * 128 : (i + 1) * 128],
                                u[:, :, :], start=True, stop=False, perf_mode=DR,
                            )
                            nc.tensor.matmul(
                                ps[:, h, :],
                                augw[0:1, :, i * 128 : (i + 1) * 128],
                                augx[0:1, :, :], start=False, stop=True,
                                perf_mode=DR,
                            )
                        dist = distp.tile([128, 2, 512], F16, tag="dist")
                        nc.scalar.activation(dist[:], ps[:], AF.Sqrt, bias=b_sq[:])
                        dist_tiles[j] = dist
                    return f

                def exp_pair(j):
                    def f():
                        dist = dist_tiles[j]
                        if j in PAIRED_SET:
                            e = ep.tile([128, 2, 512], BF16, tag="e")
                            nc.scalar.activation(e[:], dist[:], AF.Exp,
                                                 scale=-1.0, bias=b_m[:])
                            nc.vector.tensor_reduce(
                                ssum[:, 2 * j : 2 * j + 2], e[:], AX.X, OP.add
                            )
                        else:
                            for h in range(2):
                                i = 2 * j + h
                                e = ep.tile([128, 2, 512], BF16, tag="e")
                                nc.scalar.activation(
                                    e[:, 0, :], dist[:, h, :], AF.Exp,
                                    scale=-1.0, bias=b_m[:],
                                    accum_out=ssum[:, i : i + 1],
                                )
                    return f

                def ln_half(h):
                    def f():
                        nc.scalar.activation(
                            lns[:, 8 * h : 8 * h + 8], ssum[:, 8 * h : 8 * h + 8],
                            AF.Ln, scale=float(np.exp(-M_SHIFT)),
                        )
                    return f

                def quad(qi, split=False):
                    def f():
                        oq = outqp.tile([128, 4, 512], F32, tag="outq")
                        for q in range(4):
                            i = 4 * qi + q
                            dist = dist_tiles[i // 2]
                            if HOMES["outp"] == "alt":
                                eng = nc.gpsimd if q % 2 == 0 else nc.vector
                            else:
                                eng = {"pool": nc.gpsimd, "dve": nc.vector}[HOMES["outp"]]
                            eng.scalar_tensor_tensor(
                                oq[:, q, :], prior_sb[:, i, :], lns[:, i : i + 1],
                                dist[:, i % 2, :], OP.subtract, OP.subtract,
                            )
                            if split and q % 2 == 1:
                                nc.sync.dma_start(
                                    out=outD[b, 512 * qi + 256 * (q // 2) :
                                             512 * qi + 256 * (q // 2) + 256,
                                             :].rearrange("(q p) t -> p q t", p=128),
                                    in_=oq[:, q - 1 : q + 1, :],
                                )
                        if not split:
                            nc.sync.dma_start(
                                out=outD[b, 512 * qi : 512 * (qi + 1), :].rearrange(
                                    "(q p) t -> p q t", p=128
                                ),
                                in_=oq[:, :, :],
                            )
                    return f

                return pair, exp_pair, ln_half, quad

            # ================= emission =================
            tx0_0 = load_tx0(0)
            ft0_0 = load_ft0(0)
            tx0_1 = load_tx0(1)
            ft0_1 = load_ft0(1)
            nc.sync.dma_start(
                out=prior_sb[:], in_=priorD.rearrange("(i p) t -> p i t", p=128)
            )

            def batch_steps(b, tx0, ft0):
                p, tiles = batch_ctx(b, tx0, ft0)
                t = tail_ctx(b, tiles)
                pair, expp, lnh, quad = t
                return [
                    p["pads"],
                    p["f1_mm"](0), p["t1_mm"], p["f1_ev"](0), p["t1_ev"],
                    p["f1_mm"](1), p["t2_mm"], p["f1_ev"](1), p["t2_ev"],
                    p["f2_mm"](0), p["txsq"], p["f2_ev"](0),
                    p["t2row"], p["h_mm"](0), p["t2q"], p["fh_ev"](0),
                    p["f1_mm"](2), p["u_mm"], p["f1_ev"](2), p["u_ev"],
                    p["f2row_mm"](0), p["f2row_ev"](0),
                    pair(0),
                    p["f2_mm"](1), p["f1_mm"](3),
                    pair(1), p["f2_ev"](1), expp(0), p["f1_ev"](3),
                    p["h_mm"](1), p["fh_ev"](1),
                    p["f2row_mm"](1), p["f2row_ev"](1),
                    pair(2), p["f2_mm"](2), expp(1), p["f2_ev"](2),
                    pair(3), p["h_mm"](2), expp(2), p["fh_ev"](2),
                    p["f2row_mm"](2), p["f2row_ev"](2),
                    pair(4), p["f2_mm"](3), expp(3), p["f2_ev"](3),
                    lnh(0), quad(0),
                    pair(5), p["h_mm"](3), expp(4), p["fh_ev"](3),
                    p["f2row_mm"](3), p["f2row_ev"](3),
                    quad(1),
                    pair(6), expp(5), pair(7), expp(6), expp(7),
                    lnh(1), quad(2, split=True), quad(3, split=True),
                ]

            steps0 = batch_steps(0, tx0_0, ft0_0)
            steps1 = batch_steps(1, tx0_1, ft0_1)
            import os
            STAG = int(os.environ.get("KV2_STAGGER", "24"))
            merged = []
            i0 = i1 = 0
            # emit STAG steps of batch0 first, then alternate
            while i0 < len(steps0) or i1 < len(steps1):
                if i0 < len(steps0):
                    merged.append(steps0[i0]); i0 += 1
                if i0 >= STAG and i1 < len(steps1):
                    merged.append(steps1[i1]); i1 += 1
            for s in merged:
                s()

    _split_excess_waits(nc)
    return nc


_NC = None


def _get_nc():
    global _NC
    if _NC is None:
        _NC = _build_nc()
    return _NC


def _q8(x, scale, limit=230.0):
    import ml_dtypes
    y = np.asarray(x, np.float32) * scale
    m = np.abs(y).max()
    assert m < limit, f"fp8 range exceeded: {m} * (scale {scale})"
    return y.astype(ml_dtypes.float8_e4m3)


def _prep_inputs(text, feats, t_w1, t_b1, t_w2, t_b2,
                 f_w1, f_b1, f_w2, f_b2, f_w3, f_b3):
    for bias in (t_b1, t_b2, f_b1, f_b2, f_b3):
        assert not np.asarray(bias).any(), "kernel assumes zero biases (per spec)"
    c = np.ascontiguousarray
    f4 = np.float32

    textT = _q8(c(np.asarray(text, f4).transpose(0, 2, 1)), SX)    # [B,256,512]
    featsT = _q8(c(np.asarray(feats, f4).transpose(0, 2, 1)), SX)  # [B,80,2048]

    # wblob [128, WBLOB_W] fp8: per lhsT (k/m) block of [p, 2, 128]
    blob = np.zeros((128, WBLOB_W), np.float32)

    def put(name, idx, arr):  # arr [128, 2, 128] f32 (pre-scale applied)
        base = WOFF[name] + idx * 256
        blob[:, base : base + 256] = arr.reshape(128, 256)

    tw1 = np.asarray(t_w1, f4).transpose(2, 1, 0)  # [3, cin, cout]
    for k in range(3):
        for m in range(2):
            a = tw1[k].reshape(2, 128, 256)[:, :, m * 128 : (m + 1) * 128]
            put("tw1", k * 2 + m, a.transpose(1, 0, 2) * S_TW1)
    tw2 = np.asarray(t_w2, f4)[:, :, 0].T  # [cin, cout]
    for m in range(2):
        a = tw2.reshape(2, 128, 256)[:, :, m * 128 : (m + 1) * 128]
        put("tw2", m, a.transpose(1, 0, 2) * S_TW2)
    fw1 = np.asarray(f_w1, f4).transpose(2, 1, 0)  # [3, 80, 256]
    for d in range(2):
        for m in range(2):
            a = np.zeros((128, 2, 128), np.float32)
            a[:80, 0] = fw1[2 * d][:, m * 128 : (m + 1) * 128]
            if 2 * d + 1 < 3:
                a[:80, 1] = fw1[2 * d + 1][:, m * 128 : (m + 1) * 128]
            put("fw1", d * 2 + m, a * S_FW1)
    fw2 = np.asarray(f_w2, f4).transpose(2, 1, 0)
    for k in range(3):
        for m in range(2):
            a = fw2[k].reshape(2, 128, 256)[:, :, m * 128 : (m + 1) * 128]
            put("fw2", k * 2 + m, a.transpose(1, 0, 2) * S_FW2)
    W3 = np.asarray(f_w3, f4)[:, :, 0]  # [cout, cin]
    G = (W3.T @ W3).astype(np.float32)
    for m in range(2):
        a = G.reshape(2, 128, 256)[:, :, m * 128 : (m + 1) * 128]
        put("G", m, a.transpose(1, 0, 2) * S_G)
    for m in range(2):  # w3u lhsT[c, d]: W3 itself
        a = W3.reshape(2, 128, 256)[:, :, m * 128 : (m + 1) * 128]
        put("w3u", m, a.transpose(1, 0, 2) * S_W3)

    m = np.abs(blob).max()
    assert m < 230.0, f"wblob fp8 range exceeded: {m}"
    import ml_dtypes
    blob8 = blob.astype(ml_dtypes.float8_e4m3)

    import ml_dtypes as _mld
    ones8 = np.ones((1, TF), _mld.float8_e4m3)
    shared = {
        "wblob": blob8.view(np.uint8),
        "onesrow": ones8.view(np.uint8),
        "prior": _beta_binomial_prior().astype(np.float16),
    }
    in_maps = []
    for core in range(N_CORES):
        mcore = dict(shared)
        mcore["textT"] = c(textT[core * B_LOC : (core + 1) * B_LOC]).view(np.uint8)
        mcore["featsT"] = c(featsT[core * B_LOC : (core + 1) * B_LOC]).view(np.uint8)
        in_maps.append(mcore)
    return in_maps


_CALLABLE = None


def _build_callable():
    """Compile once; return fn(in_maps) -> per-core output dicts (axon path)."""
    import jax
    import jax.numpy as jnp
    from jax.sharding import Mesh, NamedSharding, PartitionSpec
    from jax.experimental.shard_map import shard_map
    from concourse.bass2jax import (
        _bass_exec_p,
        install_neuronx_cc_hook,
        partition_id_tensor,
    )

    nc = _get_nc()
    install_neuronx_cc_hook()
    partition_name = nc.partition_id_tensor.name if nc.partition_id_tensor else None
    in_names, out_names, out_avals, zero_shapes = [], [], [], []
    for alloc in nc.m.functions[0].allocations:
        if not isinstance(alloc, mybir.MemoryLocationSet):
            continue
        name = alloc.memorylocations[0].name
        if alloc.kind == "ExternalInput":
            if name != partition_name:
                in_names.append(name)
        elif alloc.kind == "ExternalOutput":
            shape = tuple(alloc.tensor_shape)
            dtype = mybir.dt.np(alloc.dtype)
            out_names.append(name)
            out_avals.append(jax.core.ShapedArray(shape, dtype))
            zero_shapes.append(((N_CORES * shape[0],) + shape[1:], dtype))
    n_params = len(in_names)
    n_outs = len(out_avals)
    all_in_names = list(in_names) + out_names
    if partition_name is not None:
        all_in_names.append(partition_name)
    donate = tuple(range(n_params, n_params + n_outs))

    def _body(*args):
        operands = list(args)
        if partition_name is not None:
            operands.append(partition_id_tensor())
        outs = _bass_exec_p.bind(
            *operands,
            out_avals=tuple(out_avals),
            in_names=tuple(all_in_names),
            out_names=tuple(out_names),
            lowering_input_output_aliases=(),
            sim_require_finite=True,
            sim_require_nnan=True,
            nc=nc,
        )
        return tuple(outs)

    devices = jax.devices()[:N_CORES]
    mesh = Mesh(np.asarray(devices), ("core",))
    fn = jax.jit(
        shard_map(
            _body,
            mesh=mesh,
            in_specs=(PartitionSpec("core"),) * (n_params + n_outs),
            out_specs=(PartitionSpec("core"),) * n_outs,
            check_rep=False,
        ),
        donate_argnums=donate,
        keep_unused=True,
    )
    sharding = NamedSharding(mesh, PartitionSpec("core"))
    zfn = jax.jit(
        lambda: tuple(jnp.zeros(s, d) for s, d in zero_shapes),
        out_shardings=tuple(sharding for _ in zero_shapes),
    )

    def call(in_maps):
        concat_in = [
            np.concatenate([np.asarray(in_maps[c][n]) for c in range(N_CORES)], axis=0)
            for n in in_names
        ]
        out_arrs = fn(*concat_in, *zfn())
        return [
            {
                name: np.asarray(out_arrs[i]).reshape(
                    N_CORES, *out_avals[i].shape
                )[c]
                for i, name in enumerate(out_names)
            }
            for c in range(N_CORES)
        ]

    return call


def _run(inputs, **kw):
    global _CALLABLE
    import time as _time

    in_maps = _prep_inputs(
        inputs["text"], inputs["feats"],
        inputs["t_w1"], inputs["t_b1"], inputs["t_w2"], inputs["t_b2"],
        inputs["f_w1"], inputs["f_b1"], inputs["f_w2"], inputs["f_b2"],
        inputs["f_w3"], inputs["f_b3"],
    )
    results = None
    last_err = None
    if _CALLABLE is not False:
        for attempt in range(3):
            try:
                if _CALLABLE is None:
                    from concourse._compat import axon_active

                    if not axon_active():
                        raise RuntimeError("axon not active; use native path")
                    _CALLABLE = _build_callable()
                results = _CALLABLE(in_maps)
                break
            except Exception as e:
                last_err = e
                results = None
                if attempt < 2:
                    _time.sleep(20 * (attempt + 1))
        if results is None:
            _CALLABLE = False
    if results is None:
        from concourse.bass_utils import run_bass_kernel_spmd

        for attempt in range(3):
            try:
                results = run_bass_kernel_spmd(
                    _get_nc(), in_maps, core_ids=list(range(N_CORES))
                ).results
                break
            except Exception as e:
                last_err = e
                results = None
                if attempt < 2:
                    _time.sleep(20 * (attempt + 1))
    if results is None:
        raise last_err
    out = np.concatenate([r["out"] for r in results], axis=0)
    return out, results


def kernel(**inputs) -> np.ndarray:
    out, _ = _run(inputs)
    return out


# revision 14
# speedup vs baseline: 1.0481x; 1.0185x over previous
"""AlignmentModule on 8 Trainium2 cores — fp8 DoubleRow rewrite.

Data-parallel over batch (2 per core). All matmuls run as fp8e4 DoubleRow
(2 K-planes per pass, 0.5 cycles/row = 4x fp32r): conv stacks t1,t2 / f1,f2,
the Gram matmul H=G.ft2, the text-side-absorbed f3 (u = W3^T tx2m — the 1x1
f3 conv is algebraically moved to the 4x-smaller text side; f2 norms come from
fh = ft2*H read straight off H's PSUM), the score cross ft2^T.u, and a K=1
augmented DR that adds f2[f] + (t2[t]-c0) rank-2 terms into the same PSUM.

Norm rows: f2row/t2row are ones-weight DR matmuls into PSUM row slots
(partitions 0/32/64 + bank2), evicted same-partition to fp8 and DMA-gathered
to the partition-0 aug operand rows (engines cannot cross partitions; DMA can).

Tail per f-tile: ACT sqrt (psum pair -> fp16 dist), ACT exp(14-dist) with
f32 row-accum (a tunable number of pairs instead run paired-exp + DVE reduce),
one ACT ln per batch, then outp = (prior_f16 - cc) - dist on Pool, DMA out in
4-tile quads. Engine assignment of evictions is tuned: Pool takes t1/t2/u +
outp, DVE takes f1/f2/fh/txsq/rows, ACT takes sqrt/exp/ln.

Host pre-quantizes inputs/weights to fp8 (power-of-2 scales, ranges asserted)
and ships them as uint8 bits; fp16 prior. Total rel err vs the f32 reference
is ~6.5e-4 (validated offline), dominated by fp8 conv activations and the
fp16 prior.
"""

import numpy as np

import bass_rust as _bass_rust
import concourse.bass as bass
import concourse.mybir as mybir
from concourse.tile import TileContext

F32 = mybir.dt.float32
F16 = mybir.dt.float16
BF16 = mybir.dt.bfloat16
F8 = mybir.dt.float8e4
U8 = mybir.dt.uint8
AF = mybir.ActivationFunctionType
OP = mybir.AluOpType
AX = mybir.AxisListType
DR = mybir.MatmulPerfMode.DoubleRow

B, T_TEXT, T_FEATS, ADIM, ODIM = 16, 512, 2048, 256, 80
N_CORES = 8
B_LOC = B // N_CORES
TT, TF = T_TEXT, T_FEATS
NT = TF // 512   # 4 feats chunks of 512
NF = TF // 128   # 16 f-tiles per batch

# ---- fixed power-of-2 scales (validated in opt/sim_numerics2.py) ----
SX = 16.0
S_TW1, S_TW2 = 1024.0, 512.0
S_FW1, S_FW2, S_W3 = 512.0, 1024.0, 512.0
S_G = 64.0
A1 = 1.0 / 512.0       # tx1 evict; tile = 32*true
A2 = 1.0 / 2048.0      # tx2m evict; tile = 8*true
AF1 = 1.0 / 512.0      # ft1 tile = 16*true
AF2 = 1.0 / 8192.0     # ft2 tile = 2*true
BH = 1.0 / 32.0        # fh evict scalar
BU = 1.0 / 4096.0      # u evict; u tile = -1*true(W3^T tx2)
ONES2_F2 = 1.0 / 8.0   # f2row ones-weight = 1/(S_G*s_ft2^2*BH)
T2Q_MUL = 1.0 / 64.0   # t2row evict mult = 1/s_tx2m^2
C0 = 192.0             # t2q offset; sqrt bias adds it back
M_SHIFT = 14.0
SQ_BIAS = C0           # dist = sqrt(psum + C0)

PAIRED_SET = (0,)  # score pairs using paired-exp + DVE reduce
# engine homes for evictions: "dve" | "pool" | "act"(relu/identity/square ok)
# NOTE: GPSIMD (pool) cannot access PSUM on HW — psum-evictions are dve/act only.
HOMES = {
    "t1e": "act", "t2e": "act", "ue": "act",
    "f1e": "dve", "f2e": "dve", "fh": "dve",
    "rows": "act", "t2q": "dve", "txsq": "act",
    "outp": "dve",  # pool cannot do TensorScalarPtr on HW
}

WOFF = {}              # wblob free-dim offsets, filled by _pack_weights layout
WBLOB_W = 1536 + 512 + 1024 + 1536 + 512 + 512  # tw1,tw2,fw1,fw2,G,w3u


def _wblob_offsets():
    off, o = {}, 0
    for name, w in (("tw1", 1536), ("tw2", 512), ("fw1", 1024),
                    ("fw2", 1536), ("G", 512), ("w3u", 512)):
        off[name] = o
        o += w
    assert o == WBLOB_W
    return off


WOFF = _wblob_offsets()


def _split_excess_waits(nc, limit=1):
    """walrus CoreV3 CTRL codegen rejects >1 sync-wait per instruction.
    Hoist excess waits onto preceding NOPs on the same engine."""
    ctr = 0
    for f in nc.m.functions:
        for bb in f.blocks:
            insts = bb.instructions
            idx = 0
            while idx < len(insts):
                ins = insts[idx]
                si = ins.sync_info
                if si is not None and len(si.on_wait) > limit:
                    waits = list(si.on_wait)
                    extra, keep = waits[:-limit], waits[-limit:]
                    si.on_wait = keep
                    pos = idx
                    for j in range(0, len(extra), limit):
                        nop = mybir.InstNoOp(name=f"waitsplit_{ctr}", ins=[], outs=[])
                        ctr += 1
                        nop.engine = ins.engine
                        nop.sync_info = mybir.SyncInfo(
                            on_wait=extra[j : j + limit], on_update=[]
                        )
                        insts.insert(pos, nop)
                        pos += 1
                        idx += 1
                idx += 1
    return ctr


def _beta_binomial_prior():
    from scipy.special import gammaln

    T, N = T_FEATS, T_TEXT
    a = np.arange(1, T + 1, dtype=np.float64)[:, None]
    b = (T - np.arange(1, T + 1, dtype=np.float64) + 1.0)[:, None]
    k = np.arange(N, dtype=np.float64)[None, :]
    n = float(N)

    def betaln(x, y):
        return gammaln(x) + gammaln(y) - gammaln(x + y)

    logp = (
        gammaln(n + 1.0) - gammaln(k + 1.0) - gammaln(n - k + 1.0)
        + betaln(k + a, n - k + b) - betaln(a, b)
    )
    return logp.astype(np.float32)


def _build_nc():
    nc = bass.Bass(name="alignment")

    textT = nc.dram_tensor("textT", [B_LOC, ADIM, TT], U8, kind="ExternalInput")
    featsT = nc.dram_tensor("featsT", [B_LOC, ODIM, TF], U8, kind="ExternalInput")
    wblob = nc.dram_tensor("wblob", [128, WBLOB_W], U8, kind="ExternalInput")
    onesrow = nc.dram_tensor("onesrow", [1, TF], U8, kind="ExternalInput")
    priorD = nc.dram_tensor("prior", [TF, TT], F16, kind="ExternalInput")
    outD = nc.dram_tensor("out", [B_LOC, TF, TT], F32, kind="ExternalOutput")

    with TileContext(nc) as tc:
        with (
            tc.tile_pool(name="const", bufs=1) as const,
            tc.tile_pool(name="inp", bufs=2) as inp,
            tc.tile_pool(name="actp", bufs=2) as actp,
            tc.tile_pool(name="rowp", bufs=2) as rowp,
            tc.tile_pool(name="distp", bufs=17) as distp,
            tc.tile_pool(name="ep", bufs=3) as ep,
            tc.tile_pool(name="outq", bufs=3) as outqp,
            tc.tile_pool(name="ppA", bufs=2, space="PSUM") as ppA,
            tc.tile_pool(name="ppS", bufs=2, space="PSUM") as ppS,
        ):
            # ---- constants / weights ----
            wb = const.tile([128, WBLOB_W], F8)
            nc.sync.dma_start(out=wb[:].bitcast(U8), in_=wblob[:])
            ones1 = const.tile([128, 1], F8)
            nc.vector.memset(ones1[:], ONES2_F2)
            onesb = const.tile([128, 1], BF16)
            nc.vector.memset(onesb[:], 1.0)
            b_sq = const.tile([128, 1], F32)
            nc.vector.memset(b_sq[:], SQ_BIAS)
            b_m = const.tile([128, 1], F32)
            nc.vector.memset(b_m[:], M_SHIFT)
            prior_sb = const.tile([128, NF, TT], F16)

            def ev_scale_relu(home, out, ps, scale):
                if home == "act":
                    nc.scalar.activation(out, ps, AF.Relu, scale=scale)
                elif home == "split":
                    nc.vector.tensor_scalar(out[:, 0, :], ps[:, 0, :], scale,
                                            0.0, OP.mult, OP.max)
                    nc.gpsimd.tensor_scalar(out[:, 1, :], ps[:, 1, :], scale,
                                            0.0, OP.mult, OP.max)
                else:
                    eng = nc.vector if home == "dve" else nc.gpsimd
                    eng.tensor_scalar(out, ps, scale, 0.0, OP.mult, OP.max)

            def ev_scale(home, out, ps, scale):
                if home == "act":
                    nc.scalar.activation(out, ps, AF.Identity, scale=scale)
                elif home == "split":
                    nc.vector.tensor_scalar(out[:, 0, :], ps[:, 0, :], scale,
                                            None, OP.mult)
                    nc.gpsimd.tensor_scalar(out[:, 1, :], ps[:, 1, :], scale,
                                            None, OP.mult)
                else:
                    eng = nc.vector if home == "dve" else nc.gpsimd
                    eng.tensor_scalar(out, ps, scale, None, OP.mult)

            def wap(name, idx, planes=2, width=128):
                base = WOFF[name] + idx * planes * width
                return wb[:, base : base + planes * width].rearrange(
                    "p (c w) -> p c w", c=planes
                )

            def load_tx0(b):
                tx0 = inp.tile([128, 2, TT + 2], F8, tag="tx0")
                nc.vector.memset(tx0[:, :, 0:1], 0.0)
                nc.vector.memset(tx0[:, :, TT + 1 : TT + 2], 0.0)
                nc.sync.dma_start(
                    out=tx0[:, :, 1 : TT + 1].bitcast(U8),
                    in_=textT[b].rearrange("(c p) t -> p c t", p=128),
                )
                return tx0

            def load_ft0(b):
                # two copies, plane1 shifted +1 col, so the f1 tap-pair DR reads
                # non-overlapping ifmap planes (overlapping APs wedge the PE)
                ft0 = inp.tile([ODIM, 2, TF + 4], F8, tag="ft0")
                nc.vector.memset(ft0[:, :, 0:2], 0.0)
                nc.vector.memset(ft0[:, :, TF + 1 : TF + 4], 0.0)
                nc.sync.dma_start(out=ft0[:, 0, 1 : TF + 1].bitcast(U8), in_=featsT[b])
                nc.sync.dma_start(out=ft0[:, 1, 2 : TF + 2].bitcast(U8), in_=featsT[b])
                return ft0

            def batch_ctx(b, tx0, ft0):
                """Allocate per-batch tiles and return the conv step list plus
                the tile handles the tail needs."""
                tx1 = actp.tile([128, 2, TT], F8, tag="tx1")
                tx2m = actp.tile([128, 2, TT], F8, tag="tx2m")
                txsq = actp.tile([128, 2, TT], BF16, tag="txsq")
                ft1 = actp.tile([128, 2, TF + 2], F8, tag="ft1")
                ft2 = actp.tile([128, 2, TF], F8, tag="ft2")
                fh = actp.tile([128, 2, TF], F8, tag="fh")
                u = actp.tile([128, 2, TT], F8, tag="u")
                augw = rowp.tile([1, 2, TF], F8, tag="augw")
                augx = rowp.tile([1, 2, TT], F8, tag="augx")

                box = {}
                steps = []

                def pads():
                    nc.vector.memset(ft1[:, :, 0:1], 0.0)
                    nc.vector.memset(ft1[:, :, TF + 1 : TF + 2], 0.0)
                    nc.sync.dma_start(out=augw[0:1, 1, :].bitcast(U8),
                                      in_=onesrow[0:1, :])
                    nc.sync.dma_start(out=augx[0:1, 0, :].bitcast(U8),
                                      in_=onesrow[0:1, 0:TT])

                def tap_pair(start):
                    a = ft0[0:ODIM, start : start + 512]
                    w = a.copy()
                    w.ap = _bass_rust.VecI64Pair([list(a.ap[0]), [1, 2], [1, 512]])
                    return w

                def tap_pair(start):
                    """[80, 2, 512] ifmap: plane0 = ft0 copy0 at col start,
                    plane1 = copy1 at the same col (holding tap start+1).
                    Non-overlapping plane stride TF+4."""
                    a = ft0[0:ODIM, 0, start : start + 512]
                    w = a.copy()
                    w.ap = _bass_rust.VecI64Pair(
                        [list(a.ap[0]), [TF + 6, 2], [1, 512]]
                    )
                    return w

                def f1_mm(n):
                    def f():
                        ps = ppA.tile([128, 2, 512], F32, tag="psA")
                        box[("f1", n)] = ps
                        for m in range(2):
                            for d in range(2):
                                base = WOFF["fw1"] + (d * 2 + m) * 256
                                lhs = wb[0:ODIM, base : base + 256].rearrange(
                                    "p (c w) -> p c w", c=2
                                )
                                nc.tensor.matmul(
                                    ps[:, m, :], lhs, tap_pair(n * 512 + 2 * d),
                                    start=(d == 0), stop=(d == 1), perf_mode=DR,
                                )
                    return f

                def f1_ev(n):
                    def f():
                        ps = box.pop(("f1", n))
                        ev_scale_relu(HOMES["f1e"],
                                      ft1[:, :, 1 + n * 512 : 1 + (n + 1) * 512],
                                      ps[:, :, :], AF1)
                    return f

                def t1_mm():
                    ps = ppA.tile([128, 2, 512], F32, tag="psA")
                    box["t1"] = ps
                    for m in range(2):
                        for k in range(3):
                            nc.tensor.matmul(
                                ps[:, m, :], wap("tw1", k * 2 + m),
                                tx0[:, :, k : k + TT],
                                start=(k == 0), stop=(k == 2), perf_mode=DR,
                            )

                def t1_ev():
                    ps = box.pop("t1")
                    ev_scale_relu(HOMES["t1e"], tx1[:, :, :], ps[:, :, :], A1)

                def t2_mm():
                    ps = ppA.tile([128, 2, 512], F32, tag="psA")
                    box["t2"] = ps
                    for m in range(2):
                        nc.tensor.matmul(
                            ps[:, m, :], wap("tw2", m), tx1[:, :, :],
                            start=True, stop=True, perf_mode=DR,
                        )

                def t2_ev():
                    ps = box["t2"]
                    ev_scale(HOMES["t2e"], tx2m[:, :, :], ps[:, :, :], A2)

                def txsq_f():
                    ps = box["t2"]
                    if HOMES["txsq"] == "act":
                        nc.scalar.activation(txsq[:, :, :], ps[:, :, :], AF.Square,
                                             scale=A2)
                    else:
                        eng = nc.vector if HOMES["txsq"] == "dve" else nc.gpsimd
                        eng.tensor_tensor(txsq[:, :, :], tx2m[:, :, :],
                                          tx2m[:, :, :], OP.mult)

                def t2row_mm():
                    psT = box["t2"]  # reuse t2 psum tile (already evicted)
                    for cc_ in range(2):
                        nc.tensor.matmul(psT[0:1, 0, :], onesb[:], txsq[:, cc_, :],
                                         start=(cc_ == 0), stop=(cc_ == 1))

                def t2q_ev():
                    psT = box.pop("t2")
                    _e = {"dve": nc.vector, "pool": nc.gpsimd}[HOMES["t2q"]]
                    _e.tensor_scalar(
                        augx[0:1, 1, :], psT[0:1, 0, :],
                        T2Q_MUL, C0, OP.mult, OP.subtract,
                    )

                def u_mm():
                    ps = ppA.tile([128, 2, 512], F32, tag="psA")
                    box["u"] = ps
                    for m in range(2):
                        nc.tensor.matmul(
                            ps[:, m, :], wap("w3u", m), tx2m[:, :, :],
                            start=True, stop=True, perf_mode=DR,
                        )

                def u_ev():
                    ps = box.pop("u")
                    ev_scale(HOMES["ue"], u[:, :, :], ps[:, :, :], -BU)

                def f2_mm(n):
                    def f():
                        ps = ppA.tile([128, 2, 512], F32, tag="psA")
                        box[("f2", n)] = ps
                        for m in range(2):
                            for k in range(3):
                                nc.tensor.matmul(
                                    ps[:, m, :], wap("fw2", k * 2 + m),
                                    ft1[:, :, n * 512 + k : n * 512 + k + 512],
                                    start=(k == 0), stop=(k == 2), perf_mode=DR,
                                )
                    return f

                def f2_ev(n):
                    def f():
                        ps = box.pop(("f2", n))
                        ev_scale_relu(HOMES["f2e"],
                                      ft2[:, :, n * 512 : (n + 1) * 512],
                                      ps[:, :, :], AF2)
                    return f

                def h_mm(n):
                    def f():
                        ps = ppA.tile([128, 2, 512], F32, tag="psA")
                        box[("h", n)] = ps
                        for m in range(2):
                            nc.tensor.matmul(
                                ps[:, m, :], wap("G", m),
                                ft2[:, :, n * 512 : (n + 1) * 512],
                                start=True, stop=True, perf_mode=DR,
                            )
                    return f

                def fh_ev(n):
                    def f():
                        ps = box[("h", n)]
                        sl = slice(n * 512, (n + 1) * 512)
                        if HOMES["fh"] == "split":
                            nc.vector.scalar_tensor_tensor(
                                fh[:, 0, sl], ps[:, 0, :], BH, ft2[:, 0, sl],
                                OP.mult, OP.mult)
                            nc.gpsimd.scalar_tensor_tensor(
                                fh[:, 1, sl], ps[:, 1, :], BH, ft2[:, 1, sl],
                                OP.mult, OP.mult)
                        else:
                            eng = nc.vector if HOMES["fh"] == "dve" else nc.gpsimd
                            eng.scalar_tensor_tensor(
                                fh[:, :, sl], ps[:, :, :], BH, ft2[:, :, sl],
                                OP.mult, OP.mult)
                    return f

                def f2row_mm(n):
                    def f():
                        psH = box[("h", n)]  # reuse after fh_ev consumed it
                        for c_ in range(2):
                            nc.tensor.matmul(
                                psH[0:1, 0, :], ones1[:, :],
                                fh[:, c_, n * 512 : (n + 1) * 512],
                                start=(c_ == 0), stop=(c_ == 1),
                            )
                    return f

                def f2row_ev(n):
                    def f():
                        psH = box.pop(("h", n))
                        ev_scale(HOMES["rows"],
                                 augw[0:1, 0, n * 512 : (n + 1) * 512],
                                 psH[0:1, 0, :], 1.0)
                    return f

                parts = dict(
                    pads=pads, f1_mm=f1_mm, f1_ev=f1_ev, t1_mm=t1_mm, t1_ev=t1_ev,
                    t2_mm=t2_mm, t2_ev=t2_ev, txsq=txsq_f, t2row=t2row_mm,
                    t2q=t2q_ev, u_mm=u_mm, u_ev=u_ev, f2_mm=f2_mm, f2_ev=f2_ev,
                    h_mm=h_mm, fh_ev=fh_ev, f2row_mm=f2row_mm, f2row_ev=f2row_ev,
                )
                tiles = dict(tx2m=tx2m, ft2=ft2, u=u, augw=augw, augx=augx)
                return parts, tiles

            def tail_ctx(b, tiles):
                ft2, u = tiles["ft2"], tiles["u"]
                augw, augx = tiles["augw"], tiles["augx"]
                ssum = rowp.tile([128, NF], F32, tag="ssum")
                lns = rowp.tile([128, NF], F32, tag="lns")
                dist_tiles = {}

                def pair(j):
                    def f():
                        ps = ppS.tile([128, 2, 512], F32, tag="psS")
                        for h in range(2):
                            i = 2 * j + h
                            nc.tensor.matmul(
                                ps[:, h, :],
                                ft2[:, :, i * 128 : (i + 1) * 128],
                                u[:, :, :], start=True, stop=False, perf_mode=DR,
                            )
                            nc.tensor.matmul(
                                ps[:, h, :],
                                augw[0:1, :, i * 128 : (i + 1) * 128],
                                augx[0:1, :, :], start=False, stop=True,
                                perf_mode=DR,
                            )
                        dist = distp.tile([128, 2, 512], F16, tag="dist")
                        nc.scalar.activation(dist[:], ps[:], AF.Sqrt, bias=b_sq[:])
                        dist_tiles[j] = dist
                    return f

                def exp_pair(j):
                    def f():
                        dist = dist_tiles[j]
                        if j in PAIRED_SET:
                            e = ep.tile([128, 2, 512], BF16, tag="e")
                            nc.scalar.activation(e[:], dist[:], AF.Exp,
                                                 scale=-1.0, bias=b_m[:])
                            nc.vector.tensor_reduce(
                                ssum[:, 2 * j : 2 * j + 2], e[:], AX.X, OP.add
                            )
                        else:
                            for h in range(2):
                                i = 2 * j + h
                                e = ep.tile([128, 2, 512], BF16, tag="e")
                                nc.scalar.activation(
                                    e[:, 0, :], dist[:, h, :], AF.Exp,
                                    scale=-1.0, bias=b_m[:],
                                    accum_out=ssum[:, i : i + 1],
                                )
                    return f

                def ln_half(h):
                    def f():
                        nc.scalar.activation(
                            lns[:, 8 * h : 8 * h + 8], ssum[:, 8 * h : 8 * h + 8],
                            AF.Ln, scale=float(np.exp(-M_SHIFT)),
                        )
                    return f

                def quad(qi, split=False):
                    def f():
                        oq = outqp.tile([128, 4, 512], F32, tag="outq")
                        for q in range(4):
                            i = 4 * qi + q
                            dist = dist_tiles[i // 2]
                            if HOMES["outp"] == "alt":
                                eng = nc.gpsimd if q % 2 == 0 else nc.vector
                            else:
                                eng = {"pool": nc.gpsimd, "dve": nc.vector}[HOMES["outp"]]
                            eng.scalar_tensor_tensor(
                                oq[:, q, :], prior_sb[:, i, :], lns[:, i : i + 1],
                                dist[:, i % 2, :], OP.subtract, OP.subtract,
                            )
                            if split and q % 2 == 1:
                                nc.sync.dma_start(
                                    out=outD[b, 512 * qi + 256 * (q // 2) :
                                             512 * qi + 256 * (q // 2) + 256,
                                             :].rearrange("(q p) t -> p q t", p=128),
                                    in_=oq[:, q - 1 : q + 1, :],
                                )
                        if not split:
                            nc.sync.dma_start(
                                out=outD[b, 512 * qi : 512 * (qi + 1), :].rearrange(
                                    "(q p) t -> p q t", p=128
                                ),
                                in_=oq[:, :, :],
                            )
                    return f

                return pair, exp_pair, ln_half, quad

            # ================= emission =================
            tx0_0 = load_tx0(0)
            ft0_0 = load_ft0(0)
            tx0_1 = load_tx0(1)
            ft0_1 = load_ft0(1)
            nc.sync.dma_start(
                out=prior_sb[:], in_=priorD.rearrange("(i p) t -> p i t", p=128)
            )

            def batch_steps(b, tx0, ft0):
                p, tiles = batch_ctx(b, tx0, ft0)
                t = tail_ctx(b, tiles)
                pair, expp, lnh, quad = t
                return [
                    p["pads"],
                    p["f1_mm"](0), p["t1_mm"], p["f1_ev"](0), p["t1_ev"],
                    p["f1_mm"](1), p["t2_mm"], p["f1_ev"](1), p["t2_ev"],
                    p["f2_mm"](0), p["txsq"], p["f2_ev"](0),
                    p["t2row"], p["h_mm"](0), p["t2q"], p["fh_ev"](0),
                    p["f1_mm"](2), p["u_mm"], p["f1_ev"](2), p["u_ev"],
                    p["f2row_mm"](0), p["f2row_ev"](0),
                    pair(0),
                    p["f2_mm"](1), p["f1_mm"](3),
                    pair(1), p["f2_ev"](1), expp(0), p["f1_ev"](3),
                    p["h_mm"](1), p["fh_ev"](1),
                    p["f2row_mm"](1), p["f2row_ev"](1),
                    pair(2), p["f2_mm"](2), expp(1), p["f2_ev"](2),
                    pair(3), p["h_mm"](2), expp(2), p["fh_ev"](2),
                    p["f2row_mm"](2), p["f2row_ev"](2),
                    pair(4), p["f2_mm"](3), expp(3), p["f2_ev"](3),
                    lnh(0), quad(0),
                    pair(5), p["h_mm"](3), expp(4), p["fh_ev"](3),
                    p["f2row_mm"](3), p["f2row_ev"](3),
                    quad(1),
                    pair(6), expp(5), pair(7), expp(6), expp(7),
                    lnh(1), quad(2, split=True), quad(3, split=True),
                ]

            steps0 = batch_steps(0, tx0_0, ft0_0)
            steps1 = batch_steps(1, tx0_1, ft0_1)
            import os
            STAG = int(os.environ.get("KV2_STAGGER", "24"))
            merged = []
            i0 = i1 = 0
            # emit STAG steps of batch0 first, then alternate
            while i0 < len(steps0) or i1 < len(steps1):
                if i0 < len(steps0):
                    merged.append(steps0[i0]); i0 += 1
                if i0 >= STAG and i1 < len(steps1):
                    merged.append(steps1[i1]); i1 += 1
            for s in merged:
                s()

    _split_excess_waits(nc)
    return nc


_NC = None


def _get_nc():
    global _NC
    if _NC is None:
        _NC = _build_nc()
    return _NC


def _q8(x, scale, limit=230.0):
    import ml_dtypes
    y = np.asarray(x, np.float32) * scale
    m = np.abs(y).max()
    assert m < limit, f"fp8 range exceeded: {m} * (scale {scale})"
    return y.astype(ml_dtypes.float8_e4m3)


def _prep_inputs(text, feats, t_w1, t_b1, t_w2, t_b2,
                 f_w1, f_b1, f_w2, f_b2, f_w3, f_b3):
    for bias in (t_b1, t_b2, f_b1, f_b2, f_b3):
        assert not np.asarray(bias).any(), "kernel assumes zero biases (per spec)"
    c = np.ascontiguousarray
    f4 = np.float32

    textT = _q8(c(np.asarray(text, f4).transpose(0, 2, 1)), SX)    # [B,256,512]
    featsT = _q8(c(np.asarray(feats, f4).transpose(0, 2, 1)), SX)  # [B,80,2048]

    # wblob [128, WBLOB_W] fp8: per lhsT (k/m) block of [p, 2, 128]
    blob = np.zeros((128, WBLOB_W), np.float32)

    def put(name, idx, arr):  # arr [128, 2, 128] f32 (pre-scale applied)
        base = WOFF[name] + idx * 256
        blob[:, base : base + 256] = arr.reshape(128, 256)

    tw1 = np.asarray(t_w1, f4).transpose(2, 1, 0)  # [3, cin, cout]
    for k in range(3):
        for m in range(2):
            a = tw1[k].reshape(2, 128, 256)[:, :, m * 128 : (m + 1) * 128]
            put("tw1", k * 2 + m, a.transpose(1, 0, 2) * S_TW1)
    tw2 = np.asarray(t_w2, f4)[:, :, 0].T  # [cin, cout]
    for m in range(2):
        a = tw2.reshape(2, 128, 256)[:, :, m * 128 : (m + 1) * 128]
        put("tw2", m, a.transpose(1, 0, 2) * S_TW2)
    fw1 = np.asarray(f_w1, f4).transpose(2, 1, 0)  # [3, 80, 256]
    for d in range(2):
        for m in range(2):
            a = np.zeros((128, 2, 128), np.float32)
            a[:80, 0] = fw1[2 * d][:, m * 128 : (m + 1) * 128]
            if 2 * d + 1 < 3:
                a[:80, 1] = fw1[2 * d + 1][:, m * 128 : (m + 1) * 128]
            put("fw1", d * 2 + m, a * S_FW1)
    fw2 = np.asarray(f_w2, f4).transpose(2, 1, 0)
    for k in range(3):
        for m in range(2):
            a = fw2[k].reshape(2, 128, 256)[:, :, m * 128 : (m + 1) * 128]
            put("fw2", k * 2 + m, a.transpose(1, 0, 2) * S_FW2)
    W3 = np.asarray(f_w3, f4)[:, :, 0]  # [cout, cin]
    G = (W3.T @ W3).astype(np.float32)
    for m in range(2):
        a = G.reshape(2, 128, 256)[:, :, m * 128 : (m + 1) * 128]
        put("G", m, a.transpose(1, 0, 2) * S_G)
    for m in range(2):  # w3u lhsT[c, d]: W3 itself
        a = W3.reshape(2, 128, 256)[:, :, m * 128 : (m + 1) * 128]
        put("w3u", m, a.transpose(1, 0, 2) * S_W3)

    m = np.abs(blob).max()
    assert m < 230.0, f"wblob fp8 range exceeded: {m}"
    import ml_dtypes
    blob8 = blob.astype(ml_dtypes.float8_e4m3)

    import ml_dtypes as _mld
    ones8 = np.ones((1, TF), _mld.float8_e4m3)
    shared = {
        "wblob": blob8.view(np.uint8),
        "onesrow": ones8.view(np.uint8),
        "prior": _beta_binomial_prior().astype(np.float16),
    }
    in_maps = []
    for core in range(N_CORES):
        mcore = dict(shared)
        mcore["textT"] = c(textT[core * B_LOC : (core + 1) * B_LOC]).view(np.uint8)
        mcore["featsT"] = c(featsT[core * B_LOC : (core + 1) * B_LOC]).view(np.uint8)
        in_maps.append(mcore)
    return in_maps


_CALLABLE = None


def _build_callable():
    """Compile once; return fn(in_maps) -> per-core output dicts (axon path)."""
    import jax
    import jax.numpy as jnp
    from jax.sharding import Mesh, NamedSharding, PartitionSpec
    from jax.experimental.shard_map import shard_map
    from concourse.bass2jax import (
        _bass_exec_p,
        install_neuronx_cc_hook,
        partition_id_tensor,
    )

    nc = _get_nc()
    install_neuronx_cc_hook()
    partition_name = nc.partition_id_tensor.name if nc.partition_id_tensor else None
    in_names, out_names, out_avals, zero_shapes = [], [], [], []
    for alloc in nc.m.functions[0].allocations:
        if not isinstance(alloc, mybir.MemoryLocationSet):
            continue
        name = alloc.memorylocations[0].name
        if alloc.kind == "ExternalInput":
            if name != partition_name:
                in_names.append(name)
        elif alloc.kind == "ExternalOutput":
            shape = tuple(alloc.tensor_shape)
            dtype = mybir.dt.np(alloc.dtype)
            out_names.append(name)
            out_avals.append(jax.core.ShapedArray(shape, dtype))
            zero_shapes.append(((N_CORES * shape[0],) + shape[1:], dtype))
    n_params = len(in_names)
    n_outs = len(out_avals)
    all_in_names = list(in_names) + out_names
    if partition_name is not None:
        all_in_names.append(partition_name)
    donate = tuple(range(n_params, n_params + n_outs))

    def _body(*args):
        operands = list(args)
        if partition_name is not None:
            operands.append(partition_id_tensor())
        outs = _bass_exec_p.bind(
            *operands,
            out_avals=tuple(out_avals),
            in_names=tuple(all_in_names),
            out_names=tuple(out_names),
            lowering_input_output_aliases=(),
            sim_require_finite=True,
            sim_require_nnan=True,
            nc=nc,
        )
        return tuple(outs)

    devices = jax.devices()[:N_CORES]
    mesh = Mesh(np.asarray(devices), ("core",))
    fn = jax.jit(
        shard_map(
            _body,
            mesh=mesh,
            in_specs=(PartitionSpec("core"),) * (n_params + n_outs),
            out_specs=(PartitionSpec("core"),) * n_outs,
            check_rep=False,
        ),
        donate_argnums=donate,
        keep_unused=True,
    )
    sharding = NamedSharding(mesh, PartitionSpec("core"))
    zfn = jax.jit(
        lambda: tuple(jnp.zeros(s, d) for s, d in zero_shapes),
        out_shardings=tuple(sharding for _ in zero_shapes),
    )

    def call(in_maps):
        concat_in = [
            np.concatenate([np.asarray(in_maps[c][n]) for c in range(N_CORES)], axis=0)
            for n in in_names
        ]
        out_arrs = fn(*concat_in, *zfn())
        return [
            {
                name: np.asarray(out_arrs[i]).reshape(
                    N_CORES, *out_avals[i].shape
                )[c]
                for i, name in enumerate(out_names)
            }
            for c in range(N_CORES)
        ]

    return call


def _run(inputs, **kw):
    global _CALLABLE
    import time as _time

    in_maps = _prep_inputs(
        inputs["text"], inputs["feats"],
        inputs["t_w1"], inputs["t_b1"], inputs["t_w2"], inputs["t_b2"],
        inputs["f_w1"], inputs["f_b1"], inputs["f_w2"], inputs["f_b2"],
        inputs["f_w3"], inputs["f_b3"],
    )
    results = None
    last_err = None
    if _CALLABLE is not False:
        for attempt in range(3):
            try:
                if _CALLABLE is None:
                    from concourse._compat import axon_active

                    if not axon_active():
                        raise RuntimeError("axon not active; use native path")
                    _CALLABLE = _build_callable()
                results = _CALLABLE(in_maps)
                break
            except Exception as e:
                last_err = e
                results = None
                if attempt < 2:
                    _time.sleep(20 * (attempt + 1))
        if results is None:
            _CALLABLE = False
    if results is None:
        from concourse.bass_utils import run_bass_kernel_spmd

        for attempt in range(3):
            try:
                results = run_bass_kernel_spmd(
                    _get_nc(), in_maps, core_ids=list(range(N_CORES))
                ).results
                break
            except Exception as e:
                last_err = e
                results = None
                if attempt < 2:
                    _time.sleep(20 * (attempt + 1))
    if results is None:
        raise last_err
    out = np.concatenate([r["out"] for r in results], axis=0)
    return out, results


def kernel(**inputs) -> np.ndarray:
    out, _ = _run(inputs)
    return out


# revision 15
# speedup vs baseline: 1.0646x; 1.0157x over previous
"""AlignmentModule on 8 Trainium2 cores — fp8 DoubleRow rewrite.

Data-parallel over batch (2 per core). All matmuls run as fp8e4 DoubleRow
(2 K-planes per pass, 0.5 cycles/row = 4x fp32r): conv stacks t1,t2 / f1,f2,
the Gram matmul H=G.ft2, the text-side-absorbed f3 (u = W3^T tx2m — the 1x1
f3 conv is algebraically moved to the 4x-smaller text side; f2 norms come from
fh = ft2*H read straight off H's PSUM), the score cross ft2^T.u, and a K=1
augmented DR that adds f2[f] + (t2[t]-c0) rank-2 terms into the same PSUM.

Norm rows: f2row/t2row are ones-weight DR matmuls into PSUM row slots
(partitions 0/32/64 + bank2), evicted same-partition to fp8 and DMA-gathered
to the partition-0 aug operand rows (engines cannot cross partitions; DMA can).

Tail per f-tile: ACT sqrt (psum pair -> fp16 dist), ACT exp(14-dist) with
f32 row-accum (a tunable number of pairs instead run paired-exp + DVE reduce),
one ACT ln per batch, then outp = (prior_f16 - cc) - dist on Pool, DMA out in
4-tile quads. Engine assignment of evictions is tuned: Pool takes t1/t2/u +
outp, DVE takes f1/f2/fh/txsq/rows, ACT takes sqrt/exp/ln.

Host pre-quantizes inputs/weights to fp8 (power-of-2 scales, ranges asserted)
and ships them as uint8 bits; fp16 prior. Total rel err vs the f32 reference
is ~6.5e-4 (validated offline), dominated by fp8 conv activations and the
fp16 prior.
"""

import numpy as np

import bass_rust as _bass_rust
import concourse.bass as bass
import concourse.mybir as mybir
from concourse.tile import TileContext

F32 = mybir.dt.float32
F16 = mybir.dt.float16
BF16 = mybir.dt.bfloat16
F8 = mybir.dt.float8e4
U8 = mybir.dt.uint8
AF = mybir.ActivationFunctionType
OP = mybir.AluOpType
AX = mybir.AxisListType
DR = mybir.MatmulPerfMode.DoubleRow

B, T_TEXT, T_FEATS, ADIM, ODIM = 16, 512, 2048, 256, 80
N_CORES = 8
B_LOC = B // N_CORES
TT, TF = T_TEXT, T_FEATS
NT = TF // 512   # 4 feats chunks of 512
NF = TF // 128   # 16 f-tiles per batch

# ---- fixed power-of-2 scales (validated in opt/sim_numerics2.py) ----
SX = 16.0
S_TW1, S_TW2 = 1024.0, 512.0
S_FW1, S_FW2, S_W3 = 512.0, 1024.0, 512.0
S_G = 64.0
A1 = 1.0 / 512.0       # tx1 evict; tile = 32*true
A2 = 1.0 / 2048.0      # tx2m evict; tile = 8*true
AF1 = 1.0 / 512.0      # ft1 tile = 16*true
AF2 = 1.0 / 8192.0     # ft2 tile = 2*true
BH = 1.0 / 32.0        # fh evict scalar
BU = 1.0 / 4096.0      # u evict; u tile = -1*true(W3^T tx2)
ONES2_F2 = 1.0 / 8.0   # f2row ones-weight = 1/(S_G*s_ft2^2*BH)
T2Q_MUL = 1.0 / 64.0   # t2row evict mult = 1/s_tx2m^2
C0 = 192.0             # t2q offset; sqrt bias adds it back
M_SHIFT = 14.0
SQ_BIAS = C0           # dist = sqrt(psum + C0)

PAIRED_SET = (0,)  # score pairs using paired-exp + DVE reduce
POOL_OUTP = (2, 5, 8, 11, 14)  # f-tiles whose outp runs as 2 Pool tt passes
# engine homes for evictions: "dve" | "pool" | "act"(relu/identity/square ok)
# NOTE: GPSIMD (pool) cannot access PSUM on HW — psum-evictions are dve/act only.
HOMES = {
    "t1e": "dve", "t2e": "act", "ue": "dve",
    "f1e": "dve", "f2e": "dve", "fh": "dve",
    "rows": "act", "t2q": "dve", "txsq": "act",
    "outp": "dve",  # pool cannot do TensorScalarPtr on HW
}

WOFF = {}              # wblob free-dim offsets, filled by _pack_weights layout
WBLOB_W = 1536 + 512 + 1024 + 1536 + 512 + 512  # tw1,tw2,fw1,fw2,G,w3u


def _wblob_offsets():
    off, o = {}, 0
    for name, w in (("tw1", 1536), ("tw2", 512), ("fw1", 1024),
                    ("fw2", 1536), ("G", 512), ("w3u", 512)):
        off[name] = o
        o += w
    assert o == WBLOB_W
    return off


WOFF = _wblob_offsets()


def _split_excess_waits(nc, limit=1):
    """walrus CoreV3 CTRL codegen rejects >1 sync-wait per instruction.
    Hoist excess waits onto preceding NOPs on the same engine."""
    ctr = 0
    for f in nc.m.functions:
        for bb in f.blocks:
            insts = bb.instructions
            idx = 0
            while idx < len(insts):
                ins = insts[idx]
                si = ins.sync_info
                if si is not None and len(si.on_wait) > limit:
                    waits = list(si.on_wait)
                    extra, keep = waits[:-limit], waits[-limit:]
                    si.on_wait = keep
                    pos = idx
                    for j in range(0, len(extra), limit):
                        nop = mybir.InstNoOp(name=f"waitsplit_{ctr}", ins=[], outs=[])
                        ctr += 1
                        nop.engine = ins.engine
                        nop.sync_info = mybir.SyncInfo(
                            on_wait=extra[j : j + limit], on_update=[]
                        )
                        insts.insert(pos, nop)
                        pos += 1
                        idx += 1
                idx += 1
    return ctr


def _beta_binomial_prior():
    from scipy.special import gammaln

    T, N = T_FEATS, T_TEXT
    a = np.arange(1, T + 1, dtype=np.float64)[:, None]
    b = (T - np.arange(1, T + 1, dtype=np.float64) + 1.0)[:, None]
    k = np.arange(N, dtype=np.float64)[None, :]
    n = float(N)

    def betaln(x, y):
        return gammaln(x) + gammaln(y) - gammaln(x + y)

    logp = (
        gammaln(n + 1.0) - gammaln(k + 1.0) - gammaln(n - k + 1.0)
        + betaln(k + a, n - k + b) - betaln(a, b)
    )
    return logp.astype(np.float32)


def _build_nc():
    nc = bass.Bass(name="alignment")

    textT = nc.dram_tensor("textT", [B_LOC, ADIM, TT], U8, kind="ExternalInput")
    featsT = nc.dram_tensor("featsT", [B_LOC, ODIM, TF], U8, kind="ExternalInput")
    wblob = nc.dram_tensor("wblob", [128, WBLOB_W], U8, kind="ExternalInput")
    onesrow = nc.dram_tensor("onesrow", [1, TF], U8, kind="ExternalInput")
    priorD = nc.dram_tensor("prior", [TF, TT], F16, kind="ExternalInput")
    outD = nc.dram_tensor("out", [B_LOC, TF, TT], F32, kind="ExternalOutput")

    with TileContext(nc) as tc:
        with (
            tc.tile_pool(name="const", bufs=1) as const,
            tc.tile_pool(name="inp", bufs=2) as inp,
            tc.tile_pool(name="actp", bufs=2) as actp,
            tc.tile_pool(name="rowp", bufs=2) as rowp,
            tc.tile_pool(name="distp", bufs=17) as distp,
            tc.tile_pool(name="ep", bufs=3) as ep,
            tc.tile_pool(name="outq", bufs=3) as outqp,
            tc.tile_pool(name="po1", bufs=2) as po1,
            tc.tile_pool(name="ppA", bufs=2, space="PSUM") as ppA,
            tc.tile_pool(name="ppS", bufs=2, space="PSUM") as ppS,
        ):
            # ---- constants / weights ----
            wb = const.tile([128, WBLOB_W], F8)
            nc.sync.dma_start(out=wb[:].bitcast(U8), in_=wblob[:])
            ones1 = const.tile([128, 1], F8)
            nc.vector.memset(ones1[:], ONES2_F2)
            onesb = const.tile([128, 1], BF16)
            nc.vector.memset(onesb[:], 1.0)
            b_sq = const.tile([128, 1], F32)
            nc.vector.memset(b_sq[:], SQ_BIAS)
            b_m = const.tile([128, 1], F32)
            nc.vector.memset(b_m[:], M_SHIFT)
            prior_sb = const.tile([128, NF, TT], F16)

            def ev_scale_relu(home, out, ps, scale):
                if home == "act":
                    nc.scalar.activation(out, ps, AF.Relu, scale=scale)
                elif home == "split":
                    nc.vector.tensor_scalar(out[:, 0, :], ps[:, 0, :], scale,
                                            0.0, OP.mult, OP.max)
                    nc.gpsimd.tensor_scalar(out[:, 1, :], ps[:, 1, :], scale,
                                            0.0, OP.mult, OP.max)
                else:
                    eng = nc.vector if home == "dve" else nc.gpsimd
                    eng.tensor_scalar(out, ps, scale, 0.0, OP.mult, OP.max)

            def ev_scale(home, out, ps, scale):
                if home == "act":
                    nc.scalar.activation(out, ps, AF.Identity, scale=scale)
                elif home == "split":
                    nc.vector.tensor_scalar(out[:, 0, :], ps[:, 0, :], scale,
                                            None, OP.mult)
                    nc.gpsimd.tensor_scalar(out[:, 1, :], ps[:, 1, :], scale,
                                            None, OP.mult)
                else:
                    eng = nc.vector if home == "dve" else nc.gpsimd
                    eng.tensor_scalar(out, ps, scale, None, OP.mult)

            def wap(name, idx, planes=2, width=128):
                base = WOFF[name] + idx * planes * width
                return wb[:, base : base + planes * width].rearrange(
                    "p (c w) -> p c w", c=planes
                )

            def load_tx0(b):
                tx0 = inp.tile([128, 2, TT + 2], F8, tag="tx0")
                nc.vector.memset(tx0[:, :, 0:1], 0.0)
                nc.vector.memset(tx0[:, :, TT + 1 : TT + 2], 0.0)
                nc.sync.dma_start(
                    out=tx0[:, :, 1 : TT + 1].bitcast(U8),
                    in_=textT[b].rearrange("(c p) t -> p c t", p=128),
                )
                return tx0

            def load_ft0(b):
                # two copies, plane1 shifted +1 col, so the f1 tap-pair DR reads
                # non-overlapping ifmap planes (overlapping APs wedge the PE)
                ft0 = inp.tile([ODIM, 2, TF + 4], F8, tag="ft0")
                nc.vector.memset(ft0[:, :, 0:2], 0.0)
                nc.vector.memset(ft0[:, :, TF + 1 : TF + 4], 0.0)
                nc.sync.dma_start(out=ft0[:, 0, 1 : TF + 1].bitcast(U8), in_=featsT[b])
                nc.sync.dma_start(out=ft0[:, 1, 2 : TF + 2].bitcast(U8), in_=featsT[b])
                return ft0

            def batch_ctx(b, tx0, ft0):
                """Allocate per-batch tiles and return the conv step list plus
                the tile handles the tail needs."""
                tx1 = actp.tile([128, 2, TT], F8, tag="tx1")
                tx2m = actp.tile([128, 2, TT], F8, tag="tx2m")
                txsq = actp.tile([128, 2, TT], BF16, tag="txsq")
                ft1 = actp.tile([128, 2, TF + 2], F8, tag="ft1")
                ft2 = actp.tile([128, 2, TF], F8, tag="ft2")
                fh = actp.tile([128, 2, TF], F8, tag="fh")
                u = actp.tile([128, 2, TT], F8, tag="u")
                augw = rowp.tile([1, 2, TF], F8, tag="augw")
                augx = rowp.tile([1, 2, TT], F8, tag="augx")

                box = {}
                steps = []

                def pads():
                    nc.vector.memset(ft1[:, :, 0:1], 0.0)
                    nc.vector.memset(ft1[:, :, TF + 1 : TF + 2], 0.0)
                    nc.sync.dma_start(out=augw[0:1, 1, :].bitcast(U8),
                                      in_=onesrow[0:1, :])
                    nc.sync.dma_start(out=augx[0:1, 0, :].bitcast(U8),
                                      in_=onesrow[0:1, 0:TT])

                def tap_pair(start):
                    a = ft0[0:ODIM, start : start + 512]
                    w = a.copy()
                    w.ap = _bass_rust.VecI64Pair([list(a.ap[0]), [1, 2], [1, 512]])
                    return w

                def tap_pair(start):
                    """[80, 2, 512] ifmap: plane0 = ft0 copy0 at col start,
                    plane1 = copy1 at the same col (holding tap start+1).
                    Non-overlapping plane stride TF+4."""
                    a = ft0[0:ODIM, 0, start : start + 512]
                    w = a.copy()
                    w.ap = _bass_rust.VecI64Pair(
                        [list(a.ap[0]), [TF + 6, 2], [1, 512]]
                    )
                    return w

                def f1_mm(n):
                    def f():
                        ps = ppA.tile([128, 2, 512], F32, tag="psA")
                        box[("f1", n)] = ps
                        for m in range(2):
                            for d in range(2):
                                base = WOFF["fw1"] + (d * 2 + m) * 256
                                lhs = wb[0:ODIM, base : base + 256].rearrange(
                                    "p (c w) -> p c w", c=2
                                )
                                nc.tensor.matmul(
                                    ps[:, m, :], lhs, tap_pair(n * 512 + 2 * d),
                                    start=(d == 0), stop=(d == 1), perf_mode=DR,
                                )
                    return f

                def f1_ev(n):
                    def f():
                        ps = box.pop(("f1", n))
                        ev_scale_relu(HOMES["f1e"],
                                      ft1[:, :, 1 + n * 512 : 1 + (n + 1) * 512],
                                      ps[:, :, :], AF1)
                    return f

                def t1_mm():
                    ps = ppA.tile([128, 2, 512], F32, tag="psA")
                    box["t1"] = ps
                    for m in range(2):
                        for k in range(3):
                            nc.tensor.matmul(
                                ps[:, m, :], wap("tw1", k * 2 + m),
                                tx0[:, :, k : k + TT],
                                start=(k == 0), stop=(k == 2), perf_mode=DR,
                            )

                def t1_ev():
                    ps = box.pop("t1")
                    ev_scale_relu(HOMES["t1e"], tx1[:, :, :], ps[:, :, :], A1)

                def t2_mm():
                    ps = ppA.tile([128, 2, 512], F32, tag="psA")
                    box["t2"] = ps
                    for m in range(2):
                        nc.tensor.matmul(
                            ps[:, m, :], wap("tw2", m), tx1[:, :, :],
                            start=True, stop=True, perf_mode=DR,
                        )

                def t2_ev():
                    ps = box["t2"]
                    ev_scale(HOMES["t2e"], tx2m[:, :, :], ps[:, :, :], A2)

                def txsq_f():
                    ps = box["t2"]
                    if HOMES["txsq"] == "act":
                        nc.scalar.activation(txsq[:, :, :], ps[:, :, :], AF.Square,
                                             scale=A2)
                    else:
                        eng = nc.vector if HOMES["txsq"] == "dve" else nc.gpsimd
                        eng.tensor_tensor(txsq[:, :, :], tx2m[:, :, :],
                                          tx2m[:, :, :], OP.mult)

                def t2row_mm():
                    psT = box["t2"]  # reuse t2 psum tile (already evicted)
                    for cc_ in range(2):
                        nc.tensor.matmul(psT[0:1, 0, :], onesb[:], txsq[:, cc_, :],
                                         start=(cc_ == 0), stop=(cc_ == 1))

                def t2q_ev():
                    psT = box.pop("t2")
                    _e = {"dve": nc.vector, "pool": nc.gpsimd}[HOMES["t2q"]]
                    _e.tensor_scalar(
                        augx[0:1, 1, :], psT[0:1, 0, :],
                        T2Q_MUL, C0, OP.mult, OP.subtract,
                    )

                def u_mm():
                    ps = ppA.tile([128, 2, 512], F32, tag="psA")
                    box["u"] = ps
                    for m in range(2):
                        nc.tensor.matmul(
                            ps[:, m, :], wap("w3u", m), tx2m[:, :, :],
                            start=True, stop=True, perf_mode=DR,
                        )

                def u_ev():
                    ps = box.pop("u")
                    ev_scale(HOMES["ue"], u[:, :, :], ps[:, :, :], -BU)

                def f2_mm(n):
                    def f():
                        ps = ppA.tile([128, 2, 512], F32, tag="psA")
                        box[("f2", n)] = ps
                        for m in range(2):
                            for k in range(3):
                                nc.tensor.matmul(
                                    ps[:, m, :], wap("fw2", k * 2 + m),
                                    ft1[:, :, n * 512 + k : n * 512 + k + 512],
                                    start=(k == 0), stop=(k == 2), perf_mode=DR,
                                )
                    return f

                def f2_ev(n):
                    def f():
                        ps = box.pop(("f2", n))
                        ev_scale_relu(HOMES["f2e"],
                                      ft2[:, :, n * 512 : (n + 1) * 512],
                                      ps[:, :, :], AF2)
                    return f

                def h_mm(n):
                    def f():
                        ps = ppA.tile([128, 2, 512], F32, tag="psA")
                        box[("h", n)] = ps
                        for m in range(2):
                            nc.tensor.matmul(
                                ps[:, m, :], wap("G", m),
                                ft2[:, :, n * 512 : (n + 1) * 512],
                                start=True, stop=True, perf_mode=DR,
                            )
                    return f

                def fh_ev(n):
                    def f():
                        ps = box[("h", n)]
                        sl = slice(n * 512, (n + 1) * 512)
                        if HOMES["fh"] == "split":
                            nc.vector.scalar_tensor_tensor(
                                fh[:, 0, sl], ps[:, 0, :], BH, ft2[:, 0, sl],
                                OP.mult, OP.mult)
                            nc.gpsimd.scalar_tensor_tensor(
                                fh[:, 1, sl], ps[:, 1, :], BH, ft2[:, 1, sl],
                                OP.mult, OP.mult)
                        else:
                            eng = nc.vector if HOMES["fh"] == "dve" else nc.gpsimd
                            eng.scalar_tensor_tensor(
                                fh[:, :, sl], ps[:, :, :], BH, ft2[:, :, sl],
                                OP.mult, OP.mult)
                    return f

                def f2row_mm(n):
                    def f():
                        psH = box[("h", n)]  # reuse after fh_ev consumed it
                        for c_ in range(2):
                            nc.tensor.matmul(
                                psH[0:1, 0, :], ones1[:, :],
                                fh[:, c_, n * 512 : (n + 1) * 512],
                                start=(c_ == 0), stop=(c_ == 1),
                            )
                    return f

                def f2row_ev(n):
                    def f():
                        psH = box.pop(("h", n))
                        ev_scale(HOMES["rows"],
                                 augw[0:1, 0, n * 512 : (n + 1) * 512],
                                 psH[0:1, 0, :], 1.0)
                    return f

                parts = dict(
                    pads=pads, f1_mm=f1_mm, f1_ev=f1_ev, t1_mm=t1_mm, t1_ev=t1_ev,
                    t2_mm=t2_mm, t2_ev=t2_ev, txsq=txsq_f, t2row=t2row_mm,
                    t2q=t2q_ev, u_mm=u_mm, u_ev=u_ev, f2_mm=f2_mm, f2_ev=f2_ev,
                    h_mm=h_mm, fh_ev=fh_ev, f2row_mm=f2row_mm, f2row_ev=f2row_ev,
                )
                tiles = dict(tx2m=tx2m, ft2=ft2, u=u, augw=augw, augx=augx)
                return parts, tiles

            def tail_ctx(b, tiles):
                ft2, u = tiles["ft2"], tiles["u"]
                augw, augx = tiles["augw"], tiles["augx"]
                ssum = rowp.tile([128, NF], F32, tag="ssum")
                lns = rowp.tile([128, NF], F32, tag="lns")
                dist_tiles = {}

                def pair(j):
                    def f():
                        ps = ppS.tile([128, 2, 512], F32, tag="psS")
                        for h in range(2):
                            i = 2 * j + h
                            nc.tensor.matmul(
                                ps[:, h, :],
                                ft2[:, :, i * 128 : (i + 1) * 128],
                                u[:, :, :], start=True, stop=False, perf_mode=DR,
                            )
                            nc.tensor.matmul(
                                ps[:, h, :],
                                augw[0:1, :, i * 128 : (i + 1) * 128],
                                augx[0:1, :, :], start=False, stop=True,
                                perf_mode=DR,
                            )
                        dist = distp.tile([128, 2, 512], F16, tag="dist")
                        nc.scalar.activation(dist[:], ps[:], AF.Sqrt, bias=b_sq[:])
                        dist_tiles[j] = dist
                    return f

                def exp_pair(j):
                    def f():
                        dist = dist_tiles[j]
                        if j in PAIRED_SET:
                            e = ep.tile([128, 2, 512], BF16, tag="e")
                            nc.scalar.activation(e[:], dist[:], AF.Exp,
                                                 scale=-1.0, bias=b_m[:])
                            nc.vector.tensor_reduce(
                                ssum[:, 2 * j : 2 * j + 2], e[:], AX.X, OP.add
                            )
                        else:
                            for h in range(2):
                                i = 2 * j + h
                                e = ep.tile([128, 2, 512], BF16, tag="e")
                                nc.scalar.activation(
                                    e[:, 0, :], dist[:, h, :], AF.Exp,
                                    scale=-1.0, bias=b_m[:],
                                    accum_out=ssum[:, i : i + 1],
                                )
                    return f

                def ln_half(h):
                    def f():
                        nc.scalar.activation(
                            lns[:, 8 * h : 8 * h + 8], ssum[:, 8 * h : 8 * h + 8],
                            AF.Ln, scale=float(np.exp(-M_SHIFT)),
                        )
                    return f

                def quad(qi, split=False):
                    def f():
                        oq = outqp.tile([128, 4, 512], F32, tag="outq")
                        for q in range(4):
                            i = 4 * qi + q
                            dist = dist_tiles[i // 2]
                            if i in POOL_OUTP:
                                # 2 Pool tt passes (Pool lacks TensorScalarPtr);
                                # cc enters via a stride-0 broadcast operand
                                o1 = po1.tile([128, 512], F32, tag="o1")
                                nc.gpsimd.tensor_tensor(
                                    o1[:], prior_sb[:, i, :], dist[:, i % 2, :],
                                    OP.subtract)
                                nc.gpsimd.tensor_tensor(
                                    oq[:, q, :], o1[:],
                                    lns[:, i : i + 1].broadcast_to((128, TT)),
                                    OP.subtract)
                            else:
                                nc.vector.scalar_tensor_tensor(
                                    oq[:, q, :], prior_sb[:, i, :], lns[:, i : i + 1],
                                    dist[:, i % 2, :], OP.subtract, OP.subtract,
                                )
                            if split and q % 2 == 1:
                                nc.sync.dma_start(
                                    out=outD[b, 512 * qi + 256 * (q // 2) :
                                             512 * qi + 256 * (q // 2) + 256,
                                             :].rearrange("(q p) t -> p q t", p=128),
                                    in_=oq[:, q - 1 : q + 1, :],
                                )
                        if not split:
                            nc.sync.dma_start(
                                out=outD[b, 512 * qi : 512 * (qi + 1), :].rearrange(
                                    "(q p) t -> p q t", p=128
                                ),
                                in_=oq[:, :, :],
                            )
                    return f

                return pair, exp_pair, ln_half, quad

            # ================= emission =================
            tx0_0 = load_tx0(0)
            ft0_0 = load_ft0(0)
            tx0_1 = load_tx0(1)
            ft0_1 = load_ft0(1)
            nc.sync.dma_start(
                out=prior_sb[:], in_=priorD.rearrange("(i p) t -> p i t", p=128)
            )

            def batch_steps(b, tx0, ft0):
                p, tiles = batch_ctx(b, tx0, ft0)
                t = tail_ctx(b, tiles)
                pair, expp, lnh, quad = t
                return [
                    p["pads"],
                    p["f1_mm"](0), p["t1_mm"], p["f1_ev"](0), p["t1_ev"],
                    p["f1_mm"](1), p["t2_mm"], p["f1_ev"](1), p["t2_ev"],
                    p["f2_mm"](0), p["txsq"], p["f2_ev"](0),
                    p["t2row"], p["h_mm"](0), p["t2q"], p["fh_ev"](0),
                    p["f1_mm"](2), p["u_mm"], p["f1_ev"](2), p["u_ev"],
                    p["f2row_mm"](0), p["f2row_ev"](0),
                    pair(0),
                    p["f2_mm"](1), p["f1_mm"](3),
                    pair(1), p["f2_ev"](1), expp(0), p["f1_ev"](3),
                    p["h_mm"](1), p["fh_ev"](1),
                    p["f2row_mm"](1), p["f2row_ev"](1),
                    pair(2), p["f2_mm"](2), expp(1), p["f2_ev"](2),
                    pair(3), p["h_mm"](2), expp(2), p["fh_ev"](2),
                    p["f2row_mm"](2), p["f2row_ev"](2),
                    pair(4), p["f2_mm"](3), expp(3), p["f2_ev"](3),
                    lnh(0), quad(0),
                    pair(5), p["h_mm"](3), expp(4), p["fh_ev"](3),
                    p["f2row_mm"](3), p["f2row_ev"](3),
                    quad(1),
                    pair(6), expp(5), pair(7), expp(6), expp(7),
                    lnh(1), quad(2, split=True), quad(3, split=True),
                ]

            steps0 = batch_steps(0, tx0_0, ft0_0)
            steps1 = batch_steps(1, tx0_1, ft0_1)
            import os
            STAG = int(os.environ.get("KV2_STAGGER", "24"))
            merged = []
            i0 = i1 = 0
            # emit STAG steps of batch0 first, then alternate
            while i0 < len(steps0) or i1 < len(steps1):
                if i0 < len(steps0):
                    merged.append(steps0[i0]); i0 += 1
                if i0 >= STAG and i1 < len(steps1):
                    merged.append(steps1[i1]); i1 += 1
            for s in merged:
                s()

    _split_excess_waits(nc)
    return nc


_NC = None


def _get_nc():
    global _NC
    if _NC is None:
        _NC = _build_nc()
    return _NC


def _q8(x, scale, limit=230.0):
    import ml_dtypes
    y = np.asarray(x, np.float32) * scale
    m = np.abs(y).max()
    assert m < limit, f"fp8 range exceeded: {m} * (scale {scale})"
    return y.astype(ml_dtypes.float8_e4m3)


def _prep_inputs(text, feats, t_w1, t_b1, t_w2, t_b2,
                 f_w1, f_b1, f_w2, f_b2, f_w3, f_b3):
    for bias in (t_b1, t_b2, f_b1, f_b2, f_b3):
        assert not np.asarray(bias).any(), "kernel assumes zero biases (per spec)"
    c = np.ascontiguousarray
    f4 = np.float32

    textT = _q8(c(np.asarray(text, f4).transpose(0, 2, 1)), SX)    # [B,256,512]
    featsT = _q8(c(np.asarray(feats, f4).transpose(0, 2, 1)), SX)  # [B,80,2048]

    # wblob [128, WBLOB_W] fp8: per lhsT (k/m) block of [p, 2, 128]
    blob = np.zeros((128, WBLOB_W), np.float32)

    def put(name, idx, arr):  # arr [128, 2, 128] f32 (pre-scale applied)
        base = WOFF[name] + idx * 256
        blob[:, base : base + 256] = arr.reshape(128, 256)

    tw1 = np.asarray(t_w1, f4).transpose(2, 1, 0)  # [3, cin, cout]
    for k in range(3):
        for m in range(2):
            a = tw1[k].reshape(2, 128, 256)[:, :, m * 128 : (m + 1) * 128]
            put("tw1", k * 2 + m, a.transpose(1, 0, 2) * S_TW1)
    tw2 = np.asarray(t_w2, f4)[:, :, 0].T  # [cin, cout]
    for m in range(2):
        a = tw2.reshape(2, 128, 256)[:, :, m * 128 : (m + 1) * 128]
        put("tw2", m, a.transpose(1, 0, 2) * S_TW2)
    fw1 = np.asarray(f_w1, f4).transpose(2, 1, 0)  # [3, 80, 256]
    for d in range(2):
        for m in range(2):
            a = np.zeros((128, 2, 128), np.float32)
            a[:80, 0] = fw1[2 * d][:, m * 128 : (m + 1) * 128]
            if 2 * d + 1 < 3:
                a[:80, 1] = fw1[2 * d + 1][:, m * 128 : (m + 1) * 128]
            put("fw1", d * 2 + m, a * S_FW1)
    fw2 = np.asarray(f_w2, f4).transpose(2, 1, 0)
    for k in range(3):
        for m in range(2):
            a = fw2[k].reshape(2, 128, 256)[:, :, m * 128 : (m + 1) * 128]
            put("fw2", k * 2 + m, a.transpose(1, 0, 2) * S_FW2)
    W3 = np.asarray(f_w3, f4)[:, :, 0]  # [cout, cin]
    G = (W3.T @ W3).astype(np.float32)
    for m in range(2):
        a = G.reshape(2, 128, 256)[:, :, m * 128 : (m + 1) * 128]
        put("G", m, a.transpose(1, 0, 2) * S_G)
    for m in range(2):  # w3u lhsT[c, d]: W3 itself
        a = W3.reshape(2, 128, 256)[:, :, m * 128 : (m + 1) * 128]
        put("w3u", m, a.transpose(1, 0, 2) * S_W3)

    m = np.abs(blob).max()
    assert m < 230.0, f"wblob fp8 range exceeded: {m}"
    import ml_dtypes
    blob8 = blob.astype(ml_dtypes.float8_e4m3)

    import ml_dtypes as _mld
    ones8 = np.ones((1, TF), _mld.float8_e4m3)
    shared = {
        "wblob": blob8.view(np.uint8),
        "onesrow": ones8.view(np.uint8),
        "prior": _beta_binomial_prior().astype(np.float16),
    }
    in_maps = []
    for core in range(N_CORES):
        mcore = dict(shared)
        mcore["textT"] = c(textT[core * B_LOC : (core + 1) * B_LOC]).view(np.uint8)
        mcore["featsT"] = c(featsT[core * B_LOC : (core + 1) * B_LOC]).view(np.uint8)
        in_maps.append(mcore)
    return in_maps


_CALLABLE = None


def _build_callable():
    """Compile once; return fn(in_maps) -> per-core output dicts (axon path)."""
    import jax
    import jax.numpy as jnp
    from jax.sharding import Mesh, NamedSharding, PartitionSpec
    from jax.experimental.shard_map import shard_map
    from concourse.bass2jax import (
        _bass_exec_p,
        install_neuronx_cc_hook,
        partition_id_tensor,
    )

    nc = _get_nc()
    install_neuronx_cc_hook()
    partition_name = nc.partition_id_tensor.name if nc.partition_id_tensor else None
    in_names, out_names, out_avals, zero_shapes = [], [], [], []
    for alloc in nc.m.functions[0].allocations:
        if not isinstance(alloc, mybir.MemoryLocationSet):
            continue
        name = alloc.memorylocations[0].name
        if alloc.kind == "ExternalInput":
            if name != partition_name:
                in_names.append(name)
        elif alloc.kind == "ExternalOutput":
            shape = tuple(alloc.tensor_shape)
            dtype = mybir.dt.np(alloc.dtype)
            out_names.append(name)
            out_avals.append(jax.core.ShapedArray(shape, dtype))
            zero_shapes.append(((N_CORES * shape[0],) + shape[1:], dtype))
    n_params = len(in_names)
    n_outs = len(out_avals)
    all_in_names = list(in_names) + out_names
    if partition_name is not None:
        all_in_names.append(partition_name)
    donate = tuple(range(n_params, n_params + n_outs))

    def _body(*args):
        operands = list(args)
        if partition_name is not None:
            operands.append(partition_id_tensor())
        outs = _bass_exec_p.bind(
            *operands,
            out_avals=tuple(out_avals),
            in_names=tuple(all_in_names),
            out_names=tuple(out_names),
            lowering_input_output_aliases=(),
            sim_require_finite=True,
            sim_require_nnan=True,
            nc=nc,
        )
        return tuple(outs)

    devices = jax.devices()[:N_CORES]
    mesh = Mesh(np.asarray(devices), ("core",))
    fn = jax.jit(
        shard_map(
            _body,
            mesh=mesh,
            in_specs=(PartitionSpec("core"),) * (n_params + n_outs),
            out_specs=(PartitionSpec("core"),) * n_outs,
            check_rep=False,
        ),
        donate_argnums=donate,
        keep_unused=True,
    )
    sharding = NamedSharding(mesh, PartitionSpec("core"))
    zfn = jax.jit(
        lambda: tuple(jnp.zeros(s, d) for s, d in zero_shapes),
        out_shardings=tuple(sharding for _ in zero_shapes),
    )

    def call(in_maps):
        concat_in = [
            np.concatenate([np.asarray(in_maps[c][n]) for c in range(N_CORES)], axis=0)
            for n in in_names
        ]
        out_arrs = fn(*concat_in, *zfn())
        return [
            {
                name: np.asarray(out_arrs[i]).reshape(
                    N_CORES, *out_avals[i].shape
                )[c]
                for i, name in enumerate(out_names)
            }
            for c in range(N_CORES)
        ]

    return call


def _run(inputs, **kw):
    global _CALLABLE
    import time as _time

    in_maps = _prep_inputs(
        inputs["text"], inputs["feats"],
        inputs["t_w1"], inputs["t_b1"], inputs["t_w2"], inputs["t_b2"],
        inputs["f_w1"], inputs["f_b1"], inputs["f_w2"], inputs["f_b2"],
        inputs["f_w3"], inputs["f_b3"],
    )
    results = None
    last_err = None
    if _CALLABLE is not False:
        for attempt in range(3):
            try:
                if _CALLABLE is None:
                    from concourse._compat import axon_active

                    if not axon_active():
                        raise RuntimeError("axon not active; use native path")
                    _CALLABLE = _build_callable()
                results = _CALLABLE(in_maps)
                break
            except Exception as e:
                last_err = e
                results = None
                if attempt < 2:
                    _time.sleep(20 * (attempt + 1))
        if results is None:
            _CALLABLE = False
    if results is None:
        from concourse.bass_utils import run_bass_kernel_spmd

        for attempt in range(3):
            try:
                results = run_bass_kernel_spmd(
                    _get_nc(), in_maps, core_ids=list(range(N_CORES))
                ).results
                break
            except Exception as e:
                last_err = e
                results = None
                if attempt < 2:
                    _time.sleep(20 * (attempt + 1))
    if results is None:
        raise last_err
    out = np.concatenate([r["out"] for r in results], axis=0)
    return out, results


def kernel(**inputs) -> np.ndarray:
    out, _ = _run(inputs)
    return out


# revision 16
# speedup vs baseline: 1.1009x; 1.0341x over previous
"""AlignmentModule on 8 Trainium2 cores — fp8 DoubleRow rewrite.

Data-parallel over batch (2 per core). All matmuls run as fp8e4 DoubleRow
(2 K-planes per pass, 0.5 cycles/row = 4x fp32r): conv stacks t1,t2 / f1,f2,
the Gram matmul H=G.ft2, the text-side-absorbed f3 (u = W3^T tx2m — the 1x1
f3 conv is algebraically moved to the 4x-smaller text side; f2 norms come from
fh = ft2*H read straight off H's PSUM), the score cross ft2^T.u, and a K=1
augmented DR that adds f2[f] + (t2[t]-c0) rank-2 terms into the same PSUM.

Norm rows: f2row/t2row are ones-weight DR matmuls into PSUM row slots
(partitions 0/32/64 + bank2), evicted same-partition to fp8 and DMA-gathered
to the partition-0 aug operand rows (engines cannot cross partitions; DMA can).

Tail per f-tile: ACT sqrt (psum pair -> fp16 dist), ACT exp(14-dist) with
f32 row-accum (a tunable number of pairs instead run paired-exp + DVE reduce),
one ACT ln per batch, then outp = (prior_f16 - cc) - dist on Pool, DMA out in
4-tile quads. Engine assignment of evictions is tuned: Pool takes t1/t2/u +
outp, DVE takes f1/f2/fh/txsq/rows, ACT takes sqrt/exp/ln.

Host pre-quantizes inputs/weights to fp8 (power-of-2 scales, ranges asserted)
and ships them as uint8 bits; fp16 prior. Total rel err vs the f32 reference
is ~6.5e-4 (validated offline), dominated by fp8 conv activations and the
fp16 prior.
"""

import numpy as np

import bass_rust as _bass_rust
import concourse.bass as bass
import concourse.mybir as mybir
from concourse.tile import TileContext

F32 = mybir.dt.float32
F16 = mybir.dt.float16
BF16 = mybir.dt.bfloat16
F8 = mybir.dt.float8e4
U8 = mybir.dt.uint8
AF = mybir.ActivationFunctionType
OP = mybir.AluOpType
AX = mybir.AxisListType
DR = mybir.MatmulPerfMode.DoubleRow

B, T_TEXT, T_FEATS, ADIM, ODIM = 16, 512, 2048, 256, 80
N_CORES = 8
B_LOC = B // N_CORES
TT, TF = T_TEXT, T_FEATS
NT = TF // 512   # 4 feats chunks of 512
NF = TF // 128   # 16 f-tiles per batch

# ---- fixed power-of-2 scales (validated in opt/sim_numerics2.py) ----
SX = 16.0
S_TW1, S_TW2 = 1024.0, 512.0
S_FW1, S_FW2, S_W3 = 512.0, 1024.0, 512.0
S_G = 64.0
A1 = 1.0 / 512.0       # tx1 evict; tile = 32*true
A2 = 1.0 / 2048.0      # tx2m evict; tile = 8*true
AF1 = 1.0 / 512.0      # ft1 tile = 16*true
AF2 = 1.0 / 8192.0     # ft2 tile = 2*true
BH = 1.0 / 32.0        # fh evict scalar
BU = 1.0 / 4096.0      # u evict; u tile = -1*true(W3^T tx2)
ONES2_F2 = 1.0 / 8.0   # f2row ones-weight = 1/(S_G*s_ft2^2*BH)
T2Q_MUL = 1.0 / 64.0   # t2row evict mult = 1/s_tx2m^2
C0 = 192.0             # t2q offset; sqrt bias adds it back
M_SHIFT = 14.0
SQ_BIAS = C0           # dist = sqrt(psum + C0)

PAIRED_SET = (1, 2, 3)  # score pairs using paired-exp + DVE reduce
POOL_OUTP = (2, 5, 8, 11, 14)  # f-tiles whose outp runs as 2 Pool tt passes
# engine homes for evictions: "dve" | "pool" | "act"(relu/identity/square ok)
# NOTE: GPSIMD (pool) cannot access PSUM on HW — psum-evictions are dve/act only.
HOMES = {
    "t1e": "dve", "t2e": "act", "ue": "dve",
    "f1e": "dve", "f2e": "dve", "fh": "dve",
    "rows": "act", "t2q": "dve", "txsq": "act",
    "outp": "dve",  # pool cannot do TensorScalarPtr on HW
}

WOFF = {}              # wblob free-dim offsets, filled by _pack_weights layout
WBLOB_W = 1536 + 512 + 1024 + 1536 + 512 + 512  # tw1,tw2,fw1,fw2,G,w3u


def _wblob_offsets():
    off, o = {}, 0
    for name, w in (("tw1", 1536), ("tw2", 512), ("fw1", 1024),
                    ("fw2", 1536), ("G", 512), ("w3u", 512)):
        off[name] = o
        o += w
    assert o == WBLOB_W
    return off


WOFF = _wblob_offsets()


def _split_excess_waits(nc, limit=1):
    """walrus CoreV3 CTRL codegen rejects >1 sync-wait per instruction.
    Hoist excess waits onto preceding NOPs on the same engine."""
    ctr = 0
    for f in nc.m.functions:
        for bb in f.blocks:
            insts = bb.instructions
            idx = 0
            while idx < len(insts):
                ins = insts[idx]
                si = ins.sync_info
                if si is not None and len(si.on_wait) > limit:
                    waits = list(si.on_wait)
                    extra, keep = waits[:-limit], waits[-limit:]
                    si.on_wait = keep
                    pos = idx
                    for j in range(0, len(extra), limit):
                        nop = mybir.InstNoOp(name=f"waitsplit_{ctr}", ins=[], outs=[])
                        ctr += 1
                        nop.engine = ins.engine
                        nop.sync_info = mybir.SyncInfo(
                            on_wait=extra[j : j + limit], on_update=[]
                        )
                        insts.insert(pos, nop)
                        pos += 1
                        idx += 1
                idx += 1
    return ctr


def _beta_binomial_prior():
    from scipy.special import gammaln

    T, N = T_FEATS, T_TEXT
    a = np.arange(1, T + 1, dtype=np.float64)[:, None]
    b = (T - np.arange(1, T + 1, dtype=np.float64) + 1.0)[:, None]
    k = np.arange(N, dtype=np.float64)[None, :]
    n = float(N)

    def betaln(x, y):
        return gammaln(x) + gammaln(y) - gammaln(x + y)

    logp = (
        gammaln(n + 1.0) - gammaln(k + 1.0) - gammaln(n - k + 1.0)
        + betaln(k + a, n - k + b) - betaln(a, b)
    )
    return logp.astype(np.float32)


def _build_nc():
    nc = bass.Bass(name="alignment")

    textT = nc.dram_tensor("textT", [B_LOC, ADIM, TT], U8, kind="ExternalInput")
    featsT = nc.dram_tensor("featsT", [B_LOC, ODIM, TF], U8, kind="ExternalInput")
    wblob = nc.dram_tensor("wblob", [128, WBLOB_W], U8, kind="ExternalInput")
    onesrow = nc.dram_tensor("onesrow", [1, TF], U8, kind="ExternalInput")
    priorD = nc.dram_tensor("prior", [TF, TT], F16, kind="ExternalInput")
    outD = nc.dram_tensor("out", [B_LOC, TF, TT], F32, kind="ExternalOutput")

    with TileContext(nc) as tc:
        with (
            tc.tile_pool(name="const", bufs=1) as const,
            tc.tile_pool(name="inp", bufs=2) as inp,
            tc.tile_pool(name="actp", bufs=2) as actp,
            tc.tile_pool(name="rowp", bufs=2) as rowp,
            tc.tile_pool(name="distp", bufs=17) as distp,
            tc.tile_pool(name="ep", bufs=3) as ep,
            tc.tile_pool(name="outq", bufs=3) as outqp,
            tc.tile_pool(name="po1", bufs=2) as po1,
            tc.tile_pool(name="ppA", bufs=2, space="PSUM") as ppA,
            tc.tile_pool(name="ppS", bufs=2, space="PSUM") as ppS,
        ):
            # ---- constants / weights ----
            wb = const.tile([128, WBLOB_W], F8)
            nc.sync.dma_start(out=wb[:].bitcast(U8), in_=wblob[:])
            ones1 = const.tile([128, 1], F8)
            nc.vector.memset(ones1[:], ONES2_F2)
            onesb = const.tile([128, 1], BF16)
            nc.vector.memset(onesb[:], 1.0)
            b_sq = const.tile([128, 1], F32)
            nc.vector.memset(b_sq[:], SQ_BIAS)
            b_m = const.tile([128, 1], F32)
            nc.vector.memset(b_m[:], M_SHIFT)
            prior_sb = const.tile([128, NF, TT], F16)

            def ev_scale_relu(home, out, ps, scale):
                if home == "act":
                    nc.scalar.activation(out, ps, AF.Relu, scale=scale)
                elif home == "split":
                    nc.vector.tensor_scalar(out[:, 0, :], ps[:, 0, :], scale,
                                            0.0, OP.mult, OP.max)
                    nc.gpsimd.tensor_scalar(out[:, 1, :], ps[:, 1, :], scale,
                                            0.0, OP.mult, OP.max)
                else:
                    eng = nc.vector if home == "dve" else nc.gpsimd
                    eng.tensor_scalar(out, ps, scale, 0.0, OP.mult, OP.max)

            def ev_scale(home, out, ps, scale):
                if home == "act":
                    nc.scalar.activation(out, ps, AF.Identity, scale=scale)
                elif home == "split":
                    nc.vector.tensor_scalar(out[:, 0, :], ps[:, 0, :], scale,
                                            None, OP.mult)
                    nc.gpsimd.tensor_scalar(out[:, 1, :], ps[:, 1, :], scale,
                                            None, OP.mult)
                else:
                    eng = nc.vector if home == "dve" else nc.gpsimd
                    eng.tensor_scalar(out, ps, scale, None, OP.mult)

            def wap(name, idx, planes=2, width=128):
                base = WOFF[name] + idx * planes * width
                return wb[:, base : base + planes * width].rearrange(
                    "p (c w) -> p c w", c=planes
                )

            def load_tx0(b):
                tx0 = inp.tile([128, 2, TT + 2], F8, tag="tx0")
                nc.vector.memset(tx0[:, :, 0:1], 0.0)
                nc.vector.memset(tx0[:, :, TT + 1 : TT + 2], 0.0)
                nc.sync.dma_start(
                    out=tx0[:, :, 1 : TT + 1].bitcast(U8),
                    in_=textT[b].rearrange("(c p) t -> p c t", p=128),
                )
                return tx0

            def load_ft0(b):
                # two copies, plane1 shifted +1 col, so the f1 tap-pair DR reads
                # non-overlapping ifmap planes (overlapping APs wedge the PE)
                ft0 = inp.tile([ODIM, 2, TF + 4], F8, tag="ft0")
                nc.vector.memset(ft0[:, :, 0:2], 0.0)
                nc.vector.memset(ft0[:, :, TF + 1 : TF + 4], 0.0)
                nc.sync.dma_start(out=ft0[:, 0, 1 : TF + 1].bitcast(U8), in_=featsT[b])
                nc.sync.dma_start(out=ft0[:, 1, 2 : TF + 2].bitcast(U8), in_=featsT[b])
                return ft0

            def batch_ctx(b, tx0, ft0):
                """Allocate per-batch tiles and return the conv step list plus
                the tile handles the tail needs."""
                tx1 = actp.tile([128, 2, TT], F8, tag="tx1")
                tx2m = actp.tile([128, 2, TT], F8, tag="tx2m")
                txsq = actp.tile([128, 2, TT], BF16, tag="txsq")
                ft1 = actp.tile([128, 2, TF + 2], F8, tag="ft1")
                ft2 = actp.tile([128, 2, TF], F8, tag="ft2")
                fh = actp.tile([128, 2, TF], F8, tag="fh")
                u = actp.tile([128, 2, TT], F8, tag="u")
                augw = rowp.tile([1, 2, TF], F8, tag="augw")
                augx = rowp.tile([1, 2, TT], F8, tag="augx")

                box = {}
                steps = []

                def pads():
                    nc.vector.memset(ft1[:, :, 0:1], 0.0)
                    nc.vector.memset(ft1[:, :, TF + 1 : TF + 2], 0.0)
                    nc.sync.dma_start(out=augw[0:1, 1, :].bitcast(U8),
                                      in_=onesrow[0:1, :])
                    nc.sync.dma_start(out=augx[0:1, 0, :].bitcast(U8),
                                      in_=onesrow[0:1, 0:TT])

                def tap_pair(start):
                    a = ft0[0:ODIM, start : start + 512]
                    w = a.copy()
                    w.ap = _bass_rust.VecI64Pair([list(a.ap[0]), [1, 2], [1, 512]])
                    return w

                def tap_pair(start):
                    """[80, 2, 512] ifmap: plane0 = ft0 copy0 at col start,
                    plane1 = copy1 at the same col (holding tap start+1).
                    Non-overlapping plane stride TF+4."""
                    a = ft0[0:ODIM, 0, start : start + 512]
                    w = a.copy()
                    w.ap = _bass_rust.VecI64Pair(
                        [list(a.ap[0]), [TF + 6, 2], [1, 512]]
                    )
                    return w

                def f1_mm(n):
                    def f():
                        ps = ppA.tile([128, 2, 512], F32, tag="psA")
                        box[("f1", n)] = ps
                        for m in range(2):
                            for d in range(2):
                                base = WOFF["fw1"] + (d * 2 + m) * 256
                                lhs = wb[0:ODIM, base : base + 256].rearrange(
                                    "p (c w) -> p c w", c=2
                                )
                                nc.tensor.matmul(
                                    ps[:, m, :], lhs, tap_pair(n * 512 + 2 * d),
                                    start=(d == 0), stop=(d == 1), perf_mode=DR,
                                )
                    return f

                def f1_ev(n):
                    def f():
                        ps = box.pop(("f1", n))
                        ev_scale_relu(HOMES["f1e"],
                                      ft1[:, :, 1 + n * 512 : 1 + (n + 1) * 512],
                                      ps[:, :, :], AF1)
                    return f

                def t1_mm():
                    ps = ppA.tile([128, 2, 512], F32, tag="psA")
                    box["t1"] = ps
                    for m in range(2):
                        for k in range(3):
                            nc.tensor.matmul(
                                ps[:, m, :], wap("tw1", k * 2 + m),
                                tx0[:, :, k : k + TT],
                                start=(k == 0), stop=(k == 2), perf_mode=DR,
                            )

                def t1_ev():
                    ps = box.pop("t1")
                    ev_scale_relu(HOMES["t1e"], tx1[:, :, :], ps[:, :, :], A1)

                def t2_mm():
                    ps = ppA.tile([128, 2, 512], F32, tag="psA")
                    box["t2"] = ps
                    for m in range(2):
                        nc.tensor.matmul(
                            ps[:, m, :], wap("tw2", m), tx1[:, :, :],
                            start=True, stop=True, perf_mode=DR,
                        )

                def t2_ev():
                    ps = box["t2"]
                    ev_scale(HOMES["t2e"], tx2m[:, :, :], ps[:, :, :], A2)

                def txsq_f():
                    ps = box["t2"]
                    if HOMES["txsq"] == "act":
                        nc.scalar.activation(txsq[:, :, :], ps[:, :, :], AF.Square,
                                             scale=A2)
                    else:
                        eng = nc.vector if HOMES["txsq"] == "dve" else nc.gpsimd
                        eng.tensor_tensor(txsq[:, :, :], tx2m[:, :, :],
                                          tx2m[:, :, :], OP.mult)

                def t2row_mm():
                    psT = box["t2"]  # reuse t2 psum tile (already evicted)
                    for cc_ in range(2):
                        nc.tensor.matmul(psT[0:1, 0, :], onesb[:], txsq[:, cc_, :],
                                         start=(cc_ == 0), stop=(cc_ == 1))

                def t2q_ev():
                    psT = box.pop("t2")
                    _e = {"dve": nc.vector, "pool": nc.gpsimd}[HOMES["t2q"]]
                    _e.tensor_scalar(
                        augx[0:1, 1, :], psT[0:1, 0, :],
                        T2Q_MUL, C0, OP.mult, OP.subtract,
                    )

                def u_mm():
                    ps = ppA.tile([128, 2, 512], F32, tag="psA")
                    box["u"] = ps
                    for m in range(2):
                        nc.tensor.matmul(
                            ps[:, m, :], wap("w3u", m), tx2m[:, :, :],
                            start=True, stop=True, perf_mode=DR,
                        )

                def u_ev():
                    ps = box.pop("u")
                    ev_scale(HOMES["ue"], u[:, :, :], ps[:, :, :], -BU)

                def f2_mm(n):
                    def f():
                        ps = ppA.tile([128, 2, 512], F32, tag="psA")
                        box[("f2", n)] = ps
                        for m in range(2):
                            for k in range(3):
                                nc.tensor.matmul(
                                    ps[:, m, :], wap("fw2", k * 2 + m),
                                    ft1[:, :, n * 512 + k : n * 512 + k + 512],
                                    start=(k == 0), stop=(k == 2), perf_mode=DR,
                                )
                    return f

                def f2_ev(n):
                    def f():
                        ps = box.pop(("f2", n))
                        ev_scale_relu(HOMES["f2e"],
                                      ft2[:, :, n * 512 : (n + 1) * 512],
                                      ps[:, :, :], AF2)
                    return f

                def h_mm(n):
                    def f():
                        ps = ppA.tile([128, 2, 512], F32, tag="psA")
                        box[("h", n)] = ps
                        for m in range(2):
                            nc.tensor.matmul(
                                ps[:, m, :], wap("G", m),
                                ft2[:, :, n * 512 : (n + 1) * 512],
                                start=True, stop=True, perf_mode=DR,
                            )
                    return f

                def fh_ev(n):
                    def f():
                        ps = box[("h", n)]
                        sl = slice(n * 512, (n + 1) * 512)
                        if HOMES["fh"] == "split":
                            nc.vector.scalar_tensor_tensor(
                                fh[:, 0, sl], ps[:, 0, :], BH, ft2[:, 0, sl],
                                OP.mult, OP.mult)
                            nc.gpsimd.scalar_tensor_tensor(
                                fh[:, 1, sl], ps[:, 1, :], BH, ft2[:, 1, sl],
                                OP.mult, OP.mult)
                        else:
                            eng = nc.vector if HOMES["fh"] == "dve" else nc.gpsimd
                            eng.scalar_tensor_tensor(
                                fh[:, :, sl], ps[:, :, :], BH, ft2[:, :, sl],
                                OP.mult, OP.mult)
                    return f

                def f2row_mm(n):
                    def f():
                        psH = box[("h", n)]  # reuse after fh_ev consumed it
                        for c_ in range(2):
                            nc.tensor.matmul(
                                psH[0:1, 0, :], ones1[:, :],
                                fh[:, c_, n * 512 : (n + 1) * 512],
                                start=(c_ == 0), stop=(c_ == 1),
                            )
                    return f

                def f2row_ev(n):
                    def f():
                        psH = box.pop(("h", n))
                        ev_scale(HOMES["rows"],
                                 augw[0:1, 0, n * 512 : (n + 1) * 512],
                                 psH[0:1, 0, :], 1.0)
                    return f

                parts = dict(
                    pads=pads, f1_mm=f1_mm, f1_ev=f1_ev, t1_mm=t1_mm, t1_ev=t1_ev,
                    t2_mm=t2_mm, t2_ev=t2_ev, txsq=txsq_f, t2row=t2row_mm,
                    t2q=t2q_ev, u_mm=u_mm, u_ev=u_ev, f2_mm=f2_mm, f2_ev=f2_ev,
                    h_mm=h_mm, fh_ev=fh_ev, f2row_mm=f2row_mm, f2row_ev=f2row_ev,
                )
                tiles = dict(tx2m=tx2m, ft2=ft2, u=u, augw=augw, augx=augx)
                return parts, tiles

            def tail_ctx(b, tiles):
                ft2, u = tiles["ft2"], tiles["u"]
                augw, augx = tiles["augw"], tiles["augx"]
                ssum = rowp.tile([128, NF], F32, tag="ssum")
                lns = rowp.tile([128, NF], F32, tag="lns")
                dist_tiles = {}

                def pair(j):
                    def f():
                        ps = ppS.tile([128, 2, 512], F32, tag="psS")
                        for h in range(2):
                            i = 2 * j + h
                            nc.tensor.matmul(
                                ps[:, h, :],
                                ft2[:, :, i * 128 : (i + 1) * 128],
                                u[:, :, :], start=True, stop=False, perf_mode=DR,
                            )
                            nc.tensor.matmul(
                                ps[:, h, :],
                                augw[0:1, :, i * 128 : (i + 1) * 128],
                                augx[0:1, :, :], start=False, stop=True,
                                perf_mode=DR,
                            )
                        dist = distp.tile([128, 2, 512], F16, tag="dist")
                        nc.scalar.activation(dist[:], ps[:], AF.Sqrt, bias=b_sq[:])
                        dist_tiles[j] = dist
                    return f

                def exp_pair(j):
                    def f():
                        dist = dist_tiles[j]
                        if j in PAIRED_SET:
                            e = ep.tile([128, 2, 512], BF16, tag="e")
                            nc.scalar.activation(e[:], dist[:], AF.Exp,
                                                 scale=-1.0, bias=b_m[:])
                            nc.vector.tensor_reduce(
                                ssum[:, 2 * j : 2 * j + 2], e[:], AX.X, OP.add
                            )
                        else:
                            for h in range(2):
                                i = 2 * j + h
                                e = ep.tile([128, 2, 512], BF16, tag="e")
                                nc.scalar.activation(
                                    e[:, 0, :], dist[:, h, :], AF.Exp,
                                    scale=-1.0, bias=b_m[:],
                                    accum_out=ssum[:, i : i + 1],
                                )
                    return f

                def ln_half(h):
                    def f():
                        nc.scalar.activation(
                            lns[:, 8 * h : 8 * h + 8], ssum[:, 8 * h : 8 * h + 8],
                            AF.Ln, scale=float(np.exp(-M_SHIFT)),
                        )
                    return f

                def quad(qi, split=False):
                    def f():
                        oq = outqp.tile([128, 4, 512], F32, tag="outq")
                        for q in range(4):
                            i = 4 * qi + q
                            dist = dist_tiles[i // 2]
                            if i in POOL_OUTP:
                                # 2 Pool tt passes (Pool lacks TensorScalarPtr);
                                # cc enters via a stride-0 broadcast operand
                                o1 = po1.tile([128, 512], F32, tag="o1")
                                nc.gpsimd.tensor_tensor(
                                    o1[:], prior_sb[:, i, :], dist[:, i % 2, :],
                                    OP.subtract)
                                nc.gpsimd.tensor_tensor(
                                    oq[:, q, :], o1[:],
                                    lns[:, i : i + 1].broadcast_to((128, TT)),
                                    OP.subtract)
                            else:
                                nc.vector.scalar_tensor_tensor(
                                    oq[:, q, :], prior_sb[:, i, :], lns[:, i : i + 1],
                                    dist[:, i % 2, :], OP.subtract, OP.subtract,
                                )
                            if split and q % 2 == 1:
                                nc.sync.dma_start(
                                    out=outD[b, 512 * qi + 256 * (q // 2) :
                                             512 * qi + 256 * (q // 2) + 256,
                                             :].rearrange("(q p) t -> p q t", p=128),
                                    in_=oq[:, q - 1 : q + 1, :],
                                )
                        if not split:
                            nc.sync.dma_start(
                                out=outD[b, 512 * qi : 512 * (qi + 1), :].rearrange(
                                    "(q p) t -> p q t", p=128
                                ),
                                in_=oq[:, :, :],
                            )
                    return f

                return pair, exp_pair, ln_half, quad

            # ================= emission =================
            tx0_0 = load_tx0(0)
            ft0_0 = load_ft0(0)
            tx0_1 = load_tx0(1)
            ft0_1 = load_ft0(1)
            nc.sync.dma_start(
                out=prior_sb[:], in_=priorD.rearrange("(i p) t -> p i t", p=128)
            )

            def batch_steps(b, tx0, ft0):
                p, tiles = batch_ctx(b, tx0, ft0)
                t = tail_ctx(b, tiles)
                pair, expp, lnh, quad = t
                return [
                    p["pads"],
                    p["f1_mm"](0), p["t1_mm"], p["f1_ev"](0), p["t1_ev"],
                    p["f1_mm"](1), p["t2_mm"], p["f1_ev"](1), p["t2_ev"],
                    p["f2_mm"](0), p["txsq"], p["f2_ev"](0),
                    p["t2row"], p["h_mm"](0), p["t2q"], p["fh_ev"](0),
                    p["f1_mm"](2), p["u_mm"], p["f1_ev"](2), p["u_ev"],
                    p["f2row_mm"](0), p["f2row_ev"](0),
                    pair(0),
                    p["f2_mm"](1), p["f1_mm"](3),
                    pair(1), p["f2_ev"](1), expp(0), p["f1_ev"](3),
                    p["h_mm"](1), p["fh_ev"](1),
                    p["f2row_mm"](1), p["f2row_ev"](1),
                    pair(2), p["f2_mm"](2), expp(1), p["f2_ev"](2),
                    pair(3), p["h_mm"](2), expp(2), p["fh_ev"](2),
                    p["f2row_mm"](2), p["f2row_ev"](2),
                    pair(4), p["f2_mm"](3), expp(3), p["f2_ev"](3),
                    lnh(0), quad(0),
                    pair(5), p["h_mm"](3), expp(4), p["fh_ev"](3),
                    p["f2row_mm"](3), p["f2row_ev"](3),
                    quad(1),
                    pair(6), expp(5), pair(7), expp(6), expp(7),
                    lnh(1), quad(2, split=True), quad(3, split=True),
                ]

            steps0 = batch_steps(0, tx0_0, ft0_0)
            steps1 = batch_steps(1, tx0_1, ft0_1)
            import os
            STAG = int(os.environ.get("KV2_STAGGER", "24"))
            merged = []
            i0 = i1 = 0
            # emit STAG steps of batch0 first, then alternate
            while i0 < len(steps0) or i1 < len(steps1):
                if i0 < len(steps0):
                    merged.append(steps0[i0]); i0 += 1
                if i0 >= STAG and i1 < len(steps1):
                    merged.append(steps1[i1]); i1 += 1
            for s in merged:
                s()

    _split_excess_waits(nc)
    return nc


_NC = None


def _get_nc():
    global _NC
    if _NC is None:
        _NC = _build_nc()
    return _NC


def _q8(x, scale, limit=230.0):
    import ml_dtypes
    y = np.asarray(x, np.float32) * scale
    m = np.abs(y).max()
    assert m < limit, f"fp8 range exceeded: {m} * (scale {scale})"
    return y.astype(ml_dtypes.float8_e4m3)


def _prep_inputs(text, feats, t_w1, t_b1, t_w2, t_b2,
                 f_w1, f_b1, f_w2, f_b2, f_w3, f_b3):
    for bias in (t_b1, t_b2, f_b1, f_b2, f_b3):
        assert not np.asarray(bias).any(), "kernel assumes zero biases (per spec)"
    c = np.ascontiguousarray
    f4 = np.float32

    textT = _q8(c(np.asarray(text, f4).transpose(0, 2, 1)), SX)    # [B,256,512]
    featsT = _q8(c(np.asarray(feats, f4).transpose(0, 2, 1)), SX)  # [B,80,2048]

    # wblob [128, WBLOB_W] fp8: per lhsT (k/m) block of [p, 2, 128]
    blob = np.zeros((128, WBLOB_W), np.float32)

    def put(name, idx, arr):  # arr [128, 2, 128] f32 (pre-scale applied)
        base = WOFF[name] + idx * 256
        blob[:, base : base + 256] = arr.reshape(128, 256)

    tw1 = np.asarray(t_w1, f4).transpose(2, 1, 0)  # [3, cin, cout]
    for k in range(3):
        for m in range(2):
            a = tw1[k].reshape(2, 128, 256)[:, :, m * 128 : (m + 1) * 128]
            put("tw1", k * 2 + m, a.transpose(1, 0, 2) * S_TW1)
    tw2 = np.asarray(t_w2, f4)[:, :, 0].T  # [cin, cout]
    for m in range(2):
        a = tw2.reshape(2, 128, 256)[:, :, m * 128 : (m + 1) * 128]
        put("tw2", m, a.transpose(1, 0, 2) * S_TW2)
    fw1 = np.asarray(f_w1, f4).transpose(2, 1, 0)  # [3, 80, 256]
    for d in range(2):
        for m in range(2):
            a = np.zeros((128, 2, 128), np.float32)
            a[:80, 0] = fw1[2 * d][:, m * 128 : (m + 1) * 128]
            if 2 * d + 1 < 3:
                a[:80, 1] = fw1[2 * d + 1][:, m * 128 : (m + 1) * 128]
            put("fw1", d * 2 + m, a * S_FW1)
    fw2 = np.asarray(f_w2, f4).transpose(2, 1, 0)
    for k in range(3):
        for m in range(2):
            a = fw2[k].reshape(2, 128, 256)[:, :, m * 128 : (m + 1) * 128]
            put("fw2", k * 2 + m, a.transpose(1, 0, 2) * S_FW2)
    W3 = np.asarray(f_w3, f4)[:, :, 0]  # [cout, cin]
    G = (W3.T @ W3).astype(np.float32)
    for m in range(2):
        a = G.reshape(2, 128, 256)[:, :, m * 128 : (m + 1) * 128]
        put("G", m, a.transpose(1, 0, 2) * S_G)
    for m in range(2):  # w3u lhsT[c, d]: W3 itself
        a = W3.reshape(2, 128, 256)[:, :, m * 128 : (m + 1) * 128]
        put("w3u", m, a.transpose(1, 0, 2) * S_W3)

    m = np.abs(blob).max()
    assert m < 230.0, f"wblob fp8 range exceeded: {m}"
    import ml_dtypes
    blob8 = blob.astype(ml_dtypes.float8_e4m3)

    import ml_dtypes as _mld
    ones8 = np.ones((1, TF), _mld.float8_e4m3)
    shared = {
        "wblob": blob8.view(np.uint8),
        "onesrow": ones8.view(np.uint8),
        "prior": _beta_binomial_prior().astype(np.float16),
    }
    in_maps = []
    for core in range(N_CORES):
        mcore = dict(shared)
        mcore["textT"] = c(textT[core * B_LOC : (core + 1) * B_LOC]).view(np.uint8)
        mcore["featsT"] = c(featsT[core * B_LOC : (core + 1) * B_LOC]).view(np.uint8)
        in_maps.append(mcore)
    return in_maps


_CALLABLE = None


def _build_callable():
    """Compile once; return fn(in_maps) -> per-core output dicts (axon path)."""
    import jax
    import jax.numpy as jnp
    from jax.sharding import Mesh, NamedSharding, PartitionSpec
    from jax.experimental.shard_map import shard_map
    from concourse.bass2jax import (
        _bass_exec_p,
        install_neuronx_cc_hook,
        partition_id_tensor,
    )

    nc = _get_nc()
    install_neuronx_cc_hook()
    partition_name = nc.partition_id_tensor.name if nc.partition_id_tensor else None
    in_names, out_names, out_avals, zero_shapes = [], [], [], []
    for alloc in nc.m.functions[0].allocations:
        if not isinstance(alloc, mybir.MemoryLocationSet):
            continue
        name = alloc.memorylocations[0].name
        if alloc.kind == "ExternalInput":
            if name != partition_name:
                in_names.append(name)
        elif alloc.kind == "ExternalOutput":
            shape = tuple(alloc.tensor_shape)
            dtype = mybir.dt.np(alloc.dtype)
            out_names.append(name)
            out_avals.append(jax.core.ShapedArray(shape, dtype))
            zero_shapes.append(((N_CORES * shape[0],) + shape[1:], dtype))
    n_params = len(in_names)
    n_outs = len(out_avals)
    all_in_names = list(in_names) + out_names
    if partition_name is not None:
        all_in_names.append(partition_name)
    donate = tuple(range(n_params, n_params + n_outs))

    def _body(*args):
        operands = list(args)
        if partition_name is not None:
            operands.append(partition_id_tensor())
        outs = _bass_exec_p.bind(
            *operands,
            out_avals=tuple(out_avals),
            in_names=tuple(all_in_names),
            out_names=tuple(out_names),
            lowering_input_output_aliases=(),
            sim_require_finite=True,
            sim_require_nnan=True,
            nc=nc,
        )
        return tuple(outs)

    devices = jax.devices()[:N_CORES]
    mesh = Mesh(np.asarray(devices), ("core",))
    fn = jax.jit(
        shard_map(
            _body,
            mesh=mesh,
            in_specs=(PartitionSpec("core"),) * (n_params + n_outs),
            out_specs=(PartitionSpec("core"),) * n_outs,
            check_rep=False,
        ),
        donate_argnums=donate,
        keep_unused=True,
    )
    sharding = NamedSharding(mesh, PartitionSpec("core"))
    zfn = jax.jit(
        lambda: tuple(jnp.zeros(s, d) for s, d in zero_shapes),
        out_shardings=tuple(sharding for _ in zero_shapes),
    )

    def call(in_maps):
        concat_in = [
            np.concatenate([np.asarray(in_maps[c][n]) for c in range(N_CORES)], axis=0)
            for n in in_names
        ]
        out_arrs = fn(*concat_in, *zfn())
        return [
            {
                name: np.asarray(out_arrs[i]).reshape(
                    N_CORES, *out_avals[i].shape
                )[c]
                for i, name in enumerate(out_names)
            }
            for c in range(N_CORES)
        ]

    return call


def _run(inputs, **kw):
    global _CALLABLE
    import time as _time

    in_maps = _prep_inputs(
        inputs["text"], inputs["feats"],
        inputs["t_w1"], inputs["t_b1"], inputs["t_w2"], inputs["t_b2"],
        inputs["f_w1"], inputs["f_b1"], inputs["f_w2"], inputs["f_b2"],
        inputs["f_w3"], inputs["f_b3"],
    )
    results = None
    last_err = None
    if _CALLABLE is not False:
        for attempt in range(3):
            try:
                if _CALLABLE is None:
                    from concourse._compat import axon_active

                    if not axon_active():
                        raise RuntimeError("axon not active; use native path")
                    _CALLABLE = _build_callable()
                results = _CALLABLE(in_maps)
                break
            except Exception as e:
                last_err = e
                results = None
                if attempt < 2:
                    _time.sleep(20 * (attempt + 1))
        if results is None:
            _CALLABLE = False
    if results is None:
        from concourse.bass_utils import run_bass_kernel_spmd

        for attempt in range(3):
            try:
                results = run_bass_kernel_spmd(
                    _get_nc(), in_maps, core_ids=list(range(N_CORES))
                ).results
                break
            except Exception as e:
                last_err = e
                results = None
                if attempt < 2:
                    _time.sleep(20 * (attempt + 1))
    if results is None:
        raise last_err
    out = np.concatenate([r["out"] for r in results], axis=0)
    return out, results


def kernel(**inputs) -> np.ndarray:
    out, _ = _run(inputs)
    return out


# revision 17
# speedup vs baseline: 1.1028x; 1.0017x over previous
"""AlignmentModule on 8 Trainium2 cores — fp8 DoubleRow rewrite.

Data-parallel over batch (2 per core). All matmuls run as fp8e4 DoubleRow
(2 K-planes per pass, 0.5 cycles/row = 4x fp32r): conv stacks t1,t2 / f1,f2,
the Gram matmul H=G.ft2, the text-side-absorbed f3 (u = W3^T tx2m — the 1x1
f3 conv is algebraically moved to the 4x-smaller text side; f2 norms come from
fh = ft2*H read straight off H's PSUM), the score cross ft2^T.u, and a K=1
augmented DR that adds f2[f] + (t2[t]-c0) rank-2 terms into the same PSUM.

Norm rows: f2row/t2row are ones-weight DR matmuls into PSUM row slots
(partitions 0/32/64 + bank2), evicted same-partition to fp8 and DMA-gathered
to the partition-0 aug operand rows (engines cannot cross partitions; DMA can).

Tail per f-tile: ACT sqrt (psum pair -> fp16 dist), ACT exp(14-dist) with
f32 row-accum (a tunable number of pairs instead run paired-exp + DVE reduce),
one ACT ln per batch, then outp = (prior_f16 - cc) - dist on Pool, DMA out in
4-tile quads. Engine assignment of evictions is tuned: Pool takes t1/t2/u +
outp, DVE takes f1/f2/fh/txsq/rows, ACT takes sqrt/exp/ln.

Host pre-quantizes inputs/weights to fp8 (power-of-2 scales, ranges asserted)
and ships them as uint8 bits; fp16 prior. Total rel err vs the f32 reference
is ~6.5e-4 (validated offline), dominated by fp8 conv activations and the
fp16 prior.
"""

import numpy as np

import bass_rust as _bass_rust
import concourse.bass as bass
import concourse.mybir as mybir
from concourse.tile import TileContext

F32 = mybir.dt.float32
F16 = mybir.dt.float16
BF16 = mybir.dt.bfloat16
F8 = mybir.dt.float8e4
U8 = mybir.dt.uint8
AF = mybir.ActivationFunctionType
OP = mybir.AluOpType
AX = mybir.AxisListType
DR = mybir.MatmulPerfMode.DoubleRow

B, T_TEXT, T_FEATS, ADIM, ODIM = 16, 512, 2048, 256, 80
N_CORES = 8
B_LOC = B // N_CORES
TT, TF = T_TEXT, T_FEATS
NT = TF // 512   # 4 feats chunks of 512
NF = TF // 128   # 16 f-tiles per batch

# ---- fixed power-of-2 scales (validated in opt/sim_numerics2.py) ----
SX = 16.0
S_TW1, S_TW2 = 1024.0, 512.0
S_FW1, S_FW2, S_W3 = 512.0, 1024.0, 512.0
S_G = 64.0
A1 = 1.0 / 512.0       # tx1 evict; tile = 32*true
A2 = 1.0 / 2048.0      # tx2m evict; tile = 8*true
AF1 = 1.0 / 512.0      # ft1 tile = 16*true
AF2 = 1.0 / 8192.0     # ft2 tile = 2*true
BH = 1.0 / 32.0        # fh evict scalar
BU = 1.0 / 4096.0      # u evict; u tile = -1*true(W3^T tx2)
ONES2_F2 = 1.0 / 8.0   # f2row ones-weight = 1/(S_G*s_ft2^2*BH)
T2Q_MUL = 1.0 / 64.0   # t2row evict mult = 1/s_tx2m^2
C0 = 192.0             # t2q offset; sqrt bias adds it back
M_SHIFT = 14.0
SQ_BIAS = C0           # dist = sqrt(psum + C0)

PAIRED_SET = (1, 2, 3)  # score pairs using paired-exp + DVE reduce
POOL_OUTP = (2, 5, 8, 11, 14)  # f-tiles whose outp runs as 2 Pool tt passes
# engine homes for evictions: "dve" | "pool" | "act"(relu/identity/square ok)
# NOTE: GPSIMD (pool) cannot access PSUM on HW — psum-evictions are dve/act only.
HOMES = {
    "t1e": "act", "t2e": "act", "ue": "dve",
    "f1e": "dve", "f2e": "dve", "fh": "dve",
    "rows": "act", "t2q": "dve", "txsq": "act",
    "outp": "dve",  # pool cannot do TensorScalarPtr on HW
}

WOFF = {}              # wblob free-dim offsets, filled by _pack_weights layout
WBLOB_W = 1536 + 512 + 1024 + 1536 + 512 + 512  # tw1,tw2,fw1,fw2,G,w3u


def _wblob_offsets():
    off, o = {}, 0
    for name, w in (("tw1", 1536), ("tw2", 512), ("fw1", 1024),
                    ("fw2", 1536), ("G", 512), ("w3u", 512)):
        off[name] = o
        o += w
    assert o == WBLOB_W
    return off


WOFF = _wblob_offsets()


def _split_excess_waits(nc, limit=1):
    """walrus CoreV3 CTRL codegen rejects >1 sync-wait per instruction.
    Hoist excess waits onto preceding NOPs on the same engine."""
    ctr = 0
    for f in nc.m.functions:
        for bb in f.blocks:
            insts = bb.instructions
            idx = 0
            while idx < len(insts):
                ins = insts[idx]
                si = ins.sync_info
                if si is not None and len(si.on_wait) > limit:
                    waits = list(si.on_wait)
                    extra, keep = waits[:-limit], waits[-limit:]
                    si.on_wait = keep
                    pos = idx
                    for j in range(0, len(extra), limit):
                        nop = mybir.InstNoOp(name=f"waitsplit_{ctr}", ins=[], outs=[])
                        ctr += 1
                        nop.engine = ins.engine
                        nop.sync_info = mybir.SyncInfo(
                            on_wait=extra[j : j + limit], on_update=[]
                        )
                        insts.insert(pos, nop)
                        pos += 1
                        idx += 1
                idx += 1
    return ctr


def _beta_binomial_prior():
    from scipy.special import gammaln

    T, N = T_FEATS, T_TEXT
    a = np.arange(1, T + 1, dtype=np.float64)[:, None]
    b = (T - np.arange(1, T + 1, dtype=np.float64) + 1.0)[:, None]
    k = np.arange(N, dtype=np.float64)[None, :]
    n = float(N)

    def betaln(x, y):
        return gammaln(x) + gammaln(y) - gammaln(x + y)

    logp = (
        gammaln(n + 1.0) - gammaln(k + 1.0) - gammaln(n - k + 1.0)
        + betaln(k + a, n - k + b) - betaln(a, b)
    )
    return logp.astype(np.float32)


def _build_nc():
    nc = bass.Bass(name="alignment")

    textT = nc.dram_tensor("textT", [B_LOC, ADIM, TT], U8, kind="ExternalInput")
    featsT = nc.dram_tensor("featsT", [B_LOC, ODIM, TF], U8, kind="ExternalInput")
    wblob = nc.dram_tensor("wblob", [128, WBLOB_W], U8, kind="ExternalInput")
    onesrow = nc.dram_tensor("onesrow", [1, TF], U8, kind="ExternalInput")
    priorD = nc.dram_tensor("prior", [TF, TT], F16, kind="ExternalInput")
    outD = nc.dram_tensor("out", [B_LOC, TF, TT], F32, kind="ExternalOutput")

    with TileContext(nc) as tc:
        with (
            tc.tile_pool(name="const", bufs=1) as const,
            tc.tile_pool(name="inp", bufs=2) as inp,
            tc.tile_pool(name="actp", bufs=2) as actp,
            tc.tile_pool(name="rowp", bufs=2) as rowp,
            tc.tile_pool(name="distp", bufs=17) as distp,
            tc.tile_pool(name="ep", bufs=3) as ep,
            tc.tile_pool(name="outq", bufs=3) as outqp,
            tc.tile_pool(name="po1", bufs=2) as po1,
            tc.tile_pool(name="ppA", bufs=2, space="PSUM") as ppA,
            tc.tile_pool(name="ppS", bufs=2, space="PSUM") as ppS,
        ):
            # ---- constants / weights ----
            wb = const.tile([128, WBLOB_W], F8)
            nc.sync.dma_start(out=wb[:].bitcast(U8), in_=wblob[:])
            ones1 = const.tile([128, 1], F8)
            nc.vector.memset(ones1[:], ONES2_F2)
            onesb = const.tile([128, 1], BF16)
            nc.vector.memset(onesb[:], 1.0)
            b_sq = const.tile([128, 1], F32)
            nc.vector.memset(b_sq[:], SQ_BIAS)
            b_m = const.tile([128, 1], F32)
            nc.vector.memset(b_m[:], M_SHIFT)
            prior_sb = const.tile([128, NF, TT], F16)

            def ev_scale_relu(home, out, ps, scale):
                if home == "act":
                    nc.scalar.activation(out, ps, AF.Relu, scale=scale)
                elif home == "split":
                    nc.vector.tensor_scalar(out[:, 0, :], ps[:, 0, :], scale,
                                            0.0, OP.mult, OP.max)
                    nc.gpsimd.tensor_scalar(out[:, 1, :], ps[:, 1, :], scale,
                                            0.0, OP.mult, OP.max)
                else:
                    eng = nc.vector if home == "dve" else nc.gpsimd
                    eng.tensor_scalar(out, ps, scale, 0.0, OP.mult, OP.max)

            def ev_scale(home, out, ps, scale):
                if home == "act":
                    nc.scalar.activation(out, ps, AF.Identity, scale=scale)
                elif home == "split":
                    nc.vector.tensor_scalar(out[:, 0, :], ps[:, 0, :], scale,
                                            None, OP.mult)
                    nc.gpsimd.tensor_scalar(out[:, 1, :], ps[:, 1, :], scale,
                                            None, OP.mult)
                else:
                    eng = nc.vector if home == "dve" else nc.gpsimd
                    eng.tensor_scalar(out, ps, scale, None, OP.mult)

            def wap(name, idx, planes=2, width=128):
                base = WOFF[name] + idx * planes * width
                return wb[:, base : base + planes * width].rearrange(
                    "p (c w) -> p c w", c=planes
                )

            def load_tx0(b):
                tx0 = inp.tile([128, 2, TT + 2], F8, tag="tx0")
                nc.vector.memset(tx0[:, :, 0:1], 0.0)
                nc.vector.memset(tx0[:, :, TT + 1 : TT + 2], 0.0)
                nc.sync.dma_start(
                    out=tx0[:, :, 1 : TT + 1].bitcast(U8),
                    in_=textT[b].rearrange("(c p) t -> p c t", p=128),
                )
                return tx0

            def load_ft0(b):
                # two copies, plane1 shifted +1 col, so the f1 tap-pair DR reads
                # non-overlapping ifmap planes (overlapping APs wedge the PE)
                ft0 = inp.tile([ODIM, 2, TF + 4], F8, tag="ft0")
                nc.vector.memset(ft0[:, :, 0:2], 0.0)
                nc.vector.memset(ft0[:, :, TF + 1 : TF + 4], 0.0)
                nc.sync.dma_start(out=ft0[:, 0, 1 : TF + 1].bitcast(U8), in_=featsT[b])
                nc.sync.dma_start(out=ft0[:, 1, 2 : TF + 2].bitcast(U8), in_=featsT[b])
                return ft0

            def batch_ctx(b, tx0, ft0):
                """Allocate per-batch tiles and return the conv step list plus
                the tile handles the tail needs."""
                tx1 = actp.tile([128, 2, TT], F8, tag="tx1")
                tx2m = actp.tile([128, 2, TT], F8, tag="tx2m")
                txsq = actp.tile([128, 2, TT], BF16, tag="txsq")
                ft1 = actp.tile([128, 2, TF + 2], F8, tag="ft1")
                ft2 = actp.tile([128, 2, TF], F8, tag="ft2")
                fh = actp.tile([128, 2, TF], F8, tag="fh")
                u = actp.tile([128, 2, TT], F8, tag="u")
                augw = rowp.tile([1, 2, TF], F8, tag="augw")
                augx = rowp.tile([1, 2, TT], F8, tag="augx")

                box = {}
                steps = []

                def pads():
                    nc.vector.memset(ft1[:, :, 0:1], 0.0)
                    nc.vector.memset(ft1[:, :, TF + 1 : TF + 2], 0.0)
                    nc.sync.dma_start(out=augw[0:1, 1, :].bitcast(U8),
                                      in_=onesrow[0:1, :])
                    nc.sync.dma_start(out=augx[0:1, 0, :].bitcast(U8),
                                      in_=onesrow[0:1, 0:TT])

                def tap_pair(start):
                    a = ft0[0:ODIM, start : start + 512]
                    w = a.copy()
                    w.ap = _bass_rust.VecI64Pair([list(a.ap[0]), [1, 2], [1, 512]])
                    return w

                def tap_pair(start):
                    """[80, 2, 512] ifmap: plane0 = ft0 copy0 at col start,
                    plane1 = copy1 at the same col (holding tap start+1).
                    Non-overlapping plane stride TF+4."""
                    a = ft0[0:ODIM, 0, start : start + 512]
                    w = a.copy()
                    w.ap = _bass_rust.VecI64Pair(
                        [list(a.ap[0]), [TF + 6, 2], [1, 512]]
                    )
                    return w

                def f1_mm(n):
                    def f():
                        ps = ppA.tile([128, 2, 512], F32, tag="psA")
                        box[("f1", n)] = ps
                        for m in range(2):
                            for d in range(2):
                                base = WOFF["fw1"] + (d * 2 + m) * 256
                                lhs = wb[0:ODIM, base : base + 256].rearrange(
                                    "p (c w) -> p c w", c=2
                                )
                                nc.tensor.matmul(
                                    ps[:, m, :], lhs, tap_pair(n * 512 + 2 * d),
                                    start=(d == 0), stop=(d == 1), perf_mode=DR,
                                )
                    return f

                def f1_ev(n):
                    def f():
                        ps = box.pop(("f1", n))
                        ev_scale_relu(HOMES["f1e"],
                                      ft1[:, :, 1 + n * 512 : 1 + (n + 1) * 512],
                                      ps[:, :, :], AF1)
                    return f

                def t1_mm():
                    ps = ppA.tile([128, 2, 512], F32, tag="psA")
                    box["t1"] = ps
                    for m in range(2):
                        for k in range(3):
                            nc.tensor.matmul(
                                ps[:, m, :], wap("tw1", k * 2 + m),
                                tx0[:, :, k : k + TT],
                                start=(k == 0), stop=(k == 2), perf_mode=DR,
                            )

                def t1_ev():
                    ps = box.pop("t1")
                    ev_scale_relu(HOMES["t1e"], tx1[:, :, :], ps[:, :, :], A1)

                def t2_mm():
                    ps = ppA.tile([128, 2, 512], F32, tag="psA")
                    box["t2"] = ps
                    for m in range(2):
                        nc.tensor.matmul(
                            ps[:, m, :], wap("tw2", m), tx1[:, :, :],
                            start=True, stop=True, perf_mode=DR,
                        )

                def t2_ev():
                    ps = box["t2"]
                    ev_scale(HOMES["t2e"], tx2m[:, :, :], ps[:, :, :], A2)

                def txsq_f():
                    ps = box["t2"]
                    if HOMES["txsq"] == "act":
                        nc.scalar.activation(txsq[:, :, :], ps[:, :, :], AF.Square,
                                             scale=A2)
                    else:
                        eng = nc.vector if HOMES["txsq"] == "dve" else nc.gpsimd
                        eng.tensor_tensor(txsq[:, :, :], tx2m[:, :, :],
                                          tx2m[:, :, :], OP.mult)

                def t2row_mm():
                    psT = box["t2"]  # reuse t2 psum tile (already evicted)
                    for cc_ in range(2):
                        nc.tensor.matmul(psT[0:1, 0, :], onesb[:], txsq[:, cc_, :],
                                         start=(cc_ == 0), stop=(cc_ == 1))

                def t2q_ev():
                    psT = box.pop("t2")
                    _e = {"dve": nc.vector, "pool": nc.gpsimd}[HOMES["t2q"]]
                    _e.tensor_scalar(
                        augx[0:1, 1, :], psT[0:1, 0, :],
                        T2Q_MUL, C0, OP.mult, OP.subtract,
                    )

                def u_mm():
                    ps = ppA.tile([128, 2, 512], F32, tag="psA")
                    box["u"] = ps
                    for m in range(2):
                        nc.tensor.matmul(
                            ps[:, m, :], wap("w3u", m), tx2m[:, :, :],
                            start=True, stop=True, perf_mode=DR,
                        )

                def u_ev():
                    ps = box.pop("u")
                    ev_scale(HOMES["ue"], u[:, :, :], ps[:, :, :], -BU)

                def f2_mm(n):
                    def f():
                        ps = ppA.tile([128, 2, 512], F32, tag="psA")
                        box[("f2", n)] = ps
                        for m in range(2):
                            for k in range(3):
                                nc.tensor.matmul(
                                    ps[:, m, :], wap("fw2", k * 2 + m),
                                    ft1[:, :, n * 512 + k : n * 512 + k + 512],
                                    start=(k == 0), stop=(k == 2), perf_mode=DR,
                                )
                    return f

                def f2_ev(n):
                    def f():
                        ps = box.pop(("f2", n))
                        ev_scale_relu(HOMES["f2e"],
                                      ft2[:, :, n * 512 : (n + 1) * 512],
                                      ps[:, :, :], AF2)
                    return f

                def h_mm(n):
                    def f():
                        ps = ppA.tile([128, 2, 512], F32, tag="psA")
                        box[("h", n)] = ps
                        for m in range(2):
                            nc.tensor.matmul(
                                ps[:, m, :], wap("G", m),
                                ft2[:, :, n * 512 : (n + 1) * 512],
                                start=True, stop=True, perf_mode=DR,
                            )
                    return f

                def fh_ev(n):
                    def f():
                        ps = box[("h", n)]
                        sl = slice(n * 512, (n + 1) * 512)
                        if HOMES["fh"] == "split":
                            nc.vector.scalar_tensor_tensor(
                                fh[:, 0, sl], ps[:, 0, :], BH, ft2[:, 0, sl],
                                OP.mult, OP.mult)
                            nc.gpsimd.scalar_tensor_tensor(
                                fh[:, 1, sl], ps[:, 1, :], BH, ft2[:, 1, sl],
                                OP.mult, OP.mult)
                        else:
                            eng = nc.vector if HOMES["fh"] == "dve" else nc.gpsimd
                            eng.scalar_tensor_tensor(
                                fh[:, :, sl], ps[:, :, :], BH, ft2[:, :, sl],
                                OP.mult, OP.mult)
                    return f

                def f2row_mm(n):
                    def f():
                        psH = box[("h", n)]  # reuse after fh_ev consumed it
                        for c_ in range(2):
                            nc.tensor.matmul(
                                psH[0:1, 0, :], ones1[:, :],
                                fh[:, c_, n * 512 : (n + 1) * 512],
                                start=(c_ == 0), stop=(c_ == 1),
                            )
                    return f

                def f2row_ev(n):
                    def f():
                        psH = box.pop(("h", n))
                        ev_scale(HOMES["rows"],
                                 augw[0:1, 0, n * 512 : (n + 1) * 512],
                                 psH[0:1, 0, :], 1.0)
                    return f

                parts = dict(
                    pads=pads, f1_mm=f1_mm, f1_ev=f1_ev, t1_mm=t1_mm, t1_ev=t1_ev,
                    t2_mm=t2_mm, t2_ev=t2_ev, txsq=txsq_f, t2row=t2row_mm,
                    t2q=t2q_ev, u_mm=u_mm, u_ev=u_ev, f2_mm=f2_mm, f2_ev=f2_ev,
                    h_mm=h_mm, fh_ev=fh_ev, f2row_mm=f2row_mm, f2row_ev=f2row_ev,
                )
                tiles = dict(tx2m=tx2m, ft2=ft2, u=u, augw=augw, augx=augx)
                return parts, tiles

            def tail_ctx(b, tiles):
                ft2, u = tiles["ft2"], tiles["u"]
                augw, augx = tiles["augw"], tiles["augx"]
                ssum = rowp.tile([128, NF], F32, tag="ssum")
                lns = rowp.tile([128, NF], F32, tag="lns")
                dist_tiles = {}

                def pair(j):
                    def f():
                        ps = ppS.tile([128, 2, 512], F32, tag="psS")
                        for h in range(2):
                            i = 2 * j + h
                            nc.tensor.matmul(
                                ps[:, h, :],
                                ft2[:, :, i * 128 : (i + 1) * 128],
                                u[:, :, :], start=True, stop=False, perf_mode=DR,
                            )
                            nc.tensor.matmul(
                                ps[:, h, :],
                                augw[0:1, :, i * 128 : (i + 1) * 128],
                                augx[0:1, :, :], start=False, stop=True,
                                perf_mode=DR,
                            )
                        dist = distp.tile([128, 2, 512], F16, tag="dist")
                        nc.scalar.activation(dist[:], ps[:], AF.Sqrt, bias=b_sq[:])
                        dist_tiles[j] = dist
                    return f

                def exp_pair(j):
                    def f():
                        dist = dist_tiles[j]
                        if j in PAIRED_SET:
                            e = ep.tile([128, 2, 512], BF16, tag="e")
                            nc.scalar.activation(e[:], dist[:], AF.Exp,
                                                 scale=-1.0, bias=b_m[:])
                            nc.vector.tensor_reduce(
                                ssum[:, 2 * j : 2 * j + 2], e[:], AX.X, OP.add
                            )
                        else:
                            for h in range(2):
                                i = 2 * j + h
                                e = ep.tile([128, 2, 512], BF16, tag="e")
                                nc.scalar.activation(
                                    e[:, 0, :], dist[:, h, :], AF.Exp,
                                    scale=-1.0, bias=b_m[:],
                                    accum_out=ssum[:, i : i + 1],
                                )
                    return f

                def ln_half(h):
                    def f():
                        nc.scalar.activation(
                            lns[:, 8 * h : 8 * h + 8], ssum[:, 8 * h : 8 * h + 8],
                            AF.Ln, scale=float(np.exp(-M_SHIFT)),
                        )
                    return f

                def quad(qi, split=False):
                    def f():
                        oq = outqp.tile([128, 4, 512], F32, tag="outq")
                        for q in range(4):
                            i = 4 * qi + q
                            dist = dist_tiles[i // 2]
                            if i in POOL_OUTP:
                                # 2 Pool tt passes (Pool lacks TensorScalarPtr);
                                # cc enters via a stride-0 broadcast operand
                                o1 = po1.tile([128, 512], F32, tag="o1")
                                nc.gpsimd.tensor_tensor(
                                    o1[:], prior_sb[:, i, :], dist[:, i % 2, :],
                                    OP.subtract)
                                nc.gpsimd.tensor_tensor(
                                    oq[:, q, :], o1[:],
                                    lns[:, i : i + 1].broadcast_to((128, TT)),
                                    OP.subtract)
                            else:
                                nc.vector.scalar_tensor_tensor(
                                    oq[:, q, :], prior_sb[:, i, :], lns[:, i : i + 1],
                                    dist[:, i % 2, :], OP.subtract, OP.subtract,
                                )
                            if split and q % 2 == 1:
                                nc.sync.dma_start(
                                    out=outD[b, 512 * qi + 256 * (q // 2) :
                                             512 * qi + 256 * (q // 2) + 256,
                                             :].rearrange("(q p) t -> p q t", p=128),
                                    in_=oq[:, q - 1 : q + 1, :],
                                )
                        if not split:
                            nc.sync.dma_start(
                                out=outD[b, 512 * qi : 512 * (qi + 1), :].rearrange(
                                    "(q p) t -> p q t", p=128
                                ),
                                in_=oq[:, :, :],
                            )
                    return f

                return pair, exp_pair, ln_half, quad

            # ================= emission =================
            tx0_0 = load_tx0(0)
            ft0_0 = load_ft0(0)
            tx0_1 = load_tx0(1)
            ft0_1 = load_ft0(1)
            nc.sync.dma_start(
                out=prior_sb[:], in_=priorD.rearrange("(i p) t -> p i t", p=128)
            )

            def batch_steps(b, tx0, ft0):
                p, tiles = batch_ctx(b, tx0, ft0)
                t = tail_ctx(b, tiles)
                pair, expp, lnh, quad = t
                return [
                    p["pads"],
                    p["f1_mm"](0), p["t1_mm"], p["f1_ev"](0), p["t1_ev"],
                    p["f1_mm"](1), p["t2_mm"], p["f1_ev"](1), p["t2_ev"],
                    p["f2_mm"](0), p["txsq"], p["f2_ev"](0),
                    p["t2row"], p["h_mm"](0), p["t2q"], p["fh_ev"](0),
                    p["f1_mm"](2), p["u_mm"], p["f1_ev"](2), p["u_ev"],
                    p["f2row_mm"](0), p["f2row_ev"](0),
                    pair(0),
                    p["f2_mm"](1), p["f1_mm"](3),
                    pair(1), p["f2_ev"](1), expp(0), p["f1_ev"](3),
                    p["h_mm"](1), p["fh_ev"](1),
                    p["f2row_mm"](1), p["f2row_ev"](1),
                    pair(2), p["f2_mm"](2), expp(1), p["f2_ev"](2),
                    pair(3), p["h_mm"](2), expp(2), p["fh_ev"](2),
                    p["f2row_mm"](2), p["f2row_ev"](2),
                    pair(4), p["f2_mm"](3), expp(3), p["f2_ev"](3),
                    lnh(0), quad(0),
                    pair(5), p["h_mm"](3), expp(4), p["fh_ev"](3),
                    p["f2row_mm"](3), p["f2row_ev"](3),
                    quad(1),
                    pair(6), expp(5), pair(7), expp(6), expp(7),
                    lnh(1), quad(2, split=True), quad(3, split=True),
                ]

            steps0 = batch_steps(0, tx0_0, ft0_0)
            steps1 = batch_steps(1, tx0_1, ft0_1)
            import os
            STAG = int(os.environ.get("KV2_STAGGER", "24"))
            merged = []
            i0 = i1 = 0
            # emit STAG steps of batch0 first, then alternate
            while i0 < len(steps0) or i1 < len(steps1):
                if i0 < len(steps0):
                    merged.append(steps0[i0]); i0 += 1
                if i0 >= STAG and i1 < len(steps1):
                    merged.append(steps1[i1]); i1 += 1
            for s in merged:
                s()

    _split_excess_waits(nc)
    return nc


_NC = None


def _get_nc():
    global _NC
    if _NC is None:
        _NC = _build_nc()
    return _NC


def _q8(x, scale, limit=230.0):
    import ml_dtypes
    y = np.asarray(x, np.float32) * scale
    m = np.abs(y).max()
    assert m < limit, f"fp8 range exceeded: {m} * (scale {scale})"
    return y.astype(ml_dtypes.float8_e4m3)


def _prep_inputs(text, feats, t_w1, t_b1, t_w2, t_b2,
                 f_w1, f_b1, f_w2, f_b2, f_w3, f_b3):
    for bias in (t_b1, t_b2, f_b1, f_b2, f_b3):
        assert not np.asarray(bias).any(), "kernel assumes zero biases (per spec)"
    c = np.ascontiguousarray
    f4 = np.float32

    textT = _q8(c(np.asarray(text, f4).transpose(0, 2, 1)), SX)    # [B,256,512]
    featsT = _q8(c(np.asarray(feats, f4).transpose(0, 2, 1)), SX)  # [B,80,2048]

    # wblob [128, WBLOB_W] fp8: per lhsT (k/m) block of [p, 2, 128]
    blob = np.zeros((128, WBLOB_W), np.float32)

    def put(name, idx, arr):  # arr [128, 2, 128] f32 (pre-scale applied)
        base = WOFF[name] + idx * 256
        blob[:, base : base + 256] = arr.reshape(128, 256)

    tw1 = np.asarray(t_w1, f4).transpose(2, 1, 0)  # [3, cin, cout]
    for k in range(3):
        for m in range(2):
            a = tw1[k].reshape(2, 128, 256)[:, :, m * 128 : (m + 1) * 128]
            put("tw1", k * 2 + m, a.transpose(1, 0, 2) * S_TW1)
    tw2 = np.asarray(t_w2, f4)[:, :, 0].T  # [cin, cout]
    for m in range(2):
        a = tw2.reshape(2, 128, 256)[:, :, m * 128 : (m + 1) * 128]
        put("tw2", m, a.transpose(1, 0, 2) * S_TW2)
    fw1 = np.asarray(f_w1, f4).transpose(2, 1, 0)  # [3, 80, 256]
    for d in range(2):
        for m in range(2):
            a = np.zeros((128, 2, 128), np.float32)
            a[:80, 0] = fw1[2 * d][:, m * 128 : (m + 1) * 128]
            if 2 * d + 1 < 3:
                a[:80, 1] = fw1[2 * d + 1][:, m * 128 : (m + 1) * 128]
            put("fw1", d * 2 + m, a * S_FW1)
    fw2 = np.asarray(f_w2, f4).transpose(2, 1, 0)
    for k in range(3):
        for m in range(2):
            a = fw2[k].reshape(2, 128, 256)[:, :, m * 128 : (m + 1) * 128]
            put("fw2", k * 2 + m, a.transpose(1, 0, 2) * S_FW2)
    W3 = np.asarray(f_w3, f4)[:, :, 0]  # [cout, cin]
    G = (W3.T @ W3).astype(np.float32)
    for m in range(2):
        a = G.reshape(2, 128, 256)[:, :, m * 128 : (m + 1) * 128]
        put("G", m, a.transpose(1, 0, 2) * S_G)
    for m in range(2):  # w3u lhsT[c, d]: W3 itself
        a = W3.reshape(2, 128, 256)[:, :, m * 128 : (m + 1) * 128]
        put("w3u", m, a.transpose(1, 0, 2) * S_W3)

    m = np.abs(blob).max()
    assert m < 230.0, f"wblob fp8 range exceeded: {m}"
    import ml_dtypes
    blob8 = blob.astype(ml_dtypes.float8_e4m3)

    import ml_dtypes as _mld
    ones8 = np.ones((1, TF), _mld.float8_e4m3)
    shared = {
        "wblob": blob8.view(np.uint8),
        "onesrow": ones8.view(np.uint8),
        "prior": _beta_binomial_prior().astype(np.float16),
    }
    in_maps = []
    for core in range(N_CORES):
        mcore = dict(shared)
        mcore["textT"] = c(textT[core * B_LOC : (core + 1) * B_LOC]).view(np.uint8)
        mcore["featsT"] = c(featsT[core * B_LOC : (core + 1) * B_LOC]).view(np.uint8)
        in_maps.append(mcore)
    return in_maps


_CALLABLE = None


def _build_callable():
    """Compile once; return fn(in_maps) -> per-core output dicts (axon path)."""
    import jax
    import jax.numpy as jnp
    from jax.sharding import Mesh, NamedSharding, PartitionSpec
    from jax.experimental.shard_map import shard_map
    from concourse.bass2jax import (
        _bass_exec_p,
        install_neuronx_cc_hook,
        partition_id_tensor,
    )

    nc = _get_nc()
    install_neuronx_cc_hook()
    partition_name = nc.partition_id_tensor.name if nc.partition_id_tensor else None
    in_names, out_names, out_avals, zero_shapes = [], [], [], []
    for alloc in nc.m.functions[0].allocations:
        if not isinstance(alloc, mybir.MemoryLocationSet):
            continue
        name = alloc.memorylocations[0].name
        if alloc.kind == "ExternalInput":
            if name != partition_name:
                in_names.append(name)
        elif alloc.kind == "ExternalOutput":
            shape = tuple(alloc.tensor_shape)
            dtype = mybir.dt.np(alloc.dtype)
            out_names.append(name)
            out_avals.append(jax.core.ShapedArray(shape, dtype))
            zero_shapes.append(((N_CORES * shape[0],) + shape[1:], dtype))
    n_params = len(in_names)
    n_outs = len(out_avals)
    all_in_names = list(in_names) + out_names
    if partition_name is not None:
        all_in_names.append(partition_name)
    donate = tuple(range(n_params, n_params + n_outs))

    def _body(*args):
        operands = list(args)
        if partition_name is not None:
            operands.append(partition_id_tensor())
        outs = _bass_exec_p.bind(
            *operands,
            out_avals=tuple(out_avals),
            in_names=tuple(all_in_names),
            out_names=tuple(out_names),
            lowering_input_output_aliases=(),
            sim_require_finite=True,
            sim_require_nnan=True,
            nc=nc,
        )
        return tuple(outs)

    devices = jax.devices()[:N_CORES]
    mesh = Mesh(np.asarray(devices), ("core",))
    fn = jax.jit(
        shard_map(
            _body,
            mesh=mesh,
            in_specs=(PartitionSpec("core"),) * (n_params + n_outs),
            out_specs=(PartitionSpec("core"),) * n_outs,
            check_rep=False,
        ),
        donate_argnums=donate,
        keep_unused=True,
    )
    sharding = NamedSharding(mesh, PartitionSpec("core"))
    zfn = jax.jit(
        lambda: tuple(jnp.zeros(s, d) for s, d in zero_shapes),
        out_shardings=tuple(sharding for _ in zero_shapes),
    )

    def call(in_maps):
        concat_in = [
            np.concatenate([np.asarray(in_maps[c][n]) for c in range(N_CORES)], axis=0)
            for n in in_names
        ]
        out_arrs = fn(*concat_in, *zfn())
        return [
            {
                name: np.asarray(out_arrs[i]).reshape(
                    N_CORES, *out_avals[i].shape
                )[c]
                for i, name in enumerate(out_names)
            }
            for c in range(N_CORES)
        ]

    return call


def _run(inputs, **kw):
    global _CALLABLE
    import time as _time

    in_maps = _prep_inputs(
        inputs["text"], inputs["feats"],
        inputs["t_w1"], inputs["t_b1"], inputs["t_w2"], inputs["t_b2"],
        inputs["f_w1"], inputs["f_b1"], inputs["f_w2"], inputs["f_b2"],
        inputs["f_w3"], inputs["f_b3"],
    )
    results = None
    last_err = None
    if _CALLABLE is not False:
        for attempt in range(3):
            try:
                if _CALLABLE is None:
                    from concourse._compat import axon_active

                    if not axon_active():
                        raise RuntimeError("axon not active; use native path")
                    _CALLABLE = _build_callable()
                results = _CALLABLE(in_maps)
                break
            except Exception as e:
                last_err = e
                results = None
                if attempt < 2:
                    _time.sleep(20 * (attempt + 1))
        if results is None:
            _CALLABLE = False
    if results is None:
        from concourse.bass_utils import run_bass_kernel_spmd

        for attempt in range(3):
            try:
                results = run_bass_kernel_spmd(
                    _get_nc(), in_maps, core_ids=list(range(N_CORES))
                ).results
                break
            except Exception as e:
                last_err = e
                results = None
                if attempt < 2:
                    _time.sleep(20 * (attempt + 1))
    if results is None:
        raise last_err
    out = np.concatenate([r["out"] for r in results], axis=0)
    return out, results


def kernel(**inputs) -> np.ndarray:
    out, _ = _run(inputs)
    return out
